# revision 1
# baseline (speedup 1.0000x reference)
"""Trainium2 Bass kernel for nn_AnyTSRpp (sparse_attention).

Strategy: pure data-parallel over the HR pixel grid (65536 px/batch),
8192 px/batch/core on 8 NeuronCores. Host prepares a padded 2x2-patch
gather table + per-pixel indices/scalars; device does the feature
gather (per-tile indirect DMA, pixel-major), PE transposes to
channel-major, all matmuls/relu/softmax/gelu, and a tiny AllReduce
for the global attention logits (contraction over all pixels).
off_t = attn_t @ v_t is folded as (W00_off_t @ attn_t) @ v_t so the
attention output is never materialized.

Self-contained: hardcodes all shapes. kernel(**inputs) -> np.ndarray.
"""

import functools
import numpy as np
import ml_dtypes

BF16 = ml_dtypes.bfloat16

NCORES = 8
B = 2
C = 64
HLR = WLR = 64
HQ = WQ = 256
NPB = HQ * WQ            # 65536 pixels per batch
NLOC = NPB // NCORES     # 8192 pixels per batch per core
TD = 66                  # padded base-index grid dim (0..65)
NTAB = TD * TD           # 4356 table rows
ELEM = 256               # bf16 elements per table row (4 corners x 64ch)
CHUNK = 512              # matmul moving-N chunk
NCHUNK = NLOC // CHUNK   # 16
PCH = 1024               # MLP pixel super-chunk
EPS = np.float32(1e-6)


# --------------------------------------------------------------------------
# host-side math (mirrors reference semantics in f32)
# --------------------------------------------------------------------------

def _corner_indices(co):
    """co: [N] f32 coords in one axis. Returns (base j in [0,65], iy_minus,
    iy_plus) exactly matching the reference's per-corner nearest indices."""
    # reference: c_t = clip(co + v/64 + eps, -1+1e-6, 1-1e-6);
    #            i_t = clip(round((c_t+1)*32 - 0.5), 0, 63)
    out = []
    for v in (-1.0, 1.0):
        c = np.clip(co + np.float32(v / 64.0) + EPS,
                    np.float32(-1 + 1e-6), np.float32(1 - 1e-6))
        i = np.clip(np.round((c + 1) * np.float32(32.0) - np.float32(0.5)),
                    0, 63).astype(np.int32)
        out.append(i)
    im, ip = out
    # padded-table base: j = clip(floor(ay), -1, 64) + 1 where ay = 32*(co+eps)+31.5
    ay = (co + EPS) * np.float32(32.0) + np.float32(31.5)
    j = np.clip(np.floor(ay), -1, 64).astype(np.int32) + 1
    return j, im, ip


def _wrap16(a):
    """[N] -> [16, N//16] with element n at [n%16, n//16] (dma_gather layout)."""
    return np.ascontiguousarray(a.reshape(-1, 16).T)


def _host_prep(inputs):
    feat = np.asarray(inputs['feat'], np.float32)
    inp = np.asarray(inputs['inp'], np.float32)
    coord = np.asarray(inputs['coord'], np.float32)
    cell = np.asarray(inputs['cell'], np.float32)
    scale = np.asarray(inputs['scale'], np.float32)
    Wq = np.asarray(inputs['Wq'], np.float32); bq = np.asarray(inputs['bq'], np.float32)
    Wk = np.asarray(inputs['Wk'], np.float32); bk = np.asarray(inputs['bk'], np.float32)
    Wv = np.asarray(inputs['Wv'], np.float32); bv = np.asarray(inputs['bv'], np.float32)
    W00 = np.asarray(inputs['W00'], np.float32); b00 = np.asarray(inputs['b00'], np.float32)
    W1 = np.asarray(inputs['W1'], np.float32); b1 = np.asarray(inputs['b1'], np.float32)
    W2 = np.asarray(inputs['W2'], np.float32); b2 = np.asarray(inputs['b2'], np.float32)
    ls = np.asarray(inputs['ls'], np.float32)

    # ---- gather table: padded 67x67 edge-replicated, 2x2 corner patches ----
    # P67[jy, jx] = feat[:, clip(jy-1,0,63), clip(jx-1,0,63)]
    pad_idx = np.clip(np.arange(-1, 66), 0, 63)
    tables = np.empty((B, NTAB, ELEM), dtype=BF16)
    for b in range(B):
        P = feat[b][:, pad_idx][:, :, pad_idx]          # [64, 67, 67]
        Pb = P.astype(BF16)
        c00 = Pb[:, 0:66, 0:66]; c01 = Pb[:, 0:66, 1:67]
        c10 = Pb[:, 1:67, 0:66]; c11 = Pb[:, 1:67, 1:67]
        # row s=(jy*66+jx): [c00(64) | c01(64) | c10(64) | c11(64)]
        row = np.concatenate([c00.reshape(64, -1), c01.reshape(64, -1),
                              c10.reshape(64, -1), c11.reshape(64, -1)], axis=0)
        tables[b] = row.T.reshape(NTAB, ELEM)

    coord_y = coord[..., 0].reshape(B, NPB)
    coord_x = coord[..., 1].reshape(B, NPB)

    # per-(b) base indices + per-corner old/rel/weight
    idx_all = np.empty((B, NPB), np.int32)
    rel_all = np.empty((B, 4, 3, NPB), BF16)   # [rel_y, rel_x, ones]
    w_all = np.empty((B, 4, NPB), np.float32)
    hw = np.float32(64.0)
    ls2 = ls[0] * ls[0]
    for b in range(B):
        jy, iym, iyp = _corner_indices(coord_y[b])
        jx, ixm, ixp = _corner_indices(coord_x[b])
        idx_all[b] = (jy * TD + jx).astype(np.int32)
        iy = {-1: iym, 1: iyp}
        ix = {-1: ixm, 1: ixp}
        t = 0
        for vx in (-1, 1):          # y offset
            for vy in (-1, 1):      # x offset
                oy = (iy[vx].astype(np.float32) + np.float32(0.5)) / np.float32(32.0) - 1
                ox = (ix[vy].astype(np.float32) + np.float32(0.5)) / np.float32(32.0) - 1
                ry = coord_y[b] - oy
                rx = coord_x[b] - ox
                rel_all[b, t, 0] = ry.astype(BF16)
                rel_all[b, t, 1] = rx.astype(BF16)
                rel_all[b, t, 2] = np.float32(1.0)
                rd = (ry * hw) ** 2 + (rx * hw) ** 2
                w_all[b, t] = np.exp(rd / ls2 * np.float32(-0.5))
                t += 1

    wsm = w_all.astype(BF16)        # [B, 4, NPB] — replicated on device

    # ---- bilinear sample of inp (border, align_corners=False) + b2 ----
    bil = np.empty((B, NPB), np.float32)
    for b in range(B):
        im = inp[b, 0]
        y = np.clip((coord_y[b] + 1) * np.float32(32.0) - np.float32(0.5), 0.0, 63.0)
        x = np.clip((coord_x[b] + 1) * np.float32(32.0) - np.float32(0.5), 0.0, 63.0)
        y0 = np.floor(y); x0 = np.floor(x)
        wy = (y - y0).astype(np.float32); wx = (x - x0).astype(np.float32)
        y0i = np.clip(y0.astype(np.int32), 0, 63)
        y1i = np.clip(y0.astype(np.int32) + 1, 0, 63)
        x0i = np.clip(x0.astype(np.int32), 0, 63)
        x1i = np.clip(x0.astype(np.int32) + 1, 0, 63)
        v00 = im[y0i, x0i]; v01 = im[y0i, x1i]
        v10 = im[y1i, x0i]; v11 = im[y1i, x1i]
        bil[b] = (v00 * (1 - wy) * (1 - wx) + v01 * (1 - wy) * wx
                  + v10 * wy * (1 - wx) + v11 * wy * wx) + b2[0]

    # ---- weight repacks ----
    wq_rhs = np.concatenate([Wq.T, bq[None, :]], axis=0).astype(BF16)       # [3, 64]
    wk_rhs = np.concatenate([Wk.T, bk[None, :]], axis=0).astype(BF16)       # [65, 64]
    wv_lhsT = Wv.T.astype(BF16)                                             # [64, 64]
    w00off_rhs = np.stack([W00[:, t * 64:(t + 1) * 64].T for t in range(4)]
                          ).astype(BF16)                                    # [4, 64, 256]
    w00fs_lhsT = np.stack(
        [np.concatenate([W00[:, 256 + t * 64: 256 + (t + 1) * 64].T,
                         np.zeros((1, 256), np.float32)], axis=0)
         for t in range(4)]).astype(BF16)                                   # [4, 65, 256]
    b00eff = np.empty((B, 1, 256), BF16)
    for b in range(B):
        vec4 = np.concatenate([cell[b] * hw, scale[b]]).astype(np.float32)
        b00eff[b, 0] = (b00 + W00[:, 512:516] @ vec4).astype(BF16)
    w1_lhsT = np.ascontiguousarray(W1.T.astype(BF16).reshape(2, 128, 256))  # [2, 128, 256]
    w2_lhsT = np.ascontiguousarray(W2.T.astype(BF16).reshape(2, 128, 1))    # [2, 128, 1]

    # ---- shard per core ----
    in_maps = []
    for cidx in range(NCORES):
        sl = slice(cidx * NLOC, (cidx + 1) * NLOC)
        # idx2d[b, p, j] = base index of local pixel j*128+p (pixel-major tiles)
        idx2d = np.ascontiguousarray(
            idx_all[:, sl].reshape(B, 64, 128).transpose(0, 2, 1))
        m = {
            'table0': tables[0], 'table1': tables[1],
            'idx': idx2d,
            'wsm': np.ascontiguousarray(wsm[:, :, sl]),
            'relq': np.ascontiguousarray(rel_all[:, :, :, sl]),
            'bil': np.ascontiguousarray(bil[:, sl]),
            'wq_rhs': wq_rhs, 'wk_rhs': wk_rhs, 'wv_lhsT': wv_lhsT,
            'bv': bv.reshape(64, 1).astype(np.float32),
            'w00off_rhs': w00off_rhs, 'w00fs_lhsT': w00fs_lhsT,
            'b00eff': b00eff,
            'w1_lhsT': w1_lhsT,
            'b1': np.ascontiguousarray(b1.astype(np.float32).reshape(2, 128, 1)),
            'w2_lhsT': w2_lhsT,
        }
        in_maps.append(m)
    return in_maps


# --------------------------------------------------------------------------
# device kernel
# --------------------------------------------------------------------------

@functools.lru_cache(maxsize=1)
def _build():
    import concourse.bass as bass
    import concourse.tile as tile
    from concourse import bacc, mybir
    dt = mybir.dt
    F32, BF, I16 = dt.float32, dt.bfloat16, dt.int16
    AF = mybir.ActivationFunctionType
    ALU = mybir.AluOpType

    nc = bacc.Bacc(None, target_bir_lowering=False)

    tables_d = [nc.dram_tensor(f'table{_b}', [NTAB, ELEM], BF, kind='ExternalInput')
                for _b in range(B)]
    idx = nc.dram_tensor('idx', [B, 128, 64], dt.int32, kind='ExternalInput')
    wsm = nc.dram_tensor('wsm', [B, 4, NLOC], BF, kind='ExternalInput')
    relq = nc.dram_tensor('relq', [B, 4, 3, NLOC], BF, kind='ExternalInput')
    bil = nc.dram_tensor('bil', [B, NLOC], F32, kind='ExternalInput')
    wq_rhs = nc.dram_tensor('wq_rhs', [3, 64], BF, kind='ExternalInput')
    wk_rhs = nc.dram_tensor('wk_rhs', [65, 64], BF, kind='ExternalInput')
    wv_lhsT = nc.dram_tensor('wv_lhsT', [64, 64], BF, kind='ExternalInput')
    bv = nc.dram_tensor('bv', [64, 1], F32, kind='ExternalInput')
    w00off_rhs = nc.dram_tensor('w00off_rhs', [4, 64, 256], BF, kind='ExternalInput')
    w00fs_lhsT = nc.dram_tensor('w00fs_lhsT', [4, 65, 256], BF, kind='ExternalInput')
    b00eff = nc.dram_tensor('b00eff', [B, 1, 256], BF, kind='ExternalInput')
    w1_lhsT = nc.dram_tensor('w1_lhsT', [2, 128, 256], BF, kind='ExternalInput')
    b1 = nc.dram_tensor('b1', [2, 128, 1], F32, kind='ExternalInput')
    w2_lhsT = nc.dram_tensor('w2_lhsT', [2, 128, 1], BF, kind='ExternalInput')
    out = nc.dram_tensor('out', [B, NLOC], F32, kind='ExternalOutput')

    NU = B * 4  # 8 attention units

    with tile.TileContext(nc) as tc:
        with (
            tc.tile_pool(name='const', bufs=1) as constp,
            tc.tile_pool(name='fs', bufs=1) as fsp,
            tc.tile_pool(name='gat', bufs=1) as gatp,
            tc.tile_pool(name='wr', bufs=1) as wrp,
            tc.tile_pool(name='qk', bufs=1) as qkp,
            tc.tile_pool(name='rel', bufs=1) as relp,
            tc.tile_pool(name='v', bufs=1) as vp,
            tc.tile_pool(name='mlp', bufs=1) as mlpp,
            tc.tile_pool(name='small', bufs=1) as smallp,
            tc.tile_pool(name='ps', bufs=1, space='PSUM') as psp,
            tc.tile_pool(name='psx', bufs=1, space='PSUM') as psxp,
            tc.tile_pool(name='dram', bufs=1, space='DRAM') as dramp,
        ):
            # ---- constant weights to SBUF ----
            wq_sb = constp.tile([3, 64], BF)
            wk_sb = constp.tile([65, 64], BF)
            wv_sb = constp.tile([64, 64], BF)
            bv_sb = constp.tile([64, 1], F32)
            w00o_sb = constp.tile([64, 4 * 256], BF)
            w00f_sb = constp.tile([65, 4 * 256], BF)
            w1_sb = constp.tile([128, 2, 256], BF)
            b1_sb = constp.tile([128, 2], F32)
            w2_sb = constp.tile([128, 2], BF)
            nc.sync.dma_start(out=wq_sb[:], in_=wq_rhs[:, :])
            nc.sync.dma_start(out=wk_sb[:], in_=wk_rhs[:, :])
            nc.sync.dma_start(out=wv_sb[:], in_=wv_lhsT[:, :])
            nc.sync.dma_start(out=bv_sb[:], in_=bv[:, :])
            for t in range(4):
                nc.sync.dma_start(out=w00o_sb[:, t * 256:(t + 1) * 256],
                                  in_=w00off_rhs[t, :, :])
                nc.sync.dma_start(out=w00f_sb[:, t * 256:(t + 1) * 256],
                                  in_=w00fs_lhsT[t, :, :])
            for kk in range(2):
                nc.sync.dma_start(out=w1_sb[:, kk, :], in_=w1_lhsT[kk, :, :])
                nc.sync.dma_start(out=b1_sb[:, kk:kk + 1], in_=b1[kk, :, :])
                nc.sync.dma_start(out=w2_sb[:, kk:kk + 1], in_=w2_lhsT[kk, :, :])

            Sp_sb = constp.tile([64, NU * 64], F32)   # partial logits, all units


            # =========== phases 1+2 per batch: gather, fs, q/k, S ===========
            from concourse.masks import make_identity
            ident_sb = constp.tile([128, 128], BF)
            make_identity(nc, ident_sb[:])
            ones_col = constp.tile([1, 64], BF)
            nc.vector.memset(ones_col[:], 1.0)

            def replicate_w(b, t, wdst):
                for q in range(4):
                    wrow = wrp.tile([1, NLOC // 4], BF, name='wrow')
                    nc.sync.dma_start(out=wrow[:],
                                      in_=wsm[b, t, q * (NLOC // 4):(q + 1) * (NLOC // 4)][None, :])
                    for g in range(4):
                        gsl = slice((q * 4 + g) * 512, (q * 4 + g + 1) * 512)
                        r_full = psp.tile([64, 512], F32, name='misc_ps')
                        nc.tensor.matmul(out=r_full[:], lhsT=ones_col[:],
                                         rhs=wrow[:, g * 512:(g + 1) * 512],
                                         start=True, stop=True)
                        nc.scalar.copy(out=wdst[:, gsl], in_=r_full[:])

            def gather_fs(b, fs_tiles):
                idx_sb = gatp.tile([128, 64], dt.int32)
                nc.sync.dma_start(out=idx_sb[:], in_=idx[b, :, :])
                g_pm = gatp.tile([128, 64, ELEM], BF)
                for j in range(64):
                    nc.gpsimd.indirect_dma_start(
                        out=g_pm[:, j, :], out_offset=None,
                        in_=tables_d[b][:, :],
                        in_offset=bass.IndirectOffsetOnAxis(
                            ap=idx_sb[:, j:j + 1], axis=0))
                for cpair in range(2):
                    te, to = fs_tiles[2 * cpair], fs_tiles[2 * cpair + 1]
                    we = wrp.tile([64, NLOC], BF, name='we')
                    replicate_w(b, 2 * cpair, we)
                    hi_stage = wrp.tile([128, NLOC], BF, name='hi_stage')
                    for jg in range(16):
                        tp_ps = psp.tile([128, 512], BF)
                        for jj in range(4):
                            j = jg * 4 + jj
                            nc.tensor.transpose(
                                out=tp_ps[:, jj * 128:(jj + 1) * 128],
                                in_=g_pm[:, j, cpair * 128:(cpair + 1) * 128],
                                identity=ident_sb[:])
                        gsl = slice(jg * 512, (jg + 1) * 512)
                        nc.vector.tensor_tensor(out=te[0:64, gsl], in0=tp_ps[0:64, :],
                                                in1=we[:, gsl], op=ALU.mult)
                        nc.scalar.copy(out=hi_stage[64:128, gsl], in_=tp_ps[64:128, :])
                    # move t-odd channels down to partitions 0-63, then scale
                    nc.sync.dma_start(out=to[0:64, :], in_=hi_stage[64:128, :])
                    wo = wrp.tile([64, NLOC], BF, name='we')
                    replicate_w(b, 2 * cpair + 1, wo)
                    nc.vector.tensor_tensor(out=to[0:64, :], in0=to[0:64, :],
                                            in1=wo[:], op=ALU.mult)
                for _t in range(4):
                    nc.vector.memset(fs_tiles[_t][64:65, :], 1.0)

            fs_spill = [[dramp.tile([65, NLOC], BF, name=f'fsspill{_b}_{_t}')
                         for _t in range(4)] for _b in range(B)]
            for b in range(B):
                fs_tiles = [fsp.tile([65, NLOC], BF, name=f'fs{_t}') for _t in range(4)]
                gather_fs(b, fs_tiles)

                for t in range(4):
                    rel_sb = relp.tile([3, NLOC], BF)
                    nc.sync.dma_start(out=rel_sb[:], in_=relq[b, t, :, :])
                    qT_sb = qkp.tile([128, 64 * 64], BF)
                    kT_sb = qkp.tile([128, 64 * 64], BF)
                    s_ps = psp.tile([64, 64], F32, name='s_ps')
                    for jg in range(8):          # groups of 8 pixel-tiles
                        q_ps = psp.tile([128, 512], F32)
                        k_ps = psp.tile([128, 512], F32)
                        for jj in range(8):
                            j = jg * 8 + jj
                            nc.tensor.matmul(
                                out=q_ps[:, jj * 64:(jj + 1) * 64],
                                lhsT=rel_sb[:, j * 128:(j + 1) * 128],
                                rhs=wq_sb[:], start=True, stop=True)
                            nc.tensor.matmul(
                                out=k_ps[:, jj * 64:(jj + 1) * 64],
                                lhsT=fs_tiles[t][:, j * 128:(j + 1) * 128],
                                rhs=wk_sb[:], start=True, stop=True)
                        gsl = slice(jg * 512, (jg + 1) * 512)
                        nc.scalar.activation(out=qT_sb[:, gsl], in_=q_ps[:], func=AF.Relu)
                        nc.vector.tensor_scalar_max(out=kT_sb[:, gsl], in0=k_ps[:], scalar1=0.0)
                    for j in range(64):
                        nc.tensor.matmul(
                            out=s_ps[:],
                            lhsT=qT_sb[:, j * 64:(j + 1) * 64],
                            rhs=kT_sb[:, j * 64:(j + 1) * 64],
                            start=(j == 0), stop=(j == 63))
                    u = b * 4 + t
                    nc.vector.tensor_copy(out=Sp_sb[:, u * 64:(u + 1) * 64], in_=s_ps[:])
                for t in range(4):
                    nc.sync.dma_start(out=fs_spill[b][t][:, :], in_=fs_tiles[t][:])

            # =========== phase 3: AllReduce of logits ===========
            cc_in = dramp.tile([64, NU * 64], F32)
            cc_out = dramp.tile([64, NU * 64], F32)
            nc.gpsimd.dma_start(out=cc_in[:], in_=Sp_sb[:])
            nc.gpsimd.collective_compute(
                'AllReduce', mybir.AluOpType.add,
                replica_groups=[list(range(NCORES))],
                ins=[cc_in.opt()], outs=[cc_out.opt()],
            )
            S_sb = constp.tile([64, NU * 64], F32)
            nc.gpsimd.dma_start(out=S_sb[:], in_=cc_out[:])

            # =========== phase 4: softmax + A_t^T ===========
            attn_sb = constp.tile([64, NU * 64], BF)
            AT_tiles = []
            for u in range(NU):
                usl = slice(u * 64, (u + 1) * 64)
                mx = smallp.tile([64, 1], F32)
                nmx = smallp.tile([64, 1], F32)
                ex = smallp.tile([64, 64], F32)
                sm = smallp.tile([64, 1], F32)
                rs = smallp.tile([64, 1], F32)
                nc.vector.tensor_reduce(out=mx[:], in_=S_sb[:, usl],
                                        axis=mybir.AxisListType.X, op=ALU.max)
                nc.vector.tensor_scalar_mul(out=nmx[:], in0=mx[:], scalar1=-1.0)
                nc.scalar.activation(out=ex[:], in_=S_sb[:, usl], func=AF.Exp,
                                     bias=nmx[:, 0:1])
                nc.vector.tensor_reduce(out=sm[:], in_=ex[:],
                                        axis=mybir.AxisListType.X, op=ALU.add)
                nc.vector.reciprocal(out=rs[:], in_=sm[:])
                nc.vector.tensor_scalar_mul(out=attn_sb[:, usl], in0=ex[:],
                                            scalar1=rs[:, 0:1])
            for b in range(B):
                for t in range(4):
                    u = b * 4 + t
                    a_full = psp.tile([64, 512], F32, name='misc_ps')
                    a_ps = a_full[:, 0:256]
                    nc.tensor.matmul(out=a_ps,
                                     lhsT=attn_sb[:, u * 64:(u + 1) * 64],
                                     rhs=w00o_sb[:, t * 256:(t + 1) * 256],
                                     start=True, stop=True)
                    at = constp.tile([65, 256], BF, name=f'at{b}_{t}')
                    nc.vector.tensor_copy(out=at[0:64, :], in_=a_ps)
                    if t == 0:
                        nc.sync.dma_start(out=at[64:65, :], in_=b00eff[b, :, :])
                    AT_tiles.append(at)

            # =========== phase 5: regather + MLP ===========
            for b in range(B):
                fs_tiles = [fsp.tile([65, NLOC], BF, name=f'fs{_t}') for _t in range(4)]
                for t in range(4):
                    nc.sync.dma_start(out=fs_tiles[t][:], in_=fs_spill[b][t][:, :])

                for pc in range(NLOC // PCH):
                    psl = slice(pc * PCH, (pc + 1) * PCH)
                    # transient v tiles for this pixel super-chunk
                    v_tiles = []
                    for t in range(4):
                        vt = vp.tile([65, PCH], BF, name=f'vt{t}')
                        nc.vector.memset(vt[64:65, :], 1.0)
                        for cc in range(PCH // CHUNK):
                            vsl_l = slice(cc * CHUNK, (cc + 1) * CHUNK)
                            vsl_g = slice(pc * PCH + cc * CHUNK, pc * PCH + (cc + 1) * CHUNK)
                            v_ps = psp.tile([64, CHUNK], F32)
                            nc.tensor.matmul(out=v_ps[:], lhsT=wv_sb[:],
                                             rhs=fs_tiles[t][0:64, vsl_g],
                                             start=True, stop=True)
                            nc.scalar.activation(out=vt[0:64, vsl_l], in_=v_ps[:],
                                                 func=AF.Relu, bias=bv_sb[:, 0:1])
                        v_tiles.append(vt)

                    x1_t = [mlpp.tile([128, PCH], BF, name=f'x1_{_m}') for _m in range(2)]
                    x2_t = [mlpp.tile([128, PCH], BF, name=f'x2_{_m}') for _m in range(2)]
                    for cc in range(PCH // CHUNK):
                        lsl = slice(cc * CHUNK, (cc + 1) * CHUNK)
                        gsl = slice(pc * PCH + cc * CHUNK, pc * PCH + (cc + 1) * CHUNK)
                        for m in range(2):
                            msl = slice(m * 128, (m + 1) * 128)
                            x_ps = psxp.tile([128, CHUNK], F32)
                            for t in range(4):
                                nc.tensor.matmul(
                                    out=x_ps[:],
                                    lhsT=w00f_sb[:, t * 256 + m * 128: t * 256 + (m + 1) * 128],
                                    rhs=fs_tiles[t][:, gsl],
                                    start=(t == 0), stop=False)
                            for t in range(4):
                                at = AT_tiles[b * 4 + t]
                                kk = 65 if t == 0 else 64
                                nc.tensor.matmul(
                                    out=x_ps[:],
                                    lhsT=at[0:kk, msl],
                                    rhs=v_tiles[t][0:kk, lsl],
                                    start=False, stop=(t == 3))
                            nc.vector.tensor_copy(out=x1_t[m][:, lsl], in_=x_ps[:])
                        # W1 + gelu
                        for m in range(2):
                            msl = slice(m * 128, (m + 1) * 128)
                            x2_ps = psxp.tile([128, CHUNK], F32)
                            for kk in range(2):
                                nc.tensor.matmul(out=x2_ps[:],
                                                 lhsT=w1_sb[:, kk, msl],
                                                 rhs=x1_t[kk][:, lsl],
                                                 start=(kk == 0), stop=(kk == 1))
                            nc.scalar.activation(out=x2_t[m][:, lsl], in_=x2_ps[:],
                                                 func=AF.Gelu, bias=b1_sb[:, m:m + 1])
                        # W2 + bil add
                        o_full = psp.tile([64, 512], F32, name='misc_ps')
                        o_ps = o_full[0:1, :]
                        for kk in range(2):
                            nc.tensor.matmul(out=o_ps, lhsT=w2_sb[:, kk:kk + 1],
                                             rhs=x2_t[kk][:, lsl],
                                             start=(kk == 0), stop=(kk == 1))
                        bil_sb = smallp.tile([1, CHUNK], F32)
                        nc.sync.dma_start(out=bil_sb[:], in_=bil[b, gsl][None, :])
                        o_sb = smallp.tile([1, CHUNK], F32)
                        nc.vector.tensor_tensor(out=o_sb[:], in0=o_ps,
                                                in1=bil_sb[:], op=ALU.add)
                        nc.sync.dma_start(out=out[b, gsl][None, :], in_=o_sb[:])

    nc.compile()
    return nc


# --------------------------------------------------------------------------

def kernel(**inputs) -> np.ndarray:
    from concourse.bass_utils import run_bass_kernel_spmd
    in_maps = _host_prep(inputs)
    nc = _build()
    res = run_bass_kernel_spmd(nc, in_maps, core_ids=list(range(NCORES)))
    full = np.empty((B, 1, HQ, WQ), np.float32)
    flat = full.reshape(B, NPB)
    for cidx in range(NCORES):
        flat[:, cidx * NLOC:(cidx + 1) * NLOC] = res.results[cidx]['out']
    return full



# revision 8
# speedup vs baseline: 1.1938x; 1.1938x over previous
"""Trainium2 Bass kernel for nn_AnyTSRpp (sparse_attention).

Strategy: pure data-parallel over the HR pixel grid (65536 px/batch),
8192 px/batch/core on 8 NeuronCores. Host computes per-pixel corner
indices/scalars; device gathers feat rows directly (per-corner indirect
DMA, pixel-major), applies the RBF weight per-partition pre-transpose,
PE transposes to channel-major, runs all matmuls/relu/softmax/gelu, and
a tiny AllReduce for the global attention logits (contraction over all
pixels). off_t = attn_t @ v_t is folded as (W00_off_t @ attn_t) @ v_t
so the attention output is never materialized.

Self-contained: hardcodes all shapes. kernel(**inputs) -> np.ndarray.
"""

import functools
import numpy as np
import ml_dtypes

BF16 = ml_dtypes.bfloat16

NCORES = 8
B = 2
C = 64
HLR = WLR = 64
HQ = WQ = 256
NPB = HQ * WQ            # 65536 pixels per batch
NLOC = NPB // NCORES     # 8192 pixels per batch per core
NROW = HLR * WLR         # 4096 feat rows (y-major)
CHUNK = 512              # matmul moving-N chunk
NCHUNK = NLOC // CHUNK   # 16
PCH = 1024               # MLP pixel super-chunk
EPS = np.float32(1e-6)


# --------------------------------------------------------------------------
# host-side math (mirrors reference semantics in f32)
# --------------------------------------------------------------------------

def _corner_indices(co):
    """co: [N] f32 coords in one axis. Returns (iy_minus, iy_plus) exactly
    matching the reference's per-corner nearest indices."""
    # reference: c_t = clip(co + v/64 + eps, -1+1e-6, 1-1e-6);
    #            i_t = clip(round((c_t+1)*32 - 0.5), 0, 63)
    out = []
    for v in (-1.0, 1.0):
        c = np.clip(co + np.float32(v / 64.0) + EPS,
                    np.float32(-1 + 1e-6), np.float32(1 - 1e-6))
        i = np.clip(np.round((c + 1) * np.float32(32.0) - np.float32(0.5)),
                    0, 63).astype(np.int32)
        out.append(i)
    return out


def _host_prep(inputs):
    feat = np.asarray(inputs['feat'], np.float32)
    inp = np.asarray(inputs['inp'], np.float32)
    coord = np.asarray(inputs['coord'], np.float32)
    cell = np.asarray(inputs['cell'], np.float32)
    scale = np.asarray(inputs['scale'], np.float32)
    Wq = np.asarray(inputs['Wq'], np.float32); bq = np.asarray(inputs['bq'], np.float32)
    Wk = np.asarray(inputs['Wk'], np.float32); bk = np.asarray(inputs['bk'], np.float32)
    Wv = np.asarray(inputs['Wv'], np.float32); bv = np.asarray(inputs['bv'], np.float32)
    W00 = np.asarray(inputs['W00'], np.float32); b00 = np.asarray(inputs['b00'], np.float32)
    W1 = np.asarray(inputs['W1'], np.float32); b1 = np.asarray(inputs['b1'], np.float32)
    W2 = np.asarray(inputs['W2'], np.float32); b2 = np.asarray(inputs['b2'], np.float32)
    ls = np.asarray(inputs['ls'], np.float32)

    # feat as bf16 rows [B, 4096, 64]: row iy*64+ix = feat[b, :, iy, ix]
    featrows = np.ascontiguousarray(
        feat.transpose(0, 2, 3, 1).reshape(B, NROW, C)).astype(BF16)

    coord_y = coord[..., 0].reshape(B, NPB)
    coord_x = coord[..., 1].reshape(B, NPB)

    # per-(b, corner) gather indices + rel offsets + RBF weights
    idx_all = np.empty((B, 4, NPB), np.int32)
    rel_all = np.empty((B, 4, 2, NPB), BF16)   # [rel_y, rel_x]
    w_all = np.empty((B, 4, NPB), np.float32)
    hw = np.float32(64.0)
    ls2 = ls[0] * ls[0]
    for b in range(B):
        iym, iyp = _corner_indices(coord_y[b])
        ixm, ixp = _corner_indices(coord_x[b])
        iy = {-1: iym, 1: iyp}
        ix = {-1: ixm, 1: ixp}
        t = 0
        for vx in (-1, 1):          # y offset
            for vy in (-1, 1):      # x offset
                idx_all[b, t] = iy[vx] * np.int32(64) + ix[vy]
                oy = (iy[vx].astype(np.float32) + np.float32(0.5)) / np.float32(32.0) - 1
                ox = (ix[vy].astype(np.float32) + np.float32(0.5)) / np.float32(32.0) - 1
                ry = coord_y[b] - oy
                rx = coord_x[b] - ox
                rel_all[b, t, 0] = ry.astype(BF16)
                rel_all[b, t, 1] = rx.astype(BF16)
                rd = (ry * hw) ** 2 + (rx * hw) ** 2
                w_all[b, t] = np.exp(rd / ls2 * np.float32(-0.5))
                t += 1

    # ---- bilinear sample of inp (border, align_corners=False) + b2 ----
    bil = np.empty((B, NPB), np.float32)
    for b in range(B):
        im = inp[b, 0]
        y = np.clip((coord_y[b] + 1) * np.float32(32.0) - np.float32(0.5), 0.0, 63.0)
        x = np.clip((coord_x[b] + 1) * np.float32(32.0) - np.float32(0.5), 0.0, 63.0)
        y0 = np.floor(y); x0 = np.floor(x)
        wy = (y - y0).astype(np.float32); wx = (x - x0).astype(np.float32)
        y0i = np.clip(y0.astype(np.int32), 0, 63)
        y1i = np.clip(y0.astype(np.int32) + 1, 0, 63)
        x0i = np.clip(x0.astype(np.int32), 0, 63)
        x1i = np.clip(x0.astype(np.int32) + 1, 0, 63)
        v00 = im[y0i, x0i]; v01 = im[y0i, x1i]
        v10 = im[y1i, x0i]; v11 = im[y1i, x1i]
        bil[b] = (v00 * (1 - wy) * (1 - wx) + v01 * (1 - wy) * wx
                  + v10 * wy * (1 - wx) + v11 * wy * wx) + b2[0]

    # ---- weight repacks ----
    wq_rhs = np.concatenate([bq[None, :], Wq.T], axis=0).astype(BF16)       # [3, 64]
    wk_rhs = np.concatenate([Wk.T, bk[None, :]], axis=0).astype(BF16)       # [65, 64]
    wv_lhsT = Wv.T.astype(BF16)                                             # [64, 64]
    w00off_rhs = np.stack([W00[:, t * 64:(t + 1) * 64].T for t in range(4)]
                          ).astype(BF16)                                    # [4, 64, 256]
    w00fs_lhsT = np.stack(
        [np.concatenate([W00[:, 256 + t * 64: 256 + (t + 1) * 64].T,
                         np.zeros((1, 256), np.float32)], axis=0)
         for t in range(4)]).astype(BF16)                                   # [4, 65, 256]
    b00eff = np.empty((B, 1, 256), BF16)
    for b in range(B):
        vec4 = np.concatenate([cell[b] * hw, scale[b]]).astype(np.float32)
        b00eff[b, 0] = (b00 + W00[:, 512:516] @ vec4).astype(BF16)
    w1_lhsT = np.ascontiguousarray(W1.T.astype(BF16).reshape(2, 128, 256))  # [2, 128, 256]
    w2_lhsT = np.ascontiguousarray(W2.T.astype(BF16).reshape(2, 128, 1))    # [2, 128, 1]

    # ---- shard per core ----
    in_maps = []
    for cidx in range(NCORES):
        sl = slice(cidx * NLOC, (cidx + 1) * NLOC)
        # pixel-major tiles: local pixel j*128+p at [p, t*64+j]
        idx2d = np.ascontiguousarray(
            idx_all[:, :, sl].reshape(B, 4, 64, 128).transpose(0, 3, 1, 2)
            .reshape(B, 128, 4 * 64))
        wsm2d = np.ascontiguousarray(
            w_all[:, :, sl].reshape(B, 4, 64, 128).transpose(0, 3, 1, 2)
            .reshape(B, 128, 4 * 64))
        m = {
            'featrows0': featrows[0], 'featrows1': featrows[1],
            'idx': idx2d,
            'wsm': wsm2d,
            'relq': np.ascontiguousarray(rel_all[:, :, :, sl]),
            'bil': np.ascontiguousarray(bil[:, sl]),
            'wq_rhs': wq_rhs, 'wk_rhs': wk_rhs, 'wv_lhsT': wv_lhsT,
            'bv': bv.reshape(64, 1).astype(np.float32),
            'w00off_rhs': w00off_rhs, 'w00fs_lhsT': w00fs_lhsT,
            'b00eff': b00eff,
            'w1_lhsT': w1_lhsT,
            'b1': np.ascontiguousarray(b1.astype(np.float32).reshape(2, 128, 1)),
            'w2_lhsT': w2_lhsT,
        }
        in_maps.append(m)
    return in_maps


# --------------------------------------------------------------------------
# device kernel
# --------------------------------------------------------------------------

@functools.lru_cache(maxsize=1)
def _build():
    import concourse.bass as bass
    import concourse.tile as tile
    from concourse import bacc, mybir
    dt = mybir.dt
    F32, BF, I16 = dt.float32, dt.bfloat16, dt.int16
    AF = mybir.ActivationFunctionType
    ALU = mybir.AluOpType

    nc = bacc.Bacc(None, target_bir_lowering=False)

    featrows_d = [nc.dram_tensor(f'featrows{_b}', [NROW, C], BF, kind='ExternalInput')
                  for _b in range(B)]
    idx = nc.dram_tensor('idx', [B, 128, 4 * 64], dt.int32, kind='ExternalInput')
    wsm = nc.dram_tensor('wsm', [B, 128, 4 * 64], F32, kind='ExternalInput')
    relq = nc.dram_tensor('relq', [B, 4, 2, NLOC], BF, kind='ExternalInput')
    bil = nc.dram_tensor('bil', [B, NLOC], F32, kind='ExternalInput')
    wq_rhs = nc.dram_tensor('wq_rhs', [3, 64], BF, kind='ExternalInput')
    wk_rhs = nc.dram_tensor('wk_rhs', [65, 64], BF, kind='ExternalInput')
    wv_lhsT = nc.dram_tensor('wv_lhsT', [64, 64], BF, kind='ExternalInput')
    bv = nc.dram_tensor('bv', [64, 1], F32, kind='ExternalInput')
    w00off_rhs = nc.dram_tensor('w00off_rhs', [4, 64, 256], BF, kind='ExternalInput')
    w00fs_lhsT = nc.dram_tensor('w00fs_lhsT', [4, 65, 256], BF, kind='ExternalInput')
    b00eff = nc.dram_tensor('b00eff', [B, 1, 256], BF, kind='ExternalInput')
    w1_lhsT = nc.dram_tensor('w1_lhsT', [2, 128, 256], BF, kind='ExternalInput')
    b1 = nc.dram_tensor('b1', [2, 128, 1], F32, kind='ExternalInput')
    w2_lhsT = nc.dram_tensor('w2_lhsT', [2, 128, 1], BF, kind='ExternalInput')
    out = nc.dram_tensor('out', [B, NLOC], F32, kind='ExternalOutput')

    NU = B * 4  # 8 attention units

    with tile.TileContext(nc) as tc:
        with (
            tc.tile_pool(name='const', bufs=1) as constp,
            tc.tile_pool(name='fs', bufs=1) as fsp,
            tc.tile_pool(name='gat', bufs=1) as gatp,
            tc.tile_pool(name='qk', bufs=1) as qkp,
            tc.tile_pool(name='rel', bufs=1) as relp,
            tc.tile_pool(name='v', bufs=1) as vp,
            tc.tile_pool(name='mlp', bufs=1) as mlpp,
            tc.tile_pool(name='small', bufs=1) as smallp,
            tc.tile_pool(name='ps', bufs=1, space='PSUM') as psp,
            tc.tile_pool(name='psx', bufs=1, space='PSUM') as psxp,
            tc.tile_pool(name='dram', bufs=1, space='DRAM') as dramp,
        ):
            # ---- constant weights to SBUF ----
            wq_sb = constp.tile([3, 64], BF)
            wk_sb = constp.tile([65, 64], BF)
            wv_sb = constp.tile([64, 64], BF)
            bv_sb = constp.tile([64, 1], F32)
            w00o_sb = constp.tile([64, 4 * 256], BF)
            w00f_sb = constp.tile([65, 4 * 256], BF)
            w1_sb = constp.tile([128, 2, 256], BF)
            b1_sb = constp.tile([128, 2], F32)
            w2_sb = constp.tile([128, 2], BF)
            nc.sync.dma_start(out=wq_sb[:], in_=wq_rhs[:, :])
            nc.sync.dma_start(out=wk_sb[:], in_=wk_rhs[:, :])
            nc.sync.dma_start(out=wv_sb[:], in_=wv_lhsT[:, :])
            nc.sync.dma_start(out=bv_sb[:], in_=bv[:, :])
            for t in range(4):
                nc.sync.dma_start(out=w00o_sb[:, t * 256:(t + 1) * 256],
                                  in_=w00off_rhs[t, :, :])
                nc.sync.dma_start(out=w00f_sb[:, t * 256:(t + 1) * 256],
                                  in_=w00fs_lhsT[t, :, :])
            for kk in range(2):
                nc.sync.dma_start(out=w1_sb[:, kk, :], in_=w1_lhsT[kk, :, :])
                nc.sync.dma_start(out=b1_sb[:, kk:kk + 1], in_=b1[kk, :, :])
                nc.sync.dma_start(out=w2_sb[:, kk:kk + 1], in_=w2_lhsT[kk, :, :])

            Sp_sb = constp.tile([64, NU * 64], F32)   # partial logits, all units

            # =========== phases 1+2 per batch: gather, fs, q/k, S ===========
            from concourse.masks import make_identity
            ident_sb = constp.tile([128, 128], BF)
            make_identity(nc, ident_sb[:])

            def gather_fs(b, fs_tiles):
                idx_sb = gatp.tile([128, 4 * 64], dt.int32)
                wsm_sb = gatp.tile([128, 4 * 64], F32)
                nc.sync.dma_start(out=idx_sb[:], in_=idx[b, :, :])
                nc.sync.dma_start(out=wsm_sb[:], in_=wsm[b, :, :])
                for t in range(4):
                    g_pm = gatp.tile([128, 64, C], BF, name=f'g_pm{t % 2}')
                    for j in range(64):
                        nc.gpsimd.indirect_dma_start(
                            out=g_pm[:, j, :], out_offset=None,
                            in_=featrows_d[b][:, :],
                            in_offset=bass.IndirectOffsetOnAxis(
                                ap=idx_sb[:, t * 64 + j:t * 64 + j + 1], axis=0))
                    # RBF weight applied per-pixel (per-partition scalar)
                    for j in range(64):
                        nc.vector.tensor_scalar_mul(
                            out=g_pm[:, j, :], in0=g_pm[:, j, :],
                            scalar1=wsm_sb[:, t * 64 + j:t * 64 + j + 1])
                    for jg in range(16):
                        tp_ps = psp.tile([64, 512], BF)
                        for jj in range(4):
                            j = jg * 4 + jj
                            nc.tensor.transpose(
                                out=tp_ps[:, jj * 128:(jj + 1) * 128],
                                in_=g_pm[:, j, :],
                                identity=ident_sb[:])
                        nc.scalar.copy(out=fs_tiles[t][0:64, jg * 512:(jg + 1) * 512],
                                       in_=tp_ps[:])
                for _t in range(4):
                    nc.vector.memset(fs_tiles[_t][64:65, :], 1.0)

            fs_spill = [[dramp.tile([65, NLOC], BF, name=f'fsspill{_b}_{_t}')
                         for _t in range(4)] for _b in range(B)]
            for b in range(B):
                fs_tiles = [fsp.tile([65, NLOC], BF, name=f'fs{_t}') for _t in range(4)]
                gather_fs(b, fs_tiles)

                for t in range(4):
                    rel_sb = relp.tile([3, NLOC], BF)
                    nc.vector.memset(rel_sb[0:1, :], 1.0)
                    nc.sync.dma_start(out=rel_sb[1:3, :], in_=relq[b, t, :, :])
                    qT_sb = qkp.tile([128, 64 * 64], BF)
                    kT_sb = qkp.tile([128, 64 * 64], BF)
                    s_ps = psp.tile([64, 64], F32, name='s_ps')
                    for jg in range(8):          # groups of 8 pixel-tiles
                        q_ps = psp.tile([128, 512], F32)
                        k_ps = psp.tile([128, 512], F32)
                        for jj in range(8):
                            j = jg * 8 + jj
                            nc.tensor.matmul(
                                out=q_ps[:, jj * 64:(jj + 1) * 64],
                                lhsT=rel_sb[:, j * 128:(j + 1) * 128],
                                rhs=wq_sb[:], start=True, stop=True)
                            nc.tensor.matmul(
                                out=k_ps[:, jj * 64:(jj + 1) * 64],
                                lhsT=fs_tiles[t][:, j * 128:(j + 1) * 128],
                                rhs=wk_sb[:], start=True, stop=True)
                        gsl = slice(jg * 512, (jg + 1) * 512)
                        nc.scalar.activation(out=qT_sb[:, gsl], in_=q_ps[:], func=AF.Relu)
                        nc.vector.tensor_scalar_max(out=kT_sb[:, gsl], in0=k_ps[:], scalar1=0.0)
                    for j in range(64):
                        nc.tensor.matmul(
                            out=s_ps[:],
                            lhsT=qT_sb[:, j * 64:(j + 1) * 64],
                            rhs=kT_sb[:, j * 64:(j + 1) * 64],
                            start=(j == 0), stop=(j == 63))
                    u = b * 4 + t
                    nc.vector.tensor_copy(out=Sp_sb[:, u * 64:(u + 1) * 64], in_=s_ps[:])
                for t in range(4):
                    nc.sync.dma_start(out=fs_spill[b][t][:, :], in_=fs_tiles[t][:])

            # =========== phase 3: AllReduce of logits ===========
            cc_in = dramp.tile([64, NU * 64], F32)
            cc_out = dramp.tile([64, NU * 64], F32)
            nc.gpsimd.dma_start(out=cc_in[:], in_=Sp_sb[:])
            nc.gpsimd.collective_compute(
                'AllReduce', mybir.AluOpType.add,
                replica_groups=[list(range(NCORES))],
                ins=[cc_in.opt()], outs=[cc_out.opt()],
            )
            S_sb = constp.tile([64, NU * 64], F32)
            nc.gpsimd.dma_start(out=S_sb[:], in_=cc_out[:])

            # =========== phase 4: softmax + A_t^T ===========
            attn_sb = constp.tile([64, NU * 64], BF)
            AT_tiles = []
            for u in range(NU):
                usl = slice(u * 64, (u + 1) * 64)
                mx = smallp.tile([64, 1], F32)
                nmx = smallp.tile([64, 1], F32)
                ex = smallp.tile([64, 64], F32)
                sm = smallp.tile([64, 1], F32)
                rs = smallp.tile([64, 1], F32)
                nc.vector.tensor_reduce(out=mx[:], in_=S_sb[:, usl],
                                        axis=mybir.AxisListType.X, op=ALU.max)
                nc.vector.tensor_scalar_mul(out=nmx[:], in0=mx[:], scalar1=-1.0)
                nc.scalar.activation(out=ex[:], in_=S_sb[:, usl], func=AF.Exp,
                                     bias=nmx[:, 0:1])
                nc.vector.tensor_reduce(out=sm[:], in_=ex[:],
                                        axis=mybir.AxisListType.X, op=ALU.add)
                nc.vector.reciprocal(out=rs[:], in_=sm[:])
                nc.vector.tensor_scalar_mul(out=attn_sb[:, usl], in0=ex[:],
                                            scalar1=rs[:, 0:1])
            for b in range(B):
                for t in range(4):
                    u = b * 4 + t
                    a_full = psp.tile([64, 512], F32, name='misc_ps')
                    a_ps = a_full[:, 0:256]
                    nc.tensor.matmul(out=a_ps,
                                     lhsT=attn_sb[:, u * 64:(u + 1) * 64],
                                     rhs=w00o_sb[:, t * 256:(t + 1) * 256],
                                     start=True, stop=True)
                    at = constp.tile([65, 256], BF, name=f'at{b}_{t}')
                    nc.vector.tensor_copy(out=at[0:64, :], in_=a_ps)
                    if t == 0:
                        nc.sync.dma_start(out=at[64:65, :], in_=b00eff[b, :, :])
                    AT_tiles.append(at)

            # =========== phase 5: regather + MLP ===========
            for b in range(B):
                fs_tiles = [fsp.tile([65, NLOC], BF, name=f'fs{_t}') for _t in range(4)]
                for t in range(4):
                    nc.sync.dma_start(out=fs_tiles[t][:], in_=fs_spill[b][t][:, :])

                for pc in range(NLOC // PCH):
                    psl = slice(pc * PCH, (pc + 1) * PCH)
                    # transient v tiles for this pixel super-chunk
                    v_tiles = []
                    for t in range(4):
                        vt = vp.tile([65, PCH], BF, name=f'vt{t}')
                        nc.vector.memset(vt[64:65, :], 1.0)
                        for cc in range(PCH // CHUNK):
                            vsl_l = slice(cc * CHUNK, (cc + 1) * CHUNK)
                            vsl_g = slice(pc * PCH + cc * CHUNK, pc * PCH + (cc + 1) * CHUNK)
                            v_ps = psp.tile([64, CHUNK], F32)
                            nc.tensor.matmul(out=v_ps[:], lhsT=wv_sb[:],
                                             rhs=fs_tiles[t][0:64, vsl_g],
                                             start=True, stop=True)
                            nc.scalar.activation(out=vt[0:64, vsl_l], in_=v_ps[:],
                                                 func=AF.Relu, bias=bv_sb[:, 0:1])
                        v_tiles.append(vt)

                    x1_t = [mlpp.tile([128, PCH], BF, name=f'x1_{_m}') for _m in range(2)]
                    x2_t = [mlpp.tile([128, PCH], BF, name=f'x2_{_m}') for _m in range(2)]
                    for cc in range(PCH // CHUNK):
                        lsl = slice(cc * CHUNK, (cc + 1) * CHUNK)
                        gsl = slice(pc * PCH + cc * CHUNK, pc * PCH + (cc + 1) * CHUNK)
                        for m in range(2):
                            msl = slice(m * 128, (m + 1) * 128)
                            x_ps = psxp.tile([128, CHUNK], F32)
                            for t in range(4):
                                nc.tensor.matmul(
                                    out=x_ps[:],
                                    lhsT=w00f_sb[:, t * 256 + m * 128: t * 256 + (m + 1) * 128],
                                    rhs=fs_tiles[t][:, gsl],
                                    start=(t == 0), stop=False)
                            for t in range(4):
                                at = AT_tiles[b * 4 + t]
                                kk = 65 if t == 0 else 64
                                nc.tensor.matmul(
                                    out=x_ps[:],
                                    lhsT=at[0:kk, msl],
                                    rhs=v_tiles[t][0:kk, lsl],
                                    start=False, stop=(t == 3))
                            nc.vector.tensor_copy(out=x1_t[m][:, lsl], in_=x_ps[:])
                        # W1 + gelu
                        for m in range(2):
                            msl = slice(m * 128, (m + 1) * 128)
                            x2_ps = psxp.tile([128, CHUNK], F32)
                            for kk in range(2):
                                nc.tensor.matmul(out=x2_ps[:],
                                                 lhsT=w1_sb[:, kk, msl],
                                                 rhs=x1_t[kk][:, lsl],
                                                 start=(kk == 0), stop=(kk == 1))
                            nc.scalar.activation(out=x2_t[m][:, lsl], in_=x2_ps[:],
                                                 func=AF.Gelu, bias=b1_sb[:, m:m + 1])
                        # W2 + bil add
                        o_full = psp.tile([64, 512], F32, name='misc_ps')
                        o_ps = o_full[0:1, :]
                        for kk in range(2):
                            nc.tensor.matmul(out=o_ps, lhsT=w2_sb[:, kk:kk + 1],
                                             rhs=x2_t[kk][:, lsl],
                                             start=(kk == 0), stop=(kk == 1))
                        bil_sb = smallp.tile([1, CHUNK], F32)
                        nc.sync.dma_start(out=bil_sb[:], in_=bil[b, gsl][None, :])
                        o_sb = smallp.tile([1, CHUNK], F32)
                        nc.vector.tensor_tensor(out=o_sb[:], in0=o_ps,
                                                in1=bil_sb[:], op=ALU.add)
                        nc.sync.dma_start(out=out[b, gsl][None, :], in_=o_sb[:])

    nc.compile()
    return nc


# --------------------------------------------------------------------------

def kernel(**inputs) -> np.ndarray:
    from concourse.bass_utils import run_bass_kernel_spmd
    in_maps = _host_prep(inputs)
    nc = _build()
    res = run_bass_kernel_spmd(nc, in_maps, core_ids=list(range(NCORES)))
    full = np.empty((B, 1, HQ, WQ), np.float32)
    flat = full.reshape(B, NPB)
    for cidx in range(NCORES):
        flat[:, cidx * NLOC:(cidx + 1) * NLOC] = res.results[cidx]['out']
    return full


# revision 23
# speedup vs baseline: 3.4397x; 2.8812x over previous
"""Trainium2 Bass kernel for nn_AnyTSRpp (sparse_attention).

Strategy: pure data-parallel over the HR pixel grid (65536 px/batch),
8192 px/batch/core on 8 NeuronCores. Host computes per-pixel corner
indices/scalars; device gathers feat rows directly (per-corner indirect
DMA, pixel-major), applies the RBF weight per-partition pre-transpose,
PE transposes to channel-major, runs all matmuls/relu/softmax/gelu, and
a tiny AllReduce for the global attention logits (contraction over all
pixels). off_t = attn_t @ v_t is folded as (W00_off_t @ attn_t) @ v_t
so the attention output is never materialized.

Self-contained: hardcodes all shapes. kernel(**inputs) -> np.ndarray.
"""

import functools
import numpy as np
import ml_dtypes

BF16 = ml_dtypes.bfloat16


def _setup_jax_cache():
    """Persistent XLA compilation cache: repeated/every-process calls skip
    the neuronx-cc recompile of the identical kernel graph."""
    import jax
    try:
        jax.config.update('jax_compilation_cache_dir', '/root/.cache/jax_pcache')
        jax.config.update('jax_persistent_cache_min_compile_time_secs', 0.0)
        jax.config.update('jax_persistent_cache_min_entry_size_bytes', 0)
    except Exception:
        pass


_setup_jax_cache()

NCORES = 8
B = 2
C = 64
HLR = WLR = 64
HQ = WQ = 256
NPB = HQ * WQ            # 65536 pixels per batch
NLOC = NPB // NCORES     # 8192 pixels per batch per core
NROW = HLR * WLR         # 4096 feat rows (y-major)
CHUNK = 512              # matmul moving-N chunk
NCHUNK = NLOC // CHUNK   # 16
PCH = 1024               # MLP pixel super-chunk
EPS = np.float32(1e-6)


# --------------------------------------------------------------------------
# host-side math (mirrors reference semantics in f32)
# --------------------------------------------------------------------------

def _corner_indices(co):
    """co: [N] f32 coords in one axis. Returns (iy_minus, iy_plus) exactly
    matching the reference's per-corner nearest indices."""
    # reference: c_t = clip(co + v/64 + eps, -1+1e-6, 1-1e-6);
    #            i_t = clip(round((c_t+1)*32 - 0.5), 0, 63)
    out = []
    for v in (-1.0, 1.0):
        c = np.clip(co + np.float32(v / 64.0) + EPS,
                    np.float32(-1 + 1e-6), np.float32(1 - 1e-6))
        i = np.clip(np.round((c + 1) * np.float32(32.0) - np.float32(0.5)),
                    0, 63).astype(np.int32)
        out.append(i)
    return out


def _host_prep(inputs):
    feat = np.asarray(inputs['feat'], np.float32)
    inp = np.asarray(inputs['inp'], np.float32)
    coord = np.asarray(inputs['coord'], np.float32)
    cell = np.asarray(inputs['cell'], np.float32)
    scale = np.asarray(inputs['scale'], np.float32)
    Wq = np.asarray(inputs['Wq'], np.float32); bq = np.asarray(inputs['bq'], np.float32)
    Wk = np.asarray(inputs['Wk'], np.float32); bk = np.asarray(inputs['bk'], np.float32)
    Wv = np.asarray(inputs['Wv'], np.float32); bv = np.asarray(inputs['bv'], np.float32)
    W00 = np.asarray(inputs['W00'], np.float32); b00 = np.asarray(inputs['b00'], np.float32)
    W1 = np.asarray(inputs['W1'], np.float32); b1 = np.asarray(inputs['b1'], np.float32)
    W2 = np.asarray(inputs['W2'], np.float32); b2 = np.asarray(inputs['b2'], np.float32)
    ls = np.asarray(inputs['ls'], np.float32)

    # feat as bf16 rows [B, 4096, 64]: row iy*64+ix = feat[b, :, iy, ix]
    featrows = np.ascontiguousarray(
        feat.transpose(0, 2, 3, 1).reshape(B, NROW, C)).astype(BF16)

    coord_y = coord[..., 0].reshape(B, NPB)
    coord_x = coord[..., 1].reshape(B, NPB)

    # per-(b, corner) gather indices + rel offsets + RBF weights
    idx_all = np.empty((B, 4, NPB), np.int16)
    rel_all = np.empty((B, 4, 2, NPB), BF16)   # [rel_y, rel_x]
    w_all = np.empty((B, 4, NPB), BF16)
    hw = np.float32(64.0)
    ls2 = ls[0] * ls[0]
    for b in range(B):
        iym, iyp = _corner_indices(coord_y[b])
        ixm, ixp = _corner_indices(coord_x[b])
        iy = {-1: iym, 1: iyp}
        ix = {-1: ixm, 1: ixp}
        t = 0
        for vx in (-1, 1):          # y offset
            for vy in (-1, 1):      # x offset
                idx_all[b, t] = (iy[vx] * np.int32(64) + ix[vy]).astype(np.int16)
                oy = (iy[vx].astype(np.float32) + np.float32(0.5)) / np.float32(32.0) - 1
                ox = (ix[vy].astype(np.float32) + np.float32(0.5)) / np.float32(32.0) - 1
                ry = coord_y[b] - oy
                rx = coord_x[b] - ox
                rel_all[b, t, 0] = ry.astype(BF16)
                rel_all[b, t, 1] = rx.astype(BF16)
                rd = (ry * hw) ** 2 + (rx * hw) ** 2
                w_all[b, t] = np.exp(rd / ls2 * np.float32(-0.5)).astype(BF16)
                t += 1

    # ---- bilinear sample of inp (border, align_corners=False) + b2 ----
    bil = np.empty((B, NPB), BF16)
    for b in range(B):
        im = inp[b, 0]
        y = np.clip((coord_y[b] + 1) * np.float32(32.0) - np.float32(0.5), 0.0, 63.0)
        x = np.clip((coord_x[b] + 1) * np.float32(32.0) - np.float32(0.5), 0.0, 63.0)
        y0 = np.floor(y); x0 = np.floor(x)
        wy = (y - y0).astype(np.float32); wx = (x - x0).astype(np.float32)
        y0i = np.clip(y0.astype(np.int32), 0, 63)
        y1i = np.clip(y0.astype(np.int32) + 1, 0, 63)
        x0i = np.clip(x0.astype(np.int32), 0, 63)
        x1i = np.clip(x0.astype(np.int32) + 1, 0, 63)
        v00 = im[y0i, x0i]; v01 = im[y0i, x1i]
        v10 = im[y1i, x0i]; v11 = im[y1i, x1i]
        bil[b] = ((v00 * (1 - wy) * (1 - wx) + v01 * (1 - wy) * wx
                   + v10 * wy * (1 - wx) + v11 * wy * wx) + b2[0]).astype(BF16)

    # ---- weight repacks ----
    wq_rhs = np.concatenate([bq[None, :], Wq.T], axis=0).astype(BF16)       # [3, 64]
    wk_rhs = np.concatenate([Wk.T, bk[None, :]], axis=0).astype(BF16)       # [65, 64]
    wv_lhsT = Wv.T.astype(BF16)                                             # [64, 64]
    w00off_rhs = np.stack([W00[:, t * 64:(t + 1) * 64].T for t in range(4)]
                          ).astype(BF16)                                    # [4, 64, 256]
    w00fs_lhsT = np.stack(
        [np.concatenate([W00[:, 256 + t * 64: 256 + (t + 1) * 64].T,
                         np.zeros((1, 256), np.float32)], axis=0)
         for t in range(4)]).astype(BF16)                                   # [4, 65, 256]
    b00eff = np.empty((B, 1, 256), BF16)
    for b in range(B):
        vec4 = np.concatenate([cell[b] * hw, scale[b]]).astype(np.float32)
        b00eff[b, 0] = (b00 + W00[:, 512:516] @ vec4).astype(BF16)
    w1_lhsT = np.ascontiguousarray(W1.T.astype(BF16).reshape(2, 128, 256))  # [2, 128, 256]
    w2_lhsT = np.ascontiguousarray(W2.T.astype(BF16).reshape(2, 128, 1))    # [2, 128, 1]

    # ---- shard per core ----
    NFS = NROW // NCORES     # 512 feat rows per core shard (AllGathered on device)
    in_maps = []
    for cidx in range(NCORES):
        sl = slice(cidx * NLOC, (cidx + 1) * NLOC)
        # pixel-major tiles: local pixel j*128+p at [p, t*64+j]
        idx2d = np.ascontiguousarray(
            idx_all[:, :, sl].reshape(B, 4, 64, 128).transpose(0, 3, 1, 2)
            .reshape(B, 128, 4 * 64))
        wsm2d = np.ascontiguousarray(
            w_all[:, :, sl].reshape(B, 4, 64, 128).transpose(0, 3, 1, 2)
            .reshape(B, 128, 4 * 64))
        m = {
            'feati': np.ascontiguousarray(
                featrows[:, cidx * NFS:(cidx + 1) * NFS, :]).reshape(B, 128, 256),
            'idx': idx2d,
            'wsm': wsm2d,
            'relq': np.ascontiguousarray(rel_all[:, :, :, sl]),
            'bil': np.ascontiguousarray(bil[:, sl]),
            'wq_rhs': wq_rhs, 'wk_rhs': wk_rhs, 'wv_lhsT': wv_lhsT,
            'bv': bv.reshape(64, 1).astype(np.float32),
            'w00off_rhs': w00off_rhs, 'w00fs_lhsT': w00fs_lhsT,
            'b00eff': b00eff,
            'w1_lhsT': w1_lhsT,
            'b1': np.ascontiguousarray(b1.astype(np.float32).reshape(2, 128, 1)),
            'w2_lhsT': w2_lhsT,
        }
        in_maps.append(m)
    return in_maps


# --------------------------------------------------------------------------
# device kernel
# --------------------------------------------------------------------------

@functools.lru_cache(maxsize=1)
def _build():
    import concourse.bass as bass
    import concourse.tile as tile
    from concourse import bacc, mybir
    dt = mybir.dt
    F32, BF, I16 = dt.float32, dt.bfloat16, dt.int16
    AF = mybir.ActivationFunctionType
    ALU = mybir.AluOpType

    nc = bacc.Bacc(None, target_bir_lowering=False)

    feati = nc.dram_tensor('feati', [B, 128, 256], BF, kind='ExternalInput')
    idx = nc.dram_tensor('idx', [B, 128, 4 * 64], I16, kind='ExternalInput')
    wsm = nc.dram_tensor('wsm', [B, 128, 4 * 64], BF, kind='ExternalInput')
    relq = nc.dram_tensor('relq', [B, 4, 2, NLOC], BF, kind='ExternalInput')
    bil = nc.dram_tensor('bil', [B, NLOC], BF, kind='ExternalInput')
    wq_rhs = nc.dram_tensor('wq_rhs', [3, 64], BF, kind='ExternalInput')
    wk_rhs = nc.dram_tensor('wk_rhs', [65, 64], BF, kind='ExternalInput')
    wv_lhsT = nc.dram_tensor('wv_lhsT', [64, 64], BF, kind='ExternalInput')
    bv = nc.dram_tensor('bv', [64, 1], F32, kind='ExternalInput')
    w00off_rhs = nc.dram_tensor('w00off_rhs', [4, 64, 256], BF, kind='ExternalInput')
    w00fs_lhsT = nc.dram_tensor('w00fs_lhsT', [4, 65, 256], BF, kind='ExternalInput')
    b00eff = nc.dram_tensor('b00eff', [B, 1, 256], BF, kind='ExternalInput')
    w1_lhsT = nc.dram_tensor('w1_lhsT', [2, 128, 256], BF, kind='ExternalInput')
    b1 = nc.dram_tensor('b1', [2, 128, 1], F32, kind='ExternalInput')
    w2_lhsT = nc.dram_tensor('w2_lhsT', [2, 128, 1], BF, kind='ExternalInput')
    out = nc.dram_tensor('out', [B, NLOC], F32, kind='ExternalOutput')

    NU = B * 4  # 8 attention units

    with tile.TileContext(nc) as tc:
        with (
            tc.tile_pool(name='const', bufs=1) as constp,
            tc.tile_pool(name='fs', bufs=1) as fsp,
            tc.tile_pool(name='gat', bufs=1) as gatp,
            tc.tile_pool(name='qk', bufs=1) as qkp,
            tc.tile_pool(name='rel', bufs=1) as relp,
            tc.tile_pool(name='v', bufs=1) as vp,
            tc.tile_pool(name='mlp', bufs=1) as mlpp,
            tc.tile_pool(name='small', bufs=1) as smallp,
            tc.tile_pool(name='ps', bufs=1, space='PSUM') as psp,
            tc.tile_pool(name='psx', bufs=1, space='PSUM') as psxp,
            tc.tile_pool(name='dram', bufs=1, space='DRAM') as dramp,
        ):
            # ---- constant weights to SBUF ----
            wq_sb = constp.tile([3, 64], BF)
            wk_sb = constp.tile([65, 64], BF)
            wv_sb = constp.tile([64, 64], BF)
            bv_sb = constp.tile([64, 1], F32)
            w00o_sb = constp.tile([64, 4 * 256], BF)
            w00f_sb = constp.tile([65, 4 * 256], BF)
            w1_sb = constp.tile([128, 2, 256], BF)
            b1_sb = constp.tile([128, 2], F32)
            w2_sb = constp.tile([128, 2], BF)
            nc.sync.dma_start(out=wq_sb[:], in_=wq_rhs[:, :])
            nc.sync.dma_start(out=wk_sb[:], in_=wk_rhs[:, :])
            nc.sync.dma_start(out=wv_sb[:], in_=wv_lhsT[:, :])
            nc.sync.dma_start(out=bv_sb[:], in_=bv[:, :])
            for t in range(4):
                nc.sync.dma_start(out=w00o_sb[:, t * 256:(t + 1) * 256],
                                  in_=w00off_rhs[t, :, :])
                nc.sync.dma_start(out=w00f_sb[:, t * 256:(t + 1) * 256],
                                  in_=w00fs_lhsT[t, :, :])
            for kk in range(2):
                nc.sync.dma_start(out=w1_sb[:, kk, :], in_=w1_lhsT[kk, :, :])
                nc.sync.dma_start(out=b1_sb[:, kk:kk + 1], in_=b1[kk, :, :])
                nc.sync.dma_start(out=w2_sb[:, kk:kk + 1], in_=w2_lhsT[kk, :, :])

            Sp_sb = constp.tile([64, NU * 64], F32)   # partial logits, all units

            # ---- AllGather the feat row shards: 512 rows/core -> [NROW,C] ----
            featfull = [dramp.tile([NROW, C], BF, name=f'featfull{_b}')
                        for _b in range(B)]
            for _b in range(B):
                fstage = gatp.tile([128, 256], BF, name='fstage')
                ccf_in = dramp.tile([128, 256], BF, name=f'ccf_in{_b}')
                nc.sync.dma_start(out=fstage[:], in_=feati[_b, :, :])
                nc.gpsimd.dma_start(out=ccf_in[:], in_=fstage[:])
                nc.gpsimd.collective_compute(
                    'AllGather', mybir.AluOpType.bypass,
                    replica_groups=[list(range(NCORES))],
                    ins=[ccf_in.opt()], outs=[featfull[_b].opt()],
                )

            # =========== phases 1+2 per batch: gather, fs, q/k, S ===========
            from concourse.masks import make_identity
            ident_sb = constp.tile([128, 128], BF)
            make_identity(nc, ident_sb[:])

            def gather_fs(b, fs_tiles):
                idx16_sb = gatp.tile([128, 4 * 64], I16)
                wsm16_sb = gatp.tile([128, 4 * 64], BF)
                idx_sb = gatp.tile([128, 4 * 64], dt.int32)
                wsm_sb = gatp.tile([128, 4 * 64], F32)
                nc.sync.dma_start(out=idx16_sb[:], in_=idx[b, :, :])
                nc.sync.dma_start(out=wsm16_sb[:], in_=wsm[b, :, :])
                nc.vector.tensor_copy(out=idx_sb[:], in_=idx16_sb[:])
                nc.vector.tensor_copy(out=wsm_sb[:], in_=wsm16_sb[:])
                for t in range(4):
                    g_pm = gatp.tile([128, 64, C], BF, name=f'g_pm{t % 2}')
                    for j in range(64):
                        nc.gpsimd.indirect_dma_start(
                            out=g_pm[:, j, :], out_offset=None,
                            in_=featfull[b][:, :],
                            in_offset=bass.IndirectOffsetOnAxis(
                                ap=idx_sb[:, t * 64 + j:t * 64 + j + 1], axis=0))
                    # RBF weight applied per-pixel (per-partition scalar)
                    for j in range(64):
                        nc.vector.tensor_scalar_mul(
                            out=g_pm[:, j, :], in0=g_pm[:, j, :],
                            scalar1=wsm_sb[:, t * 64 + j:t * 64 + j + 1])
                    for jg in range(16):
                        tp_ps = psp.tile([64, 512], BF)
                        for jj in range(4):
                            j = jg * 4 + jj
                            nc.tensor.transpose(
                                out=tp_ps[:, jj * 128:(jj + 1) * 128],
                                in_=g_pm[:, j, :],
                                identity=ident_sb[:])
                        nc.scalar.copy(out=fs_tiles[t][0:64, jg * 512:(jg + 1) * 512],
                                       in_=tp_ps[:])
                for _t in range(4):
                    nc.vector.memset(fs_tiles[_t][64:65, :], 1.0)

            fs_spill = [[dramp.tile([65, NLOC], BF, name=f'fsspill{_b}_{_t}')
                         for _t in range(4)] for _b in range(B)]
            for b in range(B):
                fs_tiles = [fsp.tile([65, NLOC], BF, name=f'fs{_t}') for _t in range(4)]
                gather_fs(b, fs_tiles)

                for t in range(4):
                    rel_sb = relp.tile([3, NLOC], BF)
                    nc.vector.memset(rel_sb[0:1, :], 1.0)
                    nc.sync.dma_start(out=rel_sb[1:3, :], in_=relq[b, t, :, :])
                    qT_sb = qkp.tile([128, 64 * 64], BF)
                    kT_sb = qkp.tile([128, 64 * 64], BF)
                    s_ps = psp.tile([64, 64], F32, name='s_ps')
                    for jg in range(8):          # groups of 8 pixel-tiles
                        q_ps = psp.tile([128, 512], F32)
                        k_ps = psp.tile([128, 512], F32)
                        for jj in range(8):
                            j = jg * 8 + jj
                            nc.tensor.matmul(
                                out=q_ps[:, jj * 64:(jj + 1) * 64],
                                lhsT=rel_sb[:, j * 128:(j + 1) * 128],
                                rhs=wq_sb[:], start=True, stop=True)
                            nc.tensor.matmul(
                                out=k_ps[:, jj * 64:(jj + 1) * 64],
                                lhsT=fs_tiles[t][:, j * 128:(j + 1) * 128],
                                rhs=wk_sb[:], start=True, stop=True)
                        gsl = slice(jg * 512, (jg + 1) * 512)
                        nc.scalar.activation(out=qT_sb[:, gsl], in_=q_ps[:], func=AF.Relu)
                        nc.vector.tensor_scalar_max(out=kT_sb[:, gsl], in0=k_ps[:], scalar1=0.0)
                    for j in range(64):
                        nc.tensor.matmul(
                            out=s_ps[:],
                            lhsT=qT_sb[:, j * 64:(j + 1) * 64],
                            rhs=kT_sb[:, j * 64:(j + 1) * 64],
                            start=(j == 0), stop=(j == 63))
                    u = b * 4 + t
                    nc.vector.tensor_copy(out=Sp_sb[:, u * 64:(u + 1) * 64], in_=s_ps[:])
                for t in range(4):
                    nc.sync.dma_start(out=fs_spill[b][t][:, :], in_=fs_tiles[t][:])

            # =========== phase 3: AllReduce of logits ===========
            cc_in = dramp.tile([64, NU * 64], F32)
            cc_out = dramp.tile([64, NU * 64], F32)
            nc.gpsimd.dma_start(out=cc_in[:], in_=Sp_sb[:])
            nc.gpsimd.collective_compute(
                'AllReduce', mybir.AluOpType.add,
                replica_groups=[list(range(NCORES))],
                ins=[cc_in.opt()], outs=[cc_out.opt()],
            )
            S_sb = constp.tile([64, NU * 64], F32)
            nc.gpsimd.dma_start(out=S_sb[:], in_=cc_out[:])

            # =========== phase 4: softmax + A_t^T ===========
            attn_sb = constp.tile([64, NU * 64], BF)
            AT_tiles = []
            for u in range(NU):
                usl = slice(u * 64, (u + 1) * 64)
                mx = smallp.tile([64, 1], F32)
                nmx = smallp.tile([64, 1], F32)
                ex = smallp.tile([64, 64], F32)
                sm = smallp.tile([64, 1], F32)
                rs = smallp.tile([64, 1], F32)
                nc.vector.tensor_reduce(out=mx[:], in_=S_sb[:, usl],
                                        axis=mybir.AxisListType.X, op=ALU.max)
                nc.vector.tensor_scalar_mul(out=nmx[:], in0=mx[:], scalar1=-1.0)
                nc.scalar.activation(out=ex[:], in_=S_sb[:, usl], func=AF.Exp,
                                     bias=nmx[:, 0:1])
                nc.vector.tensor_reduce(out=sm[:], in_=ex[:],
                                        axis=mybir.AxisListType.X, op=ALU.add)
                nc.vector.reciprocal(out=rs[:], in_=sm[:])
                nc.vector.tensor_scalar_mul(out=attn_sb[:, usl], in0=ex[:],
                                            scalar1=rs[:, 0:1])
            for b in range(B):
                for t in range(4):
                    u = b * 4 + t
                    a_full = psp.tile([64, 512], F32, name='misc_ps')
                    a_ps = a_full[:, 0:256]
                    nc.tensor.matmul(out=a_ps,
                                     lhsT=attn_sb[:, u * 64:(u + 1) * 64],
                                     rhs=w00o_sb[:, t * 256:(t + 1) * 256],
                                     start=True, stop=True)
                    at = constp.tile([65, 256], BF, name=f'at{b}_{t}')
                    nc.vector.tensor_copy(out=at[0:64, :], in_=a_ps)
                    if t == 0:
                        nc.sync.dma_start(out=at[64:65, :], in_=b00eff[b, :, :])
                    AT_tiles.append(at)

            # =========== phase 5: regather + MLP ===========
            for b in range(B):
                fs_tiles = [fsp.tile([65, NLOC], BF, name=f'fs{_t}') for _t in range(4)]
                for t in range(4):
                    nc.sync.dma_start(out=fs_tiles[t][:], in_=fs_spill[b][t][:, :])
                bil_sb = smallp.tile([1, NLOC], BF, name='bil_sb')
                nc.sync.dma_start(out=bil_sb[:], in_=bil[b, :][None, :])

                for pc in range(NLOC // PCH):
                    psl = slice(pc * PCH, (pc + 1) * PCH)
                    # transient v tiles for this pixel super-chunk
                    v_tiles = []
                    for t in range(4):
                        vt = vp.tile([65, PCH], BF, name=f'vt{t}')
                        nc.vector.memset(vt[64:65, :], 1.0)
                        for cc in range(PCH // CHUNK):
                            vsl_l = slice(cc * CHUNK, (cc + 1) * CHUNK)
                            vsl_g = slice(pc * PCH + cc * CHUNK, pc * PCH + (cc + 1) * CHUNK)
                            v_ps = psp.tile([64, CHUNK], F32)
                            nc.tensor.matmul(out=v_ps[:], lhsT=wv_sb[:],
                                             rhs=fs_tiles[t][0:64, vsl_g],
                                             start=True, stop=True)
                            nc.scalar.activation(out=vt[0:64, vsl_l], in_=v_ps[:],
                                                 func=AF.Relu, bias=bv_sb[:, 0:1])
                        v_tiles.append(vt)

                    x1_t = [mlpp.tile([128, PCH], BF, name=f'x1_{_m}') for _m in range(2)]
                    x2_t = [mlpp.tile([128, PCH], BF, name=f'x2_{_m}') for _m in range(2)]
                    for cc in range(PCH // CHUNK):
                        lsl = slice(cc * CHUNK, (cc + 1) * CHUNK)
                        gsl = slice(pc * PCH + cc * CHUNK, pc * PCH + (cc + 1) * CHUNK)
                        for m in range(2):
                            msl = slice(m * 128, (m + 1) * 128)
                            x_ps = psxp.tile([128, CHUNK], F32)
                            for t in range(4):
                                nc.tensor.matmul(
                                    out=x_ps[:],
                                    lhsT=w00f_sb[:, t * 256 + m * 128: t * 256 + (m + 1) * 128],
                                    rhs=fs_tiles[t][:, gsl],
                                    start=(t == 0), stop=False)
                            for t in range(4):
                                at = AT_tiles[b * 4 + t]
                                kk = 65 if t == 0 else 64
                                nc.tensor.matmul(
                                    out=x_ps[:],
                                    lhsT=at[0:kk, msl],
                                    rhs=v_tiles[t][0:kk, lsl],
                                    start=False, stop=(t == 3))
                            nc.vector.tensor_copy(out=x1_t[m][:, lsl], in_=x_ps[:])
                        # W1 + gelu
                        for m in range(2):
                            msl = slice(m * 128, (m + 1) * 128)
                            x2_ps = psxp.tile([128, CHUNK], F32)
                            for kk in range(2):
                                nc.tensor.matmul(out=x2_ps[:],
                                                 lhsT=w1_sb[:, kk, msl],
                                                 rhs=x1_t[kk][:, lsl],
                                                 start=(kk == 0), stop=(kk == 1))
                            nc.scalar.activation(out=x2_t[m][:, lsl], in_=x2_ps[:],
                                                 func=AF.Gelu, bias=b1_sb[:, m:m + 1])
                        # W2 + bil add
                        o_full = psp.tile([64, 512], F32, name='misc_ps')
                        o_ps = o_full[0:1, :]
                        for kk in range(2):
                            nc.tensor.matmul(out=o_ps, lhsT=w2_sb[:, kk:kk + 1],
                                             rhs=x2_t[kk][:, lsl],
                                             start=(kk == 0), stop=(kk == 1))
                        o_sb = smallp.tile([1, CHUNK], F32)
                        nc.vector.tensor_tensor(out=o_sb[:], in0=o_ps,
                                                in1=bil_sb[:, gsl], op=ALU.add)
                        nc.sync.dma_start(out=out[b, gsl][None, :], in_=o_sb[:])

    nc.compile()
    return nc


# --------------------------------------------------------------------------

def kernel(**inputs) -> np.ndarray:
    from concourse.bass_utils import run_bass_kernel_spmd
    in_maps = _host_prep(inputs)
    nc = _build()
    res = run_bass_kernel_spmd(nc, in_maps, core_ids=list(range(NCORES)))
    full = np.empty((B, 1, HQ, WQ), np.float32)
    flat = full.reshape(B, NPB)
    for cidx in range(NCORES):
        flat[:, cidx * NLOC:(cidx + 1) * NLOC] = res.results[cidx]['out']
    return full


# revision 42
# speedup vs baseline: 897.2025x; 260.8359x over previous
"""Trainium2 Bass kernel for nn_AnyTSRpp (sparse_attention).

Strategy: pure data-parallel over the HR pixel grid (65536 px/batch),
8192 px/batch/core on 8 NeuronCores. Host computes per-pixel corner
indices/scalars; device gathers feat rows directly (per-corner indirect
DMA, pixel-major), applies the RBF weight per-partition pre-transpose,
PE transposes to channel-major, runs all matmuls/relu/softmax/gelu, and
a tiny AllReduce for the global attention logits (contraction over all
pixels). off_t = attn_t @ v_t is folded as (W00_off_t @ attn_t) @ v_t
so the attention output is never materialized.

Self-contained: hardcodes all shapes. kernel(**inputs) -> np.ndarray.
"""

import functools
import numpy as np
import ml_dtypes

BF16 = ml_dtypes.bfloat16


def _setup_jax_cache():
    """Persistent XLA compilation cache: repeated/every-process calls skip
    the neuronx-cc recompile of the identical kernel graph."""
    import jax
    try:
        jax.config.update('jax_compilation_cache_dir', '/root/.cache/jax_pcache')
        jax.config.update('jax_persistent_cache_min_compile_time_secs', 0.0)
        jax.config.update('jax_persistent_cache_min_entry_size_bytes', 0)
    except Exception:
        pass


_setup_jax_cache()

NCORES = 8
B = 2
C = 64
HLR = WLR = 64
HQ = WQ = 256
NPB = HQ * WQ            # 65536 pixels per batch
NLOC = NPB // NCORES     # 8192 pixels per batch per core
NROW = HLR * WLR         # 4096 feat rows (y-major)
CHUNK = 512              # matmul moving-N chunk
NCHUNK = NLOC // CHUNK   # 16
PCH = 1024               # MLP pixel super-chunk
EPS = np.float32(1e-6)

# bf16 weight blob layout (flat element offsets)
WOFF_WQ = 0                        # [3, 64]
WOFF_WK = WOFF_WQ + 3 * 64         # [65, 64]
WOFF_WV = WOFF_WK + 65 * 64        # [64, 64]
WOFF_W00O = WOFF_WV + 64 * 64      # [4, 64, 256]
WOFF_W00F = WOFF_W00O + 4 * 64 * 256   # [4, 65, 256]
WOFF_B00 = WOFF_W00F + 4 * 65 * 256    # [B, 1, 256]
WOFF_W1 = WOFF_B00 + B * 256       # [2, 128, 256]
WOFF_W2 = WOFF_W1 + 2 * 128 * 256  # [2, 128, 1]
WBLOB = WOFF_W2 + 2 * 128          # 206848 = 8 * 25856 = 8 * 128 * 202
WSH = WBLOB // NCORES


# --------------------------------------------------------------------------
# host-side math (mirrors reference semantics in f32)
# --------------------------------------------------------------------------

def _corner_indices(co):
    """co: [N] f32 coords in one axis. Returns (iy_minus, iy_plus) exactly
    matching the reference's per-corner nearest indices."""
    # reference: c_t = clip(co + v/64 + eps, -1+1e-6, 1-1e-6);
    #            i_t = clip(round((c_t+1)*32 - 0.5), 0, 63)
    out = []
    for v in (-1.0, 1.0):
        c = np.clip(co + np.float32(v / 64.0) + EPS,
                    np.float32(-1 + 1e-6), np.float32(1 - 1e-6))
        i = np.clip(np.round((c + 1) * np.float32(32.0) - np.float32(0.5)),
                    0, 63).astype(np.int32)
        out.append(i)
    return out


def _host_prep(inputs):
    feat = np.asarray(inputs['feat'], np.float32)
    inp = np.asarray(inputs['inp'], np.float32)
    coord = np.asarray(inputs['coord'], np.float32)
    cell = np.asarray(inputs['cell'], np.float32)
    scale = np.asarray(inputs['scale'], np.float32)
    Wq = np.asarray(inputs['Wq'], np.float32); bq = np.asarray(inputs['bq'], np.float32)
    Wk = np.asarray(inputs['Wk'], np.float32); bk = np.asarray(inputs['bk'], np.float32)
    Wv = np.asarray(inputs['Wv'], np.float32); bv = np.asarray(inputs['bv'], np.float32)
    W00 = np.asarray(inputs['W00'], np.float32); b00 = np.asarray(inputs['b00'], np.float32)
    W1 = np.asarray(inputs['W1'], np.float32); b1 = np.asarray(inputs['b1'], np.float32)
    W2 = np.asarray(inputs['W2'], np.float32); b2 = np.asarray(inputs['b2'], np.float32)
    ls = np.asarray(inputs['ls'], np.float32)

    # feat as bf16 rows [B, 4096, 64]: row iy*64+ix = feat[b, :, iy, ix]
    featrows = np.ascontiguousarray(
        feat.transpose(0, 2, 3, 1).reshape(B, NROW, C)).astype(BF16)

    coord_y = coord[..., 0].reshape(B, NPB)
    coord_x = coord[..., 1].reshape(B, NPB)

    # per-(b, corner) gather indices + rel offsets + RBF weights
    idx_all = np.empty((B, 4, NPB), np.int16)
    rel_all = np.empty((B, 4, 2, NPB), BF16)   # [rel_y, rel_x]
    w_all = np.empty((B, 4, NPB), BF16)
    hw = np.float32(64.0)
    ls2 = ls[0] * ls[0]
    for b in range(B):
        iym, iyp = _corner_indices(coord_y[b])
        ixm, ixp = _corner_indices(coord_x[b])
        iy = {-1: iym, 1: iyp}
        ix = {-1: ixm, 1: ixp}
        t = 0
        for vx in (-1, 1):          # y offset
            for vy in (-1, 1):      # x offset
                idx_all[b, t] = (iy[vx] * np.int32(64) + ix[vy]).astype(np.int16)
                oy = (iy[vx].astype(np.float32) + np.float32(0.5)) / np.float32(32.0) - 1
                ox = (ix[vy].astype(np.float32) + np.float32(0.5)) / np.float32(32.0) - 1
                ry = coord_y[b] - oy
                rx = coord_x[b] - ox
                rel_all[b, t, 0] = ry.astype(BF16)
                rel_all[b, t, 1] = rx.astype(BF16)
                rd = (ry * hw) ** 2 + (rx * hw) ** 2
                w_all[b, t] = np.exp(rd / ls2 * np.float32(-0.5)).astype(BF16)
                t += 1

    # ---- bilinear sample of inp (border, align_corners=False) + b2 ----
    bil = np.empty((B, NPB), BF16)
    for b in range(B):
        im = inp[b, 0]
        y = np.clip((coord_y[b] + 1) * np.float32(32.0) - np.float32(0.5), 0.0, 63.0)
        x = np.clip((coord_x[b] + 1) * np.float32(32.0) - np.float32(0.5), 0.0, 63.0)
        y0 = np.floor(y); x0 = np.floor(x)
        wy = (y - y0).astype(np.float32); wx = (x - x0).astype(np.float32)
        y0i = np.clip(y0.astype(np.int32), 0, 63)
        y1i = np.clip(y0.astype(np.int32) + 1, 0, 63)
        x0i = np.clip(x0.astype(np.int32), 0, 63)
        x1i = np.clip(x0.astype(np.int32) + 1, 0, 63)
        v00 = im[y0i, x0i]; v01 = im[y0i, x1i]
        v10 = im[y1i, x0i]; v11 = im[y1i, x1i]
        bil[b] = ((v00 * (1 - wy) * (1 - wx) + v01 * (1 - wy) * wx
                   + v10 * wy * (1 - wx) + v11 * wy * wx) + b2[0]).astype(BF16)

    # ---- rel -> int8 with the dequant scale folded into Wq's rel rows ----
    # row 2 is a constant 1 so the int8->bf16 widen also produces the
    # bias row for the q matmul.
    relmax = float(np.max(np.abs(rel_all.astype(np.float32)))) or 1.0
    QK = np.float32(127.0 / relmax)
    rel8 = np.ones((B, 4, 3, NPB), np.int8)
    rel8[:, :, 0:2] = np.clip(np.round(rel_all.astype(np.float32) * QK),
                              -127, 127).astype(np.int8)

    # ---- weight repacks ----
    wq_rhs = np.concatenate([Wq.T / QK, bq[None, :]], axis=0).astype(BF16)  # [3, 64]
    wk_rhs = np.concatenate([Wk.T, bk[None, :]], axis=0).astype(BF16)       # [65, 64]
    wv_lhsT = Wv.T.astype(BF16)                                             # [64, 64]
    w00off_rhs = np.stack([W00[:, t * 64:(t + 1) * 64].T for t in range(4)]
                          ).astype(BF16)                                    # [4, 64, 256]
    w00fs_lhsT = np.stack(
        [np.concatenate([W00[:, 256 + t * 64: 256 + (t + 1) * 64].T,
                         np.zeros((1, 256), np.float32)], axis=0)
         for t in range(4)]).astype(BF16)                                   # [4, 65, 256]
    b00eff = np.empty((B, 1, 256), BF16)
    for b in range(B):
        vec4 = np.concatenate([cell[b] * hw, scale[b]]).astype(np.float32)
        b00eff[b, 0] = (b00 + W00[:, 512:516] @ vec4).astype(BF16)
    w1_lhsT = np.ascontiguousarray(W1.T.astype(BF16).reshape(2, 128, 256))  # [2, 128, 256]
    w2_lhsT = np.ascontiguousarray(W2.T.astype(BF16).reshape(2, 128, 1))    # [2, 128, 1]

    # ---- bf16 weight blob (AllGathered on device): flat row-major concat ----
    wflat = np.concatenate([
        wq_rhs.reshape(-1), wk_rhs.reshape(-1), wv_lhsT.reshape(-1),
        w00off_rhs.reshape(-1), w00fs_lhsT.reshape(-1), b00eff.reshape(-1),
        w1_lhsT.reshape(-1), w2_lhsT.reshape(-1)])
    assert wflat.size == WBLOB, wflat.size

    # ---- shard per core ----
    NFS = NROW // NCORES     # 512 feat rows per core shard (AllGathered on device)
    in_maps = []
    for cidx in range(NCORES):
        sl = slice(cidx * NLOC, (cidx + 1) * NLOC)
        # pixel-major tiles: local pixel j*128+p at [p, j*4+t] (corner-minor
        # so one indirect DMA instruction covers 4 corners x many tiles)
        idx2d = np.ascontiguousarray(
            idx_all[:, :, sl].reshape(B, 4, 64, 128).transpose(0, 3, 2, 1)
            .reshape(B, 128, 4 * 64))
        wsm2d = np.ascontiguousarray(
            w_all[:, :, sl].reshape(B, 4, 64, 128).transpose(0, 3, 2, 1)
            .reshape(B, 128, 4 * 64))
        m = {
            'feati': np.ascontiguousarray(
                featrows[:, cidx * NFS:(cidx + 1) * NFS, :]).reshape(B, 128, 256),
            'wblob': np.ascontiguousarray(
                wflat[cidx * WSH:(cidx + 1) * WSH]).reshape(128, WSH // 128),
            'idx': idx2d,
            'wsm': wsm2d,
            'relq': np.ascontiguousarray(rel8[:, :, :, sl]),
            'bil': np.ascontiguousarray(bil[:, sl]),
            'bv': bv.reshape(64, 1).astype(np.float32),
            'b1': np.ascontiguousarray(b1.astype(np.float32).reshape(2, 128, 1)),
        }
        in_maps.append(m)
    return in_maps


# --------------------------------------------------------------------------
# device kernel
# --------------------------------------------------------------------------

@functools.lru_cache(maxsize=1)
def _build():
    import concourse.bass as bass
    import concourse.tile as tile
    from concourse import bacc, mybir
    dt = mybir.dt
    F32, BF, I16 = dt.float32, dt.bfloat16, dt.int16
    AF = mybir.ActivationFunctionType
    ALU = mybir.AluOpType

    nc = bacc.Bacc(None, target_bir_lowering=False)

    feati = nc.dram_tensor('feati', [B, 128, 256], BF, kind='ExternalInput')
    wblob = nc.dram_tensor('wblob', [128, WSH // 128], BF, kind='ExternalInput')
    idx = nc.dram_tensor('idx', [B, 128, 4 * 64], I16, kind='ExternalInput')
    wsm = nc.dram_tensor('wsm', [B, 128, 4 * 64], BF, kind='ExternalInput')
    relq = nc.dram_tensor('relq', [B, 4, 3, NLOC], dt.int8, kind='ExternalInput')
    bil = nc.dram_tensor('bil', [B, NLOC], BF, kind='ExternalInput')
    bv = nc.dram_tensor('bv', [64, 1], F32, kind='ExternalInput')
    b1 = nc.dram_tensor('b1', [2, 128, 1], F32, kind='ExternalInput')
    out = nc.dram_tensor('out', [B, NLOC], BF, kind='ExternalOutput')

    NU = B * 4  # 8 attention units

    with tile.TileContext(nc) as tc:
        with (
            tc.tile_pool(name='const', bufs=1) as constp,
            tc.tile_pool(name='fs', bufs=1) as fsp,
            tc.tile_pool(name='gat', bufs=1) as gatp,
            tc.tile_pool(name='qk', bufs=1) as qkp,
            tc.tile_pool(name='rel', bufs=1) as relp,
            tc.tile_pool(name='v', bufs=1) as vp,
            tc.tile_pool(name='mlp', bufs=1) as mlpp,
            tc.tile_pool(name='small', bufs=1) as smallp,
            tc.tile_pool(name='ps', bufs=1, space='PSUM') as psp,
            tc.tile_pool(name='psx', bufs=1, space='PSUM') as psxp,
            tc.tile_pool(name='dram', bufs=1, space='DRAM') as dramp,
        ):
            # ---- AllGather feat row shards and the weight blob ----
            featfull = [dramp.tile([NROW, C], BF, name=f'featfull{_b}')
                        for _b in range(B)]
            for _b in range(B):
                fstage = gatp.tile([128, 256], BF, name='fstage')
                ccf_in = dramp.tile([128, 256], BF, name=f'ccf_in{_b}')
                nc.sync.dma_start(out=fstage[:], in_=feati[_b, :, :])
                nc.gpsimd.dma_start(out=ccf_in[:], in_=fstage[:])
                nc.gpsimd.collective_compute(
                    'AllGather', mybir.AluOpType.bypass,
                    replica_groups=[list(range(NCORES))],
                    ins=[ccf_in.opt()], outs=[featfull[_b].opt()],
                )
            wfull = dramp.tile([WBLOB], BF, name='wfull')
            wstage = gatp.tile([128, WSH // 128], BF, name='wstage')
            wcc_in = dramp.tile([128, WSH // 128], BF, name='wcc_in')
            nc.sync.dma_start(out=wstage[:], in_=wblob[:, :])
            nc.gpsimd.dma_start(out=wcc_in[:], in_=wstage[:])
            nc.gpsimd.collective_compute(
                'AllGather', mybir.AluOpType.bypass,
                replica_groups=[list(range(NCORES))],
                ins=[wcc_in.opt()], outs=[wfull.opt()],
            )

            # ---- constant weights to SBUF (from the gathered blob) ----
            wq_sb = constp.tile([3, 64], BF)
            wk_sb = constp.tile([65, 64], BF)
            wv_sb = constp.tile([64, 64], BF)
            bv_sb = constp.tile([64, 1], F32)
            w00o_sb = constp.tile([64, 4 * 256], BF)
            w00f_sb = constp.tile([65, 4 * 256], BF)
            w1_sb = constp.tile([128, 2, 256], BF)
            b1_sb = constp.tile([128, 2], F32)
            w2_sb = constp.tile([128, 2], BF)
            nc.sync.dma_start(out=wq_sb[:], in_=wfull[WOFF_WQ:WOFF_WK])
            nc.sync.dma_start(out=wk_sb[:], in_=wfull[WOFF_WK:WOFF_WV])
            nc.sync.dma_start(out=wv_sb[:], in_=wfull[WOFF_WV:WOFF_W00O])
            nc.sync.dma_start(out=bv_sb[:], in_=bv[:, :])
            for t in range(4):
                nc.sync.dma_start(
                    out=w00o_sb[:, t * 256:(t + 1) * 256],
                    in_=wfull[WOFF_W00O + t * 16384:WOFF_W00O + (t + 1) * 16384])
                nc.sync.dma_start(
                    out=w00f_sb[:, t * 256:(t + 1) * 256],
                    in_=wfull[WOFF_W00F + t * 16640:WOFF_W00F + (t + 1) * 16640])
            for kk in range(2):
                nc.sync.dma_start(
                    out=w1_sb[:, kk, :],
                    in_=wfull[WOFF_W1 + kk * 32768:WOFF_W1 + (kk + 1) * 32768])
                nc.sync.dma_start(out=b1_sb[:, kk:kk + 1], in_=b1[kk, :, :])
                nc.sync.dma_start(
                    out=w2_sb[:, kk:kk + 1],
                    in_=wfull[WOFF_W2 + kk * 128:WOFF_W2 + (kk + 1) * 128])

            Sp_sb = constp.tile([64, NU * 64], F32)   # partial logits, all units

            # =========== phases 1+2 per batch: gather, fs, q/k, S ===========
            from concourse.masks import make_identity
            ident_sb = constp.tile([128, 128], BF)
            make_identity(nc, ident_sb[:])


            def gather_fs(b, fs_tiles):
                idx16_sb = gatp.tile([128, 4 * 64], I16)
                wsm_sb = gatp.tile([128, 4 * 64], BF)
                idx_sb = gatp.tile([128, 4 * 64], dt.int32)
                nc.sync.dma_start(out=idx16_sb[:], in_=idx[b, :, :])
                nc.sync.dma_start(out=wsm_sb[:], in_=wsm[b, :, :])
                nc.vector.tensor_copy(out=idx_sb[:], in_=idx16_sb[:])
                # quarters of 16 pixel-tiles x 4 corners: one indirect DMA and
                # one broadcast multiply each, double-buffered
                for q in range(4):
                    g_pm = gatp.tile([128, 16 * 4, C], BF, name=f'g_pm{q % 2}')
                    for o in range(64):
                        nc.gpsimd.indirect_dma_start(
                            out=g_pm[:, o, :], out_offset=None,
                            in_=featfull[b][:, :],
                            in_offset=bass.IndirectOffsetOnAxis(
                                ap=idx_sb[:, q * 64 + o:q * 64 + o + 1], axis=0))
                    wap = wsm_sb[:, q * 64:(q + 1) * 64]
                    wbc = bass.AP(wap.tensor, wap.offset, wap.ap + [(0, C)])
                    nc.vector.tensor_tensor(out=g_pm[:, :, :],
                                            in0=g_pm[:, :, :], in1=wbc,
                                            op=ALU.mult)
                    for t in range(4):
                        for jg in range(4):
                            tp_ps = psp.tile([64, 512], BF)
                            for jj in range(4):
                                jl = jg * 4 + jj
                                nc.tensor.transpose(
                                    out=tp_ps[:, jj * 128:(jj + 1) * 128],
                                    in_=g_pm[:, jl * 4 + t, :],
                                    identity=ident_sb[:])
                            goff = (q * 16 + jg * 4) * 128
                            nc.scalar.copy(
                                out=fs_tiles[t][0:64, goff:goff + 512],
                                in_=tp_ps[:])
                for _t in range(4):
                    nc.vector.memset(fs_tiles[_t][64:65, :], 1.0)

            fs_spill = [[dramp.tile([65, NLOC], BF, name=f'fsspill{_b}_{_t}')
                         for _t in range(4)] for _b in range(B)]
            for b in range(B):
                fs_tiles = [fsp.tile([65, NLOC], BF, name=f'fs{_t}') for _t in range(4)]
                gather_fs(b, fs_tiles)

                for t in range(4):
                    rel8_sb = relp.tile([3, NLOC], dt.int8, name='rel8')
                    nc.sync.dma_start(out=rel8_sb[:], in_=relq[b, t, :, :])
                    rel_sb = relp.tile([3, NLOC], BF)
                    nc.vector.tensor_copy(out=rel_sb[:], in_=rel8_sb[:])
                    qT_sb = qkp.tile([128, 64 * 64], BF)
                    kT_sb = qkp.tile([128, 64 * 64], BF)
                    s_ps = psp.tile([64, 64], F32, name='s_ps')
                    for jg in range(8):          # groups of 8 pixel-tiles
                        q_ps = psp.tile([128, 512], F32)
                        k_ps = psp.tile([128, 512], F32)
                        for jj in range(8):
                            j = jg * 8 + jj
                            nc.tensor.matmul(
                                out=q_ps[:, jj * 64:(jj + 1) * 64],
                                lhsT=rel_sb[:, j * 128:(j + 1) * 128],
                                rhs=wq_sb[:], start=True, stop=True)
                            nc.tensor.matmul(
                                out=k_ps[:, jj * 64:(jj + 1) * 64],
                                lhsT=fs_tiles[t][:, j * 128:(j + 1) * 128],
                                rhs=wk_sb[:], start=True, stop=True)
                        gsl = slice(jg * 512, (jg + 1) * 512)
                        nc.scalar.activation(out=qT_sb[:, gsl], in_=q_ps[:], func=AF.Relu)
                        nc.vector.tensor_scalar_max(out=kT_sb[:, gsl], in0=k_ps[:], scalar1=0.0)
                    for j in range(64):
                        nc.tensor.matmul(
                            out=s_ps[:],
                            lhsT=qT_sb[:, j * 64:(j + 1) * 64],
                            rhs=kT_sb[:, j * 64:(j + 1) * 64],
                            start=(j == 0), stop=(j == 63))
                    u = b * 4 + t
                    nc.vector.tensor_copy(out=Sp_sb[:, u * 64:(u + 1) * 64], in_=s_ps[:])
                for t in range(4):
                    nc.sync.dma_start(out=fs_spill[b][t][:, :], in_=fs_tiles[t][:])

            # =========== phase 3: AllReduce of logits ===========
            cc_in = dramp.tile([64, NU * 64], F32)
            cc_out = dramp.tile([64, NU * 64], F32)
            nc.gpsimd.dma_start(out=cc_in[:], in_=Sp_sb[:])
            nc.gpsimd.collective_compute(
                'AllReduce', mybir.AluOpType.add,
                replica_groups=[list(range(NCORES))],
                ins=[cc_in.opt()], outs=[cc_out.opt()],
            )
            S_sb = constp.tile([64, NU * 64], F32)
            nc.gpsimd.dma_start(out=S_sb[:], in_=cc_out[:])

            # =========== phase 4: softmax + A_t^T ===========
            attn_sb = constp.tile([64, NU * 64], BF)
            AT_tiles = []
            for u in range(NU):
                usl = slice(u * 64, (u + 1) * 64)
                mx = smallp.tile([64, 1], F32)
                nmx = smallp.tile([64, 1], F32)
                ex = smallp.tile([64, 64], F32)
                sm = smallp.tile([64, 1], F32)
                rs = smallp.tile([64, 1], F32)
                nc.vector.tensor_reduce(out=mx[:], in_=S_sb[:, usl],
                                        axis=mybir.AxisListType.X, op=ALU.max)
                nc.vector.tensor_scalar_mul(out=nmx[:], in0=mx[:], scalar1=-1.0)
                nc.scalar.activation(out=ex[:], in_=S_sb[:, usl], func=AF.Exp,
                                     bias=nmx[:, 0:1])
                nc.vector.tensor_reduce(out=sm[:], in_=ex[:],
                                        axis=mybir.AxisListType.X, op=ALU.add)
                nc.vector.reciprocal(out=rs[:], in_=sm[:])
                nc.vector.tensor_scalar_mul(out=attn_sb[:, usl], in0=ex[:],
                                            scalar1=rs[:, 0:1])
            for b in range(B):
                for t in range(4):
                    u = b * 4 + t
                    a_full = psp.tile([64, 512], F32, name='misc_ps')
                    a_ps = a_full[:, 0:256]
                    nc.tensor.matmul(out=a_ps,
                                     lhsT=attn_sb[:, u * 64:(u + 1) * 64],
                                     rhs=w00o_sb[:, t * 256:(t + 1) * 256],
                                     start=True, stop=True)
                    at = constp.tile([65, 256], BF, name=f'at{b}_{t}')
                    nc.vector.tensor_copy(out=at[0:64, :], in_=a_ps)
                    if t == 0:
                        nc.sync.dma_start(
                            out=at[64:65, :],
                            in_=wfull[WOFF_B00 + b * 256:WOFF_B00 + (b + 1) * 256])
                    AT_tiles.append(at)

            # =========== phase 5: regather + MLP ===========
            for b in range(B):
                fs_tiles = [fsp.tile([65, NLOC], BF, name=f'fs{_t}') for _t in range(4)]
                for t in range(4):
                    nc.sync.dma_start(out=fs_tiles[t][:], in_=fs_spill[b][t][:, :])
                bil_sb = smallp.tile([1, NLOC], BF, name='bil_sb')
                nc.sync.dma_start(out=bil_sb[:], in_=bil[b, :][None, :])

                for pc in range(NLOC // PCH):
                    psl = slice(pc * PCH, (pc + 1) * PCH)
                    # transient v tiles for this pixel super-chunk
                    v_tiles = []
                    for t in range(4):
                        vt = vp.tile([65, PCH], BF, name=f'vt{t}')
                        nc.vector.memset(vt[64:65, :], 1.0)
                        for cc in range(PCH // CHUNK):
                            vsl_l = slice(cc * CHUNK, (cc + 1) * CHUNK)
                            vsl_g = slice(pc * PCH + cc * CHUNK, pc * PCH + (cc + 1) * CHUNK)
                            v_ps = psp.tile([64, CHUNK], F32)
                            nc.tensor.matmul(out=v_ps[:], lhsT=wv_sb[:],
                                             rhs=fs_tiles[t][0:64, vsl_g],
                                             start=True, stop=True)
                            nc.scalar.activation(out=vt[0:64, vsl_l], in_=v_ps[:],
                                                 func=AF.Relu, bias=bv_sb[:, 0:1])
                        v_tiles.append(vt)

                    x1_t = [mlpp.tile([128, PCH], BF, name=f'x1_{_m}') for _m in range(2)]
                    x2_t = [mlpp.tile([128, PCH], BF, name=f'x2_{_m}') for _m in range(2)]
                    for cc in range(PCH // CHUNK):
                        lsl = slice(cc * CHUNK, (cc + 1) * CHUNK)
                        gsl = slice(pc * PCH + cc * CHUNK, pc * PCH + (cc + 1) * CHUNK)
                        for m in range(2):
                            msl = slice(m * 128, (m + 1) * 128)
                            x_ps = psxp.tile([128, CHUNK], F32)
                            for t in range(4):
                                nc.tensor.matmul(
                                    out=x_ps[:],
                                    lhsT=w00f_sb[:, t * 256 + m * 128: t * 256 + (m + 1) * 128],
                                    rhs=fs_tiles[t][:, gsl],
                                    start=(t == 0), stop=False)
                            for t in range(4):
                                at = AT_tiles[b * 4 + t]
                                kk = 65 if t == 0 else 64
                                nc.tensor.matmul(
                                    out=x_ps[:],
                                    lhsT=at[0:kk, msl],
                                    rhs=v_tiles[t][0:kk, lsl],
                                    start=False, stop=(t == 3))
                            nc.vector.tensor_copy(out=x1_t[m][:, lsl], in_=x_ps[:])
                        # W1 + gelu
                        for m in range(2):
                            msl = slice(m * 128, (m + 1) * 128)
                            x2_ps = psxp.tile([128, CHUNK], F32)
                            for kk in range(2):
                                nc.tensor.matmul(out=x2_ps[:],
                                                 lhsT=w1_sb[:, kk, msl],
                                                 rhs=x1_t[kk][:, lsl],
                                                 start=(kk == 0), stop=(kk == 1))
                            nc.scalar.activation(out=x2_t[m][:, lsl], in_=x2_ps[:],
                                                 func=AF.Gelu, bias=b1_sb[:, m:m + 1])
                        # W2 + bil add
                        o_full = psp.tile([64, 512], F32, name='misc_ps')
                        o_ps = o_full[0:1, :]
                        for kk in range(2):
                            nc.tensor.matmul(out=o_ps, lhsT=w2_sb[:, kk:kk + 1],
                                             rhs=x2_t[kk][:, lsl],
                                             start=(kk == 0), stop=(kk == 1))
                        o_sb = smallp.tile([1, CHUNK], BF)
                        nc.vector.tensor_tensor(out=o_sb[:], in0=o_ps,
                                                in1=bil_sb[:, gsl], op=ALU.add)
                        nc.sync.dma_start(out=out[b, gsl][None, :], in_=o_sb[:])

    nc.compile()
    return nc


# --------------------------------------------------------------------------

def kernel(**inputs) -> np.ndarray:
    from concourse.bass_utils import run_bass_kernel_spmd
    in_maps = _host_prep(inputs)
    nc = _build()
    res = run_bass_kernel_spmd(nc, in_maps, core_ids=list(range(NCORES)))
    full = np.empty((B, 1, HQ, WQ), np.float32)
    flat = full.reshape(B, NPB)
    for cidx in range(NCORES):
        flat[:, cidx * NLOC:(cidx + 1) * NLOC] = \
            res.results[cidx]['out'].astype(np.float32)
    return full


# revision 48
# speedup vs baseline: 1305.7416x; 1.4553x over previous
"""Trainium2 Bass kernel for nn_AnyTSRpp (sparse_attention).

Strategy: pure data-parallel over the HR pixel grid (65536 px/batch),
8192 px/batch/core on 8 NeuronCores. Host computes per-pixel corner
indices/scalars; device gathers feat rows directly (per-corner indirect
DMA, pixel-major), applies the RBF weight per-partition pre-transpose,
PE transposes to channel-major, runs all matmuls/relu/softmax/gelu, and
a tiny AllReduce for the global attention logits (contraction over all
pixels). off_t = attn_t @ v_t is folded as (W00_off_t @ attn_t) @ v_t
so the attention output is never materialized.

Self-contained: hardcodes all shapes. kernel(**inputs) -> np.ndarray.
"""

import functools
import numpy as np
import ml_dtypes

BF16 = ml_dtypes.bfloat16


def _setup_jax_cache():
    """Persistent XLA compilation cache: repeated/every-process calls skip
    the neuronx-cc recompile of the identical kernel graph."""
    import jax
    try:
        jax.config.update('jax_compilation_cache_dir', '/root/.cache/jax_pcache')
        jax.config.update('jax_persistent_cache_min_compile_time_secs', 0.0)
        jax.config.update('jax_persistent_cache_min_entry_size_bytes', 0)
    except Exception:
        pass


_setup_jax_cache()

NCORES = 8
B = 2
C = 64
HLR = WLR = 64
HQ = WQ = 256
NPB = HQ * WQ            # 65536 pixels per batch
NLOC = NPB // NCORES     # 8192 pixels per batch per core
NROW = HLR * WLR         # 4096 feat rows (y-major)
CHUNK = 512              # matmul moving-N chunk
NCHUNK = NLOC // CHUNK   # 16
PCH = 1024               # MLP pixel super-chunk
EPS = np.float32(1e-6)

# bf16 weight blob layout (flat element offsets)
WOFF_WQ = 0                        # [3, 64]
WOFF_WK = WOFF_WQ + 3 * 64         # [65, 64]
WOFF_WV = WOFF_WK + 65 * 64        # [64, 64]
WOFF_W00O = WOFF_WV + 64 * 64      # [4, 64, 256]
WOFF_W00F = WOFF_W00O + 4 * 64 * 256   # [4, 65, 256]
WOFF_B00 = WOFF_W00F + 4 * 65 * 256    # [B, 1, 256]
WOFF_W1 = WOFF_B00 + B * 256       # [2, 128, 256]
WOFF_W2 = WOFF_W1 + 2 * 128 * 256  # [2, 128, 1]
WBLOB = WOFF_W2 + 2 * 128          # 206848 = 8 * 25856 = 8 * 128 * 202
WSH = WBLOB // NCORES


# --------------------------------------------------------------------------
# host-side math (mirrors reference semantics in f32)
# --------------------------------------------------------------------------

def _corner_indices(co):
    """co: [N] f32 coords in one axis. Returns (base j in [0,65], i_minus,
    i_plus) exactly matching the reference's per-corner nearest indices."""
    # reference: c_t = clip(co + v/64 + eps, -1+1e-6, 1-1e-6);
    #            i_t = clip(round((c_t+1)*32 - 0.5), 0, 63)
    out = []
    for v in (-1.0, 1.0):
        c = np.clip(co + np.float32(v / 64.0) + EPS,
                    np.float32(-1 + 1e-6), np.float32(1 - 1e-6))
        i = np.clip(np.round((c + 1) * np.float32(32.0) - np.float32(0.5)),
                    0, 63).astype(np.int32)
        out.append(i)
    im, ip = out
    # padded-table base: j = clip(floor(ay), -1, 64) + 1, ay = 32*(co+eps)+31.5
    ay = (co + EPS) * np.float32(32.0) + np.float32(31.5)
    j = np.clip(np.floor(ay), -1, 64).astype(np.int32) + 1
    return j, im, ip


def _host_prep(inputs):
    feat = np.asarray(inputs['feat'], np.float32)
    inp = np.asarray(inputs['inp'], np.float32)
    coord = np.asarray(inputs['coord'], np.float32)
    cell = np.asarray(inputs['cell'], np.float32)
    scale = np.asarray(inputs['scale'], np.float32)
    Wq = np.asarray(inputs['Wq'], np.float32); bq = np.asarray(inputs['bq'], np.float32)
    Wk = np.asarray(inputs['Wk'], np.float32); bk = np.asarray(inputs['bk'], np.float32)
    Wv = np.asarray(inputs['Wv'], np.float32); bv = np.asarray(inputs['bv'], np.float32)
    W00 = np.asarray(inputs['W00'], np.float32); b00 = np.asarray(inputs['b00'], np.float32)
    W1 = np.asarray(inputs['W1'], np.float32); b1 = np.asarray(inputs['b1'], np.float32)
    W2 = np.asarray(inputs['W2'], np.float32); b2 = np.asarray(inputs['b2'], np.float32)
    ls = np.asarray(inputs['ls'], np.float32)

    # feat as bf16 rows [B, 4096, 64]: row iy*64+ix = feat[b, :, iy, ix]
    featrows = np.ascontiguousarray(
        feat.transpose(0, 2, 3, 1).reshape(B, NROW, C)).astype(BF16)

    coord_y = coord[..., 0].reshape(B, NPB)
    coord_x = coord[..., 1].reshape(B, NPB)

    # per-(b) padded-table base index; per-corner rel offsets + RBF weights
    idx_all = np.empty((B, NPB), np.int16)
    rel_all = np.empty((B, 4, 2, NPB), BF16)   # [rel_y, rel_x]
    w_all = np.empty((B, 4, NPB), BF16)
    hw = np.float32(64.0)
    ls2 = ls[0] * ls[0]
    for b in range(B):
        jy, iym, iyp = _corner_indices(coord_y[b])
        jx, ixm, ixp = _corner_indices(coord_x[b])
        idx_all[b] = (jy * np.int32(66) + jx).astype(np.int16)
        iy = {-1: iym, 1: iyp}
        ix = {-1: ixm, 1: ixp}
        t = 0
        for vx in (-1, 1):          # y offset
            for vy in (-1, 1):      # x offset
                oy = (iy[vx].astype(np.float32) + np.float32(0.5)) / np.float32(32.0) - 1
                ox = (ix[vy].astype(np.float32) + np.float32(0.5)) / np.float32(32.0) - 1
                ry = coord_y[b] - oy
                rx = coord_x[b] - ox
                rel_all[b, t, 0] = ry.astype(BF16)
                rel_all[b, t, 1] = rx.astype(BF16)
                rd = (ry * hw) ** 2 + (rx * hw) ** 2
                w_all[b, t] = np.exp(rd / ls2 * np.float32(-0.5)).astype(BF16)
                t += 1

    # ---- bilinear sample of inp (border, align_corners=False) + b2 ----
    bil = np.empty((B, NPB), BF16)
    for b in range(B):
        im = inp[b, 0]
        y = np.clip((coord_y[b] + 1) * np.float32(32.0) - np.float32(0.5), 0.0, 63.0)
        x = np.clip((coord_x[b] + 1) * np.float32(32.0) - np.float32(0.5), 0.0, 63.0)
        y0 = np.floor(y); x0 = np.floor(x)
        wy = (y - y0).astype(np.float32); wx = (x - x0).astype(np.float32)
        y0i = np.clip(y0.astype(np.int32), 0, 63)
        y1i = np.clip(y0.astype(np.int32) + 1, 0, 63)
        x0i = np.clip(x0.astype(np.int32), 0, 63)
        x1i = np.clip(x0.astype(np.int32) + 1, 0, 63)
        v00 = im[y0i, x0i]; v01 = im[y0i, x1i]
        v10 = im[y1i, x0i]; v11 = im[y1i, x1i]
        bil[b] = ((v00 * (1 - wy) * (1 - wx) + v01 * (1 - wy) * wx
                   + v10 * wy * (1 - wx) + v11 * wy * wx) + b2[0]).astype(BF16)

    # ---- rel -> int8 with the dequant scale folded into Wq's rel rows ----
    # row 2 is a constant 1 so the int8->bf16 widen also produces the
    # bias row for the q matmul.
    relmax = float(np.max(np.abs(rel_all.astype(np.float32)))) or 1.0
    QK = np.float32(127.0 / relmax)
    rel8 = np.ones((B, 4, 3, NPB), np.int8)
    rel8[:, :, 0:2] = np.clip(np.round(rel_all.astype(np.float32) * QK),
                              -127, 127).astype(np.int8)

    # ---- weight repacks ----
    wq_rhs = np.concatenate([Wq.T / QK, bq[None, :]], axis=0).astype(BF16)  # [3, 64]
    wk_rhs = np.concatenate([Wk.T, bk[None, :]], axis=0).astype(BF16)       # [65, 64]
    wv_lhsT = Wv.T.astype(BF16)                                             # [64, 64]
    w00off_rhs = np.stack([W00[:, t * 64:(t + 1) * 64].T for t in range(4)]
                          ).astype(BF16)                                    # [4, 64, 256]
    w00fs_lhsT = np.stack(
        [np.concatenate([W00[:, 256 + t * 64: 256 + (t + 1) * 64].T,
                         np.zeros((1, 256), np.float32)], axis=0)
         for t in range(4)]).astype(BF16)                                   # [4, 65, 256]
    b00eff = np.empty((B, 1, 256), BF16)
    for b in range(B):
        vec4 = np.concatenate([cell[b] * hw, scale[b]]).astype(np.float32)
        b00eff[b, 0] = (b00 + W00[:, 512:516] @ vec4).astype(BF16)
    w1_lhsT = np.ascontiguousarray(W1.T.astype(BF16).reshape(2, 128, 256))  # [2, 128, 256]
    w2_lhsT = np.ascontiguousarray(W2.T.astype(BF16).reshape(2, 128, 1))    # [2, 128, 1]

    # ---- bf16 weight blob (AllGathered on device): flat row-major concat ----
    wflat = np.concatenate([
        wq_rhs.reshape(-1), wk_rhs.reshape(-1), wv_lhsT.reshape(-1),
        w00off_rhs.reshape(-1), w00fs_lhsT.reshape(-1), b00eff.reshape(-1),
        w1_lhsT.reshape(-1), w2_lhsT.reshape(-1)])
    assert wflat.size == WBLOB, wflat.size

    # ---- shard per core ----
    NFS = NROW // NCORES     # 512 feat rows per core shard (AllGathered on device)
    in_maps = []
    for cidx in range(NCORES):
        sl = slice(cidx * NLOC, (cidx + 1) * NLOC)
        # pixel-major tiles: local pixel j*128+p at [p, j]; each gathered
        # table row holds all 4 corners (c00|c01|c10|c11), so wsm is laid
        # out corner-minor [p, j*4+t] to broadcast-multiply the row.
        idx2d = np.ascontiguousarray(
            idx_all[:, sl].reshape(B, 64, 128).transpose(0, 2, 1))
        wsm2d = np.ascontiguousarray(
            w_all[:, :, sl].reshape(B, 4, 64, 128).transpose(0, 3, 2, 1)
            .reshape(B, 128, 4 * 64))
        m = {
            'feati': np.ascontiguousarray(
                featrows[:, cidx * NFS:(cidx + 1) * NFS, :]).reshape(B, 128, 256),
            'wblob': np.ascontiguousarray(
                wflat[cidx * WSH:(cidx + 1) * WSH]).reshape(128, WSH // 128),
            'idx': idx2d,
            'wsm': wsm2d,
            'relq': np.ascontiguousarray(rel8[:, :, :, sl]),
            'bil': np.ascontiguousarray(bil[:, sl]),
            'bv': bv.reshape(64, 1).astype(np.float32),
            'b1': np.ascontiguousarray(b1.astype(np.float32).reshape(2, 128, 1)),
        }
        in_maps.append(m)
    return in_maps


# --------------------------------------------------------------------------
# device kernel
# --------------------------------------------------------------------------

@functools.lru_cache(maxsize=1)
def _build():
    import concourse.bass as bass
    import concourse.tile as tile
    from concourse import bacc, mybir
    dt = mybir.dt
    F32, BF, I16 = dt.float32, dt.bfloat16, dt.int16
    AF = mybir.ActivationFunctionType
    ALU = mybir.AluOpType

    nc = bacc.Bacc(None, target_bir_lowering=False)

    feati = nc.dram_tensor('feati', [B, 128, 256], BF, kind='ExternalInput')
    wblob = nc.dram_tensor('wblob', [128, WSH // 128], BF, kind='ExternalInput')
    idx = nc.dram_tensor('idx', [B, 128, 64], I16, kind='ExternalInput')
    wsm = nc.dram_tensor('wsm', [B, 128, 4 * 64], BF, kind='ExternalInput')
    relq = nc.dram_tensor('relq', [B, 4, 3, NLOC], dt.int8, kind='ExternalInput')
    bil = nc.dram_tensor('bil', [B, NLOC], BF, kind='ExternalInput')
    bv = nc.dram_tensor('bv', [64, 1], F32, kind='ExternalInput')
    b1 = nc.dram_tensor('b1', [2, 128, 1], F32, kind='ExternalInput')
    out = nc.dram_tensor('out', [B, NLOC], BF, kind='ExternalOutput')

    NU = B * 4  # 8 attention units

    with tile.TileContext(nc) as tc:
        with (
            tc.tile_pool(name='const', bufs=1) as constp,
            tc.tile_pool(name='fs', bufs=1) as fsp,
            tc.tile_pool(name='gat', bufs=1) as gatp,
            tc.tile_pool(name='qk', bufs=1) as qkp,
            tc.tile_pool(name='rel', bufs=1) as relp,
            tc.tile_pool(name='v', bufs=1) as vp,
            tc.tile_pool(name='mlp', bufs=1) as mlpp,
            tc.tile_pool(name='small', bufs=1) as smallp,
            tc.tile_pool(name='ps', bufs=1, space='PSUM') as psp,
            tc.tile_pool(name='psx', bufs=1, space='PSUM') as psxp,
            tc.tile_pool(name='dram', bufs=1, space='DRAM') as dramp,
        ):
            # ---- AllGather feat row shards and the weight blob ----
            featfull = [dramp.tile([NROW, C], BF, name=f'featfull{_b}')
                        for _b in range(B)]
            for _b in range(B):
                fstage = gatp.tile([128, 256], BF, name='fstage')
                ccf_in = dramp.tile([128, 256], BF, name=f'ccf_in{_b}')
                nc.sync.dma_start(out=fstage[:], in_=feati[_b, :, :])
                nc.gpsimd.dma_start(out=ccf_in[:], in_=fstage[:])
                nc.gpsimd.collective_compute(
                    'AllGather', mybir.AluOpType.bypass,
                    replica_groups=[list(range(NCORES))],
                    ins=[ccf_in.opt()], outs=[featfull[_b].opt()],
                )
            wfull = dramp.tile([WBLOB], BF, name='wfull')
            wstage = gatp.tile([128, WSH // 128], BF, name='wstage')
            wcc_in = dramp.tile([128, WSH // 128], BF, name='wcc_in')
            nc.sync.dma_start(out=wstage[:], in_=wblob[:, :])
            nc.gpsimd.dma_start(out=wcc_in[:], in_=wstage[:])
            nc.gpsimd.collective_compute(
                'AllGather', mybir.AluOpType.bypass,
                replica_groups=[list(range(NCORES))],
                ins=[wcc_in.opt()], outs=[wfull.opt()],
            )

            # ---- 66x66 edge-replicated 2x2-patch table, built on device ----
            # ptable[b][jy*66+jx] = [c00|c01|c10|c11],
            # c(dy,dx) = feat[b, :, clip(jy-1+dy,0,63), clip(jx-1+dx,0,63)]
            NTAB = 66 * 66
            ptable = [dramp.tile([NTAB, 256], BF, name=f'ptable{_b}')
                      for _b in range(B)]
            for _b in range(B):
                pt_t = ptable[_b][:, :].tensor
                ff_t = featfull[_b][:, :].tensor
                for dy in (0, 1):
                    yr = ([(0, 1, 0), (1, 64, 0), (65, 1, 63)] if dy == 0
                          else [(0, 64, 0), (64, 2, 63)])
                    for dx in (0, 1):
                        xr = ([(0, 1, 0), (1, 64, 0), (65, 1, 63)] if dx == 0
                              else [(0, 64, 0), (64, 2, 63)])
                        qoff = (dy * 2 + dx) * 64
                        for (jy0, ny, sy0) in yr:
                            for (jx0, nx, sx0) in xr:
                                dst = bass.AP(
                                    pt_t, (jy0 * 66 + jx0) * 256 + qoff,
                                    [(66 * 256, ny), (256, nx), (1, 64)])
                                src = bass.AP(
                                    ff_t, (sy0 * 64 + sx0) * 64,
                                    [(4096 if ny > 1 and sy0 == 0 else 0, ny),
                                     (64 if nx > 1 and sx0 == 0 else 0, nx),
                                     (1, 64)])
                                nc.sync.dma_start(out=dst, in_=src)

            # ---- constant weights to SBUF (from the gathered blob) ----
            wq_sb = constp.tile([3, 64], BF)
            wk_sb = constp.tile([65, 64], BF)
            wv_sb = constp.tile([64, 64], BF)
            bv_sb = constp.tile([64, 1], F32)
            w00o_sb = constp.tile([64, 4 * 256], BF)
            w00f_sb = constp.tile([65, 4 * 256], BF)
            w1_sb = constp.tile([128, 2, 256], BF)
            b1_sb = constp.tile([128, 2], F32)
            w2_sb = constp.tile([128, 2], BF)
            nc.sync.dma_start(out=wq_sb[:], in_=wfull[WOFF_WQ:WOFF_WK])
            nc.sync.dma_start(out=wk_sb[:], in_=wfull[WOFF_WK:WOFF_WV])
            nc.sync.dma_start(out=wv_sb[:], in_=wfull[WOFF_WV:WOFF_W00O])
            nc.sync.dma_start(out=bv_sb[:], in_=bv[:, :])
            for t in range(4):
                nc.sync.dma_start(
                    out=w00o_sb[:, t * 256:(t + 1) * 256],
                    in_=wfull[WOFF_W00O + t * 16384:WOFF_W00O + (t + 1) * 16384])
                nc.sync.dma_start(
                    out=w00f_sb[:, t * 256:(t + 1) * 256],
                    in_=wfull[WOFF_W00F + t * 16640:WOFF_W00F + (t + 1) * 16640])
            for kk in range(2):
                nc.sync.dma_start(
                    out=w1_sb[:, kk, :],
                    in_=wfull[WOFF_W1 + kk * 32768:WOFF_W1 + (kk + 1) * 32768])
                nc.sync.dma_start(out=b1_sb[:, kk:kk + 1], in_=b1[kk, :, :])
                nc.sync.dma_start(
                    out=w2_sb[:, kk:kk + 1],
                    in_=wfull[WOFF_W2 + kk * 128:WOFF_W2 + (kk + 1) * 128])

            Sp_sb = constp.tile([64, NU * 64], F32)   # partial logits, all units

            # =========== phases 1+2 per batch: gather, fs, q/k, S ===========
            from concourse.masks import make_identity
            ident_sb = constp.tile([128, 128], BF)
            make_identity(nc, ident_sb[:])


            def gather_fs(b, fs_tiles):
                idx16_sb = gatp.tile([128, 64], I16)
                wsm_sb = gatp.tile([128, 4 * 64], BF)
                idx_sb = gatp.tile([128, 64], dt.int32)
                nc.sync.dma_start(out=idx16_sb[:], in_=idx[b, :, :])
                nc.sync.dma_start(out=wsm_sb[:], in_=wsm[b, :, :])
                nc.vector.tensor_copy(out=idx_sb[:], in_=idx16_sb[:])
                # quarters of 16 pixel-tiles; each gathered 512B table row
                # carries all 4 corners; one broadcast multiply per quarter
                for q in range(4):
                    g_pm = gatp.tile([128, 16, 4 * C], BF, name=f'g_pm{q % 2}')
                    for o in range(16):
                        nc.gpsimd.indirect_dma_start(
                            out=g_pm[:, o, :], out_offset=None,
                            in_=ptable[b][:, :],
                            in_offset=bass.IndirectOffsetOnAxis(
                                ap=idx_sb[:, q * 16 + o:q * 16 + o + 1], axis=0))
                    wap = wsm_sb[:, q * 64:(q + 1) * 64]
                    wbc = bass.AP(wap.tensor, wap.offset, wap.ap + [(0, C)])
                    nc.vector.tensor_tensor(out=g_pm[:, :, :],
                                            in0=g_pm[:, :, :], in1=wbc,
                                            op=ALU.mult)
                    for t in range(4):
                        for jg in range(4):
                            tp_ps = psp.tile([64, 512], BF)
                            for jj in range(4):
                                jl = jg * 4 + jj
                                nc.tensor.transpose(
                                    out=tp_ps[:, jj * 128:(jj + 1) * 128],
                                    in_=g_pm[:, jl, t * C:(t + 1) * C],
                                    identity=ident_sb[:])
                            goff = (q * 16 + jg * 4) * 128
                            nc.scalar.copy(
                                out=fs_tiles[t][0:64, goff:goff + 512],
                                in_=tp_ps[:])
                for _t in range(4):
                    nc.vector.memset(fs_tiles[_t][64:65, :], 1.0)

            fs_spill = [[dramp.tile([65, NLOC], BF, name=f'fsspill{_b}_{_t}')
                         for _t in range(4)] for _b in range(B)]
            for b in range(B):
                fs_tiles = [fsp.tile([65, NLOC], BF, name=f'fs{_t}') for _t in range(4)]
                gather_fs(b, fs_tiles)

                for t in range(4):
                    rel8_sb = relp.tile([3, NLOC], dt.int8, name='rel8')
                    nc.sync.dma_start(out=rel8_sb[:], in_=relq[b, t, :, :])
                    rel_sb = relp.tile([3, NLOC], BF)
                    nc.vector.tensor_copy(out=rel_sb[:], in_=rel8_sb[:])
                    qT_sb = qkp.tile([128, 64 * 64], BF)
                    kT_sb = qkp.tile([128, 64 * 64], BF)
                    s_ps = psp.tile([64, 64], F32, name='s_ps')
                    for jg in range(8):          # groups of 8 pixel-tiles
                        q_ps = psp.tile([128, 512], F32)
                        k_ps = psp.tile([128, 512], F32)
                        for jj in range(8):
                            j = jg * 8 + jj
                            nc.tensor.matmul(
                                out=q_ps[:, jj * 64:(jj + 1) * 64],
                                lhsT=rel_sb[:, j * 128:(j + 1) * 128],
                                rhs=wq_sb[:], start=True, stop=True)
                            nc.tensor.matmul(
                                out=k_ps[:, jj * 64:(jj + 1) * 64],
                                lhsT=fs_tiles[t][:, j * 128:(j + 1) * 128],
                                rhs=wk_sb[:], start=True, stop=True)
                        gsl = slice(jg * 512, (jg + 1) * 512)
                        nc.scalar.activation(out=qT_sb[:, gsl], in_=q_ps[:], func=AF.Relu)
                        nc.vector.tensor_scalar_max(out=kT_sb[:, gsl], in0=k_ps[:], scalar1=0.0)
                    for j in range(64):
                        nc.tensor.matmul(
                            out=s_ps[:],
                            lhsT=qT_sb[:, j * 64:(j + 1) * 64],
                            rhs=kT_sb[:, j * 64:(j + 1) * 64],
                            start=(j == 0), stop=(j == 63))
                    u = b * 4 + t
                    nc.vector.tensor_copy(out=Sp_sb[:, u * 64:(u + 1) * 64], in_=s_ps[:])
                for t in range(4):
                    nc.sync.dma_start(out=fs_spill[b][t][:, :], in_=fs_tiles[t][:])

            # =========== phase 3: AllReduce of logits ===========
            cc_in = dramp.tile([64, NU * 64], F32)
            cc_out = dramp.tile([64, NU * 64], F32)
            nc.gpsimd.dma_start(out=cc_in[:], in_=Sp_sb[:])
            nc.gpsimd.collective_compute(
                'AllReduce', mybir.AluOpType.add,
                replica_groups=[list(range(NCORES))],
                ins=[cc_in.opt()], outs=[cc_out.opt()],
            )
            S_sb = constp.tile([64, NU * 64], F32)
            nc.gpsimd.dma_start(out=S_sb[:], in_=cc_out[:])

            # =========== phase 4: softmax + A_t^T ===========
            attn_sb = constp.tile([64, NU * 64], BF)
            AT_tiles = []
            for u in range(NU):
                usl = slice(u * 64, (u + 1) * 64)
                mx = smallp.tile([64, 1], F32)
                nmx = smallp.tile([64, 1], F32)
                ex = smallp.tile([64, 64], F32)
                sm = smallp.tile([64, 1], F32)
                rs = smallp.tile([64, 1], F32)
                nc.vector.tensor_reduce(out=mx[:], in_=S_sb[:, usl],
                                        axis=mybir.AxisListType.X, op=ALU.max)
                nc.vector.tensor_scalar_mul(out=nmx[:], in0=mx[:], scalar1=-1.0)
                nc.scalar.activation(out=ex[:], in_=S_sb[:, usl], func=AF.Exp,
                                     bias=nmx[:, 0:1])
                nc.vector.tensor_reduce(out=sm[:], in_=ex[:],
                                        axis=mybir.AxisListType.X, op=ALU.add)
                nc.vector.reciprocal(out=rs[:], in_=sm[:])
                nc.vector.tensor_scalar_mul(out=attn_sb[:, usl], in0=ex[:],
                                            scalar1=rs[:, 0:1])
            for b in range(B):
                for t in range(4):
                    u = b * 4 + t
                    a_full = psp.tile([64, 512], F32, name='misc_ps')
                    a_ps = a_full[:, 0:256]
                    nc.tensor.matmul(out=a_ps,
                                     lhsT=attn_sb[:, u * 64:(u + 1) * 64],
                                     rhs=w00o_sb[:, t * 256:(t + 1) * 256],
                                     start=True, stop=True)
                    at = constp.tile([65, 256], BF, name=f'at{b}_{t}')
                    nc.vector.tensor_copy(out=at[0:64, :], in_=a_ps)
                    if t == 0:
                        nc.sync.dma_start(
                            out=at[64:65, :],
                            in_=wfull[WOFF_B00 + b * 256:WOFF_B00 + (b + 1) * 256])
                    AT_tiles.append(at)

            # =========== phase 5: regather + MLP ===========
            for b in range(B):
                fs_tiles = [fsp.tile([65, NLOC], BF, name=f'fs{_t}') for _t in range(4)]
                for t in range(4):
                    nc.sync.dma_start(out=fs_tiles[t][:], in_=fs_spill[b][t][:, :])
                bil_sb = smallp.tile([1, NLOC], BF, name='bil_sb')
                nc.sync.dma_start(out=bil_sb[:], in_=bil[b, :][None, :])

                for pc in range(NLOC // PCH):
                    psl = slice(pc * PCH, (pc + 1) * PCH)
                    # transient v tiles for this pixel super-chunk
                    v_tiles = []
                    for t in range(4):
                        vt = vp.tile([65, PCH], BF, name=f'vt{t}')
                        nc.vector.memset(vt[64:65, :], 1.0)
                        for cc in range(PCH // CHUNK):
                            vsl_l = slice(cc * CHUNK, (cc + 1) * CHUNK)
                            vsl_g = slice(pc * PCH + cc * CHUNK, pc * PCH + (cc + 1) * CHUNK)
                            v_ps = psp.tile([64, CHUNK], F32)
                            nc.tensor.matmul(out=v_ps[:], lhsT=wv_sb[:],
                                             rhs=fs_tiles[t][0:64, vsl_g],
                                             start=True, stop=True)
                            nc.scalar.activation(out=vt[0:64, vsl_l], in_=v_ps[:],
                                                 func=AF.Relu, bias=bv_sb[:, 0:1])
                        v_tiles.append(vt)

                    x1_t = [mlpp.tile([128, PCH], BF, name=f'x1_{_m}') for _m in range(2)]
                    x2_t = [mlpp.tile([128, PCH], BF, name=f'x2_{_m}') for _m in range(2)]
                    for cc in range(PCH // CHUNK):
                        lsl = slice(cc * CHUNK, (cc + 1) * CHUNK)
                        gsl = slice(pc * PCH + cc * CHUNK, pc * PCH + (cc + 1) * CHUNK)
                        for m in range(2):
                            msl = slice(m * 128, (m + 1) * 128)
                            x_ps = psxp.tile([128, CHUNK], F32)
                            for t in range(4):
                                nc.tensor.matmul(
                                    out=x_ps[:],
                                    lhsT=w00f_sb[:, t * 256 + m * 128: t * 256 + (m + 1) * 128],
                                    rhs=fs_tiles[t][:, gsl],
                                    start=(t == 0), stop=False)
                            for t in range(4):
                                at = AT_tiles[b * 4 + t]
                                kk = 65 if t == 0 else 64
                                nc.tensor.matmul(
                                    out=x_ps[:],
                                    lhsT=at[0:kk, msl],
                                    rhs=v_tiles[t][0:kk, lsl],
                                    start=False, stop=(t == 3))
                            nc.vector.tensor_copy(out=x1_t[m][:, lsl], in_=x_ps[:])
                        # W1 + gelu
                        for m in range(2):
                            msl = slice(m * 128, (m + 1) * 128)
                            x2_ps = psxp.tile([128, CHUNK], F32)
                            for kk in range(2):
                                nc.tensor.matmul(out=x2_ps[:],
                                                 lhsT=w1_sb[:, kk, msl],
                                                 rhs=x1_t[kk][:, lsl],
                                                 start=(kk == 0), stop=(kk == 1))
                            nc.scalar.activation(out=x2_t[m][:, lsl], in_=x2_ps[:],
                                                 func=AF.Gelu, bias=b1_sb[:, m:m + 1])
                        # W2 + bil add
                        o_full = psp.tile([64, 512], F32, name='misc_ps')
                        o_ps = o_full[0:1, :]
                        for kk in range(2):
                            nc.tensor.matmul(out=o_ps, lhsT=w2_sb[:, kk:kk + 1],
                                             rhs=x2_t[kk][:, lsl],
                                             start=(kk == 0), stop=(kk == 1))
                        o_sb = smallp.tile([1, CHUNK], BF)
                        nc.vector.tensor_tensor(out=o_sb[:], in0=o_ps,
                                                in1=bil_sb[:, gsl], op=ALU.add)
                        nc.sync.dma_start(out=out[b, gsl][None, :], in_=o_sb[:])

    nc.compile()
    return nc


# --------------------------------------------------------------------------

def kernel(**inputs) -> np.ndarray:
    from concourse.bass_utils import run_bass_kernel_spmd
    in_maps = _host_prep(inputs)
    nc = _build()
    res = run_bass_kernel_spmd(nc, in_maps, core_ids=list(range(NCORES)))
    full = np.empty((B, 1, HQ, WQ), np.float32)
    flat = full.reshape(B, NPB)
    for cidx in range(NCORES):
        flat[:, cidx * NLOC:(cidx + 1) * NLOC] = \
            res.results[cidx]['out'].astype(np.float32)
    return full


# revision 62
# speedup vs baseline: 1451.5345x; 1.1117x over previous
"""Trainium2 Bass kernel for nn_AnyTSRpp (sparse_attention).

Strategy: pure data-parallel over the HR pixel grid (65536 px/batch),
8192 px/batch/core on 8 NeuronCores. Host computes per-pixel corner
indices/scalars; device gathers feat rows directly (per-corner indirect
DMA, pixel-major), applies the RBF weight per-partition pre-transpose,
PE transposes to channel-major, runs all matmuls/relu/softmax/gelu, and
a tiny AllReduce for the global attention logits (contraction over all
pixels). off_t = attn_t @ v_t is folded as (W00_off_t @ attn_t) @ v_t
so the attention output is never materialized.

Self-contained: hardcodes all shapes. kernel(**inputs) -> np.ndarray.
"""

import functools
import numpy as np
import ml_dtypes

BF16 = ml_dtypes.bfloat16


def _setup_jax_cache():
    """Persistent XLA compilation cache: repeated/every-process calls skip
    the neuronx-cc recompile of the identical kernel graph."""
    import jax
    try:
        jax.config.update('jax_compilation_cache_dir', '/root/.cache/jax_pcache')
        jax.config.update('jax_persistent_cache_min_compile_time_secs', 0.0)
        jax.config.update('jax_persistent_cache_min_entry_size_bytes', 0)
    except Exception:
        pass


_setup_jax_cache()

NCORES = 8
B = 2
C = 64
HLR = WLR = 64
HQ = WQ = 256
NPB = HQ * WQ            # 65536 pixels per batch
NLOC = NPB // NCORES     # 8192 pixels per batch per core
NROW = HLR * WLR         # 4096 feat rows (y-major)
CHUNK = 512              # matmul moving-N chunk
NCHUNK = NLOC // CHUNK   # 16
PCH = 1024               # MLP pixel super-chunk
EPS = np.float32(1e-6)

# bf16 weight blob layout (flat element offsets)
WOFF_WQ = 0                        # [2, 64]  Wq.T/QK
WOFF_BQ = WOFF_WQ + 2 * 64         # [1, 64]
WOFF_WK = WOFF_BQ + 64             # [64, 64] Wk.T
WOFF_BK = WOFF_WK + 64 * 64        # [1, 64]
WOFF_WV = WOFF_BK + 64             # [64, 64]
WOFF_W00O = WOFF_WV + 64 * 64      # [4, 64, 256]
WOFF_W00F = WOFF_W00O + 4 * 64 * 256   # [2, 128, 256] stacked corner pairs
WOFF_W1 = WOFF_W00F + 2 * 128 * 256    # [2, 128, 256]
WOFF_W2 = WOFF_W1 + 2 * 128 * 256  # [2, 128, 1]
WBLOB = WOFF_W2 + 2 * 128 + 512    # pad to 205824 = 8 * 128 * 201
WSH = WBLOB // NCORES


# --------------------------------------------------------------------------
# host-side math (mirrors reference semantics in f32)
# --------------------------------------------------------------------------

def _corner_indices(co):
    """co: [N] f32 coords in one axis. Returns (base j in [0,65], i_minus,
    i_plus) exactly matching the reference's per-corner nearest indices."""
    # reference: c_t = clip(co + v/64 + eps, -1+1e-6, 1-1e-6);
    #            i_t = clip(round((c_t+1)*32 - 0.5), 0, 63)
    out = []
    for v in (-1.0, 1.0):
        c = np.clip(co + np.float32(v / 64.0) + EPS,
                    np.float32(-1 + 1e-6), np.float32(1 - 1e-6))
        i = np.clip(np.round((c + 1) * np.float32(32.0) - np.float32(0.5)),
                    0, 63).astype(np.int32)
        out.append(i)
    im, ip = out
    # padded-table base: j = clip(floor(ay), -1, 64) + 1, ay = 32*(co+eps)+31.5
    ay = (co + EPS) * np.float32(32.0) + np.float32(31.5)
    j = np.clip(np.floor(ay), -1, 64).astype(np.int32) + 1
    return j, im, ip


def _host_prep(inputs):
    feat = np.asarray(inputs['feat'], np.float32)
    inp = np.asarray(inputs['inp'], np.float32)
    coord = np.asarray(inputs['coord'], np.float32)
    cell = np.asarray(inputs['cell'], np.float32)
    scale = np.asarray(inputs['scale'], np.float32)
    Wq = np.asarray(inputs['Wq'], np.float32); bq = np.asarray(inputs['bq'], np.float32)
    Wk = np.asarray(inputs['Wk'], np.float32); bk = np.asarray(inputs['bk'], np.float32)
    Wv = np.asarray(inputs['Wv'], np.float32); bv = np.asarray(inputs['bv'], np.float32)
    W00 = np.asarray(inputs['W00'], np.float32); b00 = np.asarray(inputs['b00'], np.float32)
    W1 = np.asarray(inputs['W1'], np.float32); b1 = np.asarray(inputs['b1'], np.float32)
    W2 = np.asarray(inputs['W2'], np.float32); b2 = np.asarray(inputs['b2'], np.float32)
    ls = np.asarray(inputs['ls'], np.float32)

    # feat as bf16 rows [B, 4096, 64]: row iy*64+ix = feat[b, :, iy, ix]
    featrows = np.ascontiguousarray(
        feat.transpose(0, 2, 3, 1).reshape(B, NROW, C)).astype(BF16)

    coord_y = coord[..., 0].reshape(B, NPB)
    coord_x = coord[..., 1].reshape(B, NPB)

    # per-(b) padded-table base index; per-corner rel offsets + RBF weights
    idx_all = np.empty((B, NPB), np.int16)
    rel_all = np.empty((B, 4, 2, NPB), BF16)   # [rel_y, rel_x]
    w_all = np.empty((B, 4, NPB), BF16)
    hw = np.float32(64.0)
    ls2 = ls[0] * ls[0]
    for b in range(B):
        jy, iym, iyp = _corner_indices(coord_y[b])
        jx, ixm, ixp = _corner_indices(coord_x[b])
        idx_all[b] = (jy * np.int32(66) + jx).astype(np.int16)
        iy = {-1: iym, 1: iyp}
        ix = {-1: ixm, 1: ixp}
        t = 0
        for vx in (-1, 1):          # y offset
            for vy in (-1, 1):      # x offset
                oy = (iy[vx].astype(np.float32) + np.float32(0.5)) / np.float32(32.0) - 1
                ox = (ix[vy].astype(np.float32) + np.float32(0.5)) / np.float32(32.0) - 1
                ry = coord_y[b] - oy
                rx = coord_x[b] - ox
                rel_all[b, t, 0] = ry.astype(BF16)
                rel_all[b, t, 1] = rx.astype(BF16)
                rd = (ry * hw) ** 2 + (rx * hw) ** 2
                w_all[b, t] = np.exp(rd / ls2 * np.float32(-0.5)).astype(BF16)
                t += 1

    # ---- bilinear sample of inp (border, align_corners=False) + b2 ----
    bil = np.empty((B, NPB), BF16)
    for b in range(B):
        im = inp[b, 0]
        y = np.clip((coord_y[b] + 1) * np.float32(32.0) - np.float32(0.5), 0.0, 63.0)
        x = np.clip((coord_x[b] + 1) * np.float32(32.0) - np.float32(0.5), 0.0, 63.0)
        y0 = np.floor(y); x0 = np.floor(x)
        wy = (y - y0).astype(np.float32); wx = (x - x0).astype(np.float32)
        y0i = np.clip(y0.astype(np.int32), 0, 63)
        y1i = np.clip(y0.astype(np.int32) + 1, 0, 63)
        x0i = np.clip(x0.astype(np.int32), 0, 63)
        x1i = np.clip(x0.astype(np.int32) + 1, 0, 63)
        v00 = im[y0i, x0i]; v01 = im[y0i, x1i]
        v10 = im[y1i, x0i]; v11 = im[y1i, x1i]
        bil[b] = ((v00 * (1 - wy) * (1 - wx) + v01 * (1 - wy) * wx
                   + v10 * wy * (1 - wx) + v11 * wy * wx) + b2[0]).astype(BF16)

    # ---- rel -> int8 with the dequant scale folded into Wq's rel rows ----
    relmax = float(np.max(np.abs(rel_all.astype(np.float32)))) or 1.0
    QK = np.float32(127.0 / relmax)
    rel8 = np.clip(np.round(rel_all.astype(np.float32) * QK),
                   -127, 127).astype(np.int8)                               # [B,4,2,NPB]

    # ---- weight repacks ----
    wq_rhs = (Wq.T / QK).astype(BF16)                                       # [2, 64]
    wv_lhsT = Wv.T.astype(BF16)                                             # [64, 64]
    w00off_rhs = np.stack([W00[:, t * 64:(t + 1) * 64].T for t in range(4)]
                          ).astype(BF16)                                    # [4, 64, 256]
    # stacked corner-pair lhsT for the x1 fs-term: rows 0:64 = corner 2p,
    # rows 64:128 = corner 2p+1
    w00fp = np.stack(
        [np.concatenate([W00[:, 256 + 2 * p * 64: 256 + (2 * p + 1) * 64].T,
                         W00[:, 256 + (2 * p + 1) * 64: 256 + (2 * p + 2) * 64].T],
                        axis=0) for p in range(2)]).astype(BF16)            # [2, 128, 256]
    # fold the scalar grid tail (b00 + W00[:,512:516] @ [cell*hw, scale])
    # through W1 into the gelu bias: b1eff = b1 + W1 @ b00eff
    b1eff = np.empty((B, 2, 128, 1), np.float32)
    for b in range(B):
        vec4 = np.concatenate([cell[b] * hw, scale[b]]).astype(np.float32)
        b00eff = b00 + W00[:, 512:516] @ vec4
        b1eff[b] = (b1 + W1 @ b00eff).reshape(2, 128, 1)
    w1_lhsT = np.ascontiguousarray(W1.T.astype(BF16).reshape(2, 128, 256))  # [2, 128, 256]
    w2_lhsT = np.ascontiguousarray(W2.T.astype(BF16).reshape(2, 128, 1))    # [2, 128, 1]

    # ---- bf16 weight blob (AllGathered on device): flat row-major concat ----
    wflat = np.concatenate([
        wq_rhs.reshape(-1), bq.astype(BF16), Wk.T.astype(BF16).reshape(-1),
        bk.astype(BF16), wv_lhsT.reshape(-1),
        w00off_rhs.reshape(-1), w00fp.reshape(-1),
        w1_lhsT.reshape(-1), w2_lhsT.reshape(-1),
        np.zeros(512, BF16)])
    assert wflat.size == WBLOB, wflat.size

    # ---- shard per core ----
    NFS = NROW // NCORES     # 512 feat rows per core shard (AllGathered on device)
    in_maps = []
    for cidx in range(NCORES):
        sl = slice(cidx * NLOC, (cidx + 1) * NLOC)
        # pixel-major tiles: local pixel j*128+p at [p, j]; each gathered
        # table row holds all 4 corners (c00|c01|c10|c11), so wsm is laid
        # out corner-minor [p, j*4+t] to broadcast-multiply the row.
        idx2d = np.ascontiguousarray(
            idx_all[:, sl].reshape(B, 64, 128).transpose(0, 2, 1))
        wsm2d = np.ascontiguousarray(
            w_all[:, :, sl].reshape(B, 4, 64, 128).transpose(0, 3, 2, 1)
            .reshape(B, 128, 4 * 64))
        m = {
            'feati': np.ascontiguousarray(
                featrows[:, cidx * NFS:(cidx + 1) * NFS, :]).reshape(B, 128, 256),
            'wblob': np.ascontiguousarray(
                wflat[cidx * WSH:(cidx + 1) * WSH]).reshape(128, WSH // 128),
            'idx': idx2d,
            'wsm': wsm2d,
            'relq': np.ascontiguousarray(rel8[:, :, :, sl]),
            'bil': np.ascontiguousarray(bil[:, sl]),
            'bv': np.concatenate([bv, bv]).reshape(128, 1).astype(np.float32),
            'b1': b1eff,
        }
        in_maps.append(m)
    return in_maps


# --------------------------------------------------------------------------
# device kernel
# --------------------------------------------------------------------------

@functools.lru_cache(maxsize=1)
def _build():
    import concourse.bass as bass
    import concourse.tile as tile
    from concourse import bacc, mybir
    dt = mybir.dt
    F32, BF, I16 = dt.float32, dt.bfloat16, dt.int16
    AF = mybir.ActivationFunctionType
    ALU = mybir.AluOpType

    nc = bacc.Bacc(None, target_bir_lowering=False)

    feati = nc.dram_tensor('feati', [B, 128, 256], BF, kind='ExternalInput')
    wblob = nc.dram_tensor('wblob', [128, WSH // 128], BF, kind='ExternalInput')
    idx = nc.dram_tensor('idx', [B, 128, 64], I16, kind='ExternalInput')
    wsm = nc.dram_tensor('wsm', [B, 128, 4 * 64], BF, kind='ExternalInput')
    relq = nc.dram_tensor('relq', [B, 4, 2, NLOC], dt.int8, kind='ExternalInput')
    bil = nc.dram_tensor('bil', [B, NLOC], BF, kind='ExternalInput')
    bv = nc.dram_tensor('bv', [128, 1], F32, kind='ExternalInput')
    b1 = nc.dram_tensor('b1', [B, 2, 128, 1], F32, kind='ExternalInput')
    out = nc.dram_tensor('out', [B, NLOC], BF, kind='ExternalOutput')

    NU = B * 4  # 8 attention units

    with tile.TileContext(nc) as tc:
        with (
            tc.tile_pool(name='const', bufs=1) as constp,
            tc.tile_pool(name='fs', bufs=1) as fsp,
            tc.tile_pool(name='gat', bufs=1) as gatp,
            tc.tile_pool(name='qk', bufs=1) as qkp,
            tc.tile_pool(name='rel', bufs=1) as relp,
            tc.tile_pool(name='v', bufs=1) as vp,
            tc.tile_pool(name='mlp', bufs=1) as mlpp,
            tc.tile_pool(name='small', bufs=1) as smallp,
            tc.tile_pool(name='ps', bufs=1, space='PSUM') as psp,
            tc.tile_pool(name='psx', bufs=1, space='PSUM') as psxp,
            tc.tile_pool(name='dram', bufs=1, space='DRAM') as dramp,
        ):
            # ---- AllGather feat row shards and the weight blob ----
            featfull = [dramp.tile([NROW, C], BF, name=f'featfull{_b}')
                        for _b in range(B)]
            for _b in range(B):
                fstage = gatp.tile([128, 256], BF, name='fstage')
                ccf_in = dramp.tile([128, 256], BF, name=f'ccf_in{_b}')
                nc.sync.dma_start(out=fstage[:], in_=feati[_b, :, :])
                nc.gpsimd.dma_start(out=ccf_in[:], in_=fstage[:])
                nc.gpsimd.collective_compute(
                    'AllGather', mybir.AluOpType.bypass,
                    replica_groups=[list(range(NCORES))],
                    ins=[ccf_in.opt()], outs=[featfull[_b].opt()],
                )
            wfull = dramp.tile([WBLOB], BF, name='wfull')
            wstage = gatp.tile([128, WSH // 128], BF, name='wstage')
            wcc_in = dramp.tile([128, WSH // 128], BF, name='wcc_in')
            nc.sync.dma_start(out=wstage[:], in_=wblob[:, :])
            nc.gpsimd.dma_start(out=wcc_in[:], in_=wstage[:])
            nc.gpsimd.collective_compute(
                'AllGather', mybir.AluOpType.bypass,
                replica_groups=[list(range(NCORES))],
                ins=[wcc_in.opt()], outs=[wfull.opt()],
            )

            # ---- 66x66 edge-replicated 2x2-patch table, built on device ----
            # ptable[b][jy*66+jx] = [c00|c01|c10|c11],
            # c(dy,dx) = feat[b, :, clip(jy-1+dy,0,63), clip(jx-1+dx,0,63)]
            NTAB = 66 * 66
            ptable = [dramp.tile([NTAB, 256], BF, name=f'ptable{_b}')
                      for _b in range(B)]
            for _b in range(B):
                pt_t = ptable[_b][:, :].tensor
                ff_t = featfull[_b][:, :].tensor
                for dy in (0, 1):
                    yr = ([(0, 1, 0), (1, 64, 0), (65, 1, 63)] if dy == 0
                          else [(0, 64, 0), (64, 2, 63)])
                    for dx in (0, 1):
                        xr = ([(0, 1, 0), (1, 64, 0), (65, 1, 63)] if dx == 0
                              else [(0, 64, 0), (64, 2, 63)])
                        qoff = (dy * 2 + dx) * 64
                        for (jy0, ny, sy0) in yr:
                            for (jx0, nx, sx0) in xr:
                                dst = bass.AP(
                                    pt_t, (jy0 * 66 + jx0) * 256 + qoff,
                                    [(66 * 256, ny), (256, nx), (1, 64)])
                                src = bass.AP(
                                    ff_t, (sy0 * 64 + sx0) * 64,
                                    [(4096 if ny > 1 and sy0 == 0 else 0, ny),
                                     (64 if nx > 1 and sx0 == 0 else 0, nx),
                                     (1, 64)])
                                nc.sync.dma_start(out=dst, in_=src)

            # ---- constant weights to SBUF (from the gathered blob) ----
            wq_sb = constp.tile([2, 64], BF)
            bq_sb = constp.tile([1, 64], BF)
            wk_sb = constp.tile([128, 64], BF)   # Wk.T duplicated in both halves
            bk_sb = constp.tile([1, 64], BF)
            wv_sb = constp.tile([128, 64], BF)   # Wv.T duplicated in both halves
            bv_sb = constp.tile([128, 1], F32)
            w00o_sb = constp.tile([64, 4 * 256], BF)
            w00fp_sb = constp.tile([128, 2, 256], BF)
            w1_sb = constp.tile([128, 2, 256], BF)
            b1_sb = constp.tile([128, B, 2], F32)
            w2_sb = constp.tile([128, 2], BF)
            nc.sync.dma_start(out=wq_sb[:], in_=wfull[WOFF_WQ:WOFF_BQ])
            nc.sync.dma_start(out=bq_sb[:], in_=wfull[WOFF_BQ:WOFF_WK])
            nc.sync.dma_start(out=wk_sb[0:64, :], in_=wfull[WOFF_WK:WOFF_BK])
            nc.sync.dma_start(out=wk_sb[64:128, :], in_=wfull[WOFF_WK:WOFF_BK])
            nc.sync.dma_start(out=bk_sb[:], in_=wfull[WOFF_BK:WOFF_WV])
            nc.sync.dma_start(out=wv_sb[0:64, :], in_=wfull[WOFF_WV:WOFF_W00O])
            nc.sync.dma_start(out=wv_sb[64:128, :], in_=wfull[WOFF_WV:WOFF_W00O])
            nc.sync.dma_start(out=bv_sb[:], in_=bv[:, :])
            for t in range(4):
                nc.sync.dma_start(
                    out=w00o_sb[:, t * 256:(t + 1) * 256],
                    in_=wfull[WOFF_W00O + t * 16384:WOFF_W00O + (t + 1) * 16384])
            for kk in range(2):
                nc.sync.dma_start(
                    out=w00fp_sb[:, kk, :],
                    in_=wfull[WOFF_W00F + kk * 32768:WOFF_W00F + (kk + 1) * 32768])
                nc.sync.dma_start(
                    out=w1_sb[:, kk, :],
                    in_=wfull[WOFF_W1 + kk * 32768:WOFF_W1 + (kk + 1) * 32768])
                for _b in range(B):
                    nc.sync.dma_start(out=b1_sb[:, _b, kk:kk + 1],
                                      in_=b1[_b, kk, :, :])
                nc.sync.dma_start(
                    out=w2_sb[:, kk:kk + 1],
                    in_=wfull[WOFF_W2 + kk * 128:WOFF_W2 + (kk + 1) * 128])

            Sp_sb = constp.tile([64, NU * 64], F32)   # partial logits, all units

            # =========== phases 1+2 per batch: gather, fs, q/k, S ===========
            from concourse.masks import make_identity
            ident_sb = constp.tile([128, 128], BF)
            make_identity(nc, ident_sb[:])


            ones_nl = constp.tile([1, NLOC], BF)
            nc.vector.memset(ones_nl[:], 1.0)

            def gather_fs(b, fsp_tiles):
                idx16_sb = gatp.tile([128, 64], I16)
                wsm_sb = gatp.tile([128, 4 * 64], BF)
                idx_sb = gatp.tile([128, 64], dt.int32)
                nc.sync.dma_start(out=idx16_sb[:], in_=idx[b, :, :])
                nc.sync.dma_start(out=wsm_sb[:], in_=wsm[b, :, :])
                nc.vector.tensor_copy(out=idx_sb[:], in_=idx16_sb[:])
                # quarters of 16 pixel-tiles; each gathered 512B table row
                # carries all 4 corners; one broadcast multiply per quarter.
                # [128,128] transposes put corner 2p on partitions 0:64 and
                # corner 2p+1 on 64:128 -> stacked pair tiles, no ones rows.
                for q in range(4):
                    g_pm = gatp.tile([128, 16, 4 * C], BF, name=f'g_pm{q % 2}')
                    for o in range(16):
                        nc.gpsimd.indirect_dma_start(
                            out=g_pm[:, o, :], out_offset=None,
                            in_=ptable[b][:, :],
                            in_offset=bass.IndirectOffsetOnAxis(
                                ap=idx_sb[:, q * 16 + o:q * 16 + o + 1], axis=0))
                    wap = wsm_sb[:, q * 64:(q + 1) * 64]
                    wbc = bass.AP(wap.tensor, wap.offset, wap.ap + [(0, C)])
                    nc.vector.tensor_tensor(out=g_pm[:, :, :],
                                            in0=g_pm[:, :, :], in1=wbc,
                                            op=ALU.mult)
                    for p in range(2):
                        for jg in range(4):
                            tp_ps = psp.tile([128, 512], BF)
                            for jj in range(4):
                                jl = jg * 4 + jj
                                nc.tensor.transpose(
                                    out=tp_ps[:, jj * 128:(jj + 1) * 128],
                                    in_=g_pm[:, jl, p * 128:(p + 1) * 128],
                                    identity=ident_sb[:])
                            goff = (q * 16 + jg * 4) * 128
                            nc.scalar.copy(
                                out=fsp_tiles[p][:, goff:goff + 512],
                                in_=tp_ps[:])

            fsp_all = [[fsp.tile([128, NLOC], BF, name=f'fsp{_b}_{_p}')
                        for _p in range(2)] for _b in range(B)]
            for b in range(B):
                gather_fs(b, fsp_all[b])

                for t in range(4):
                    rel8_sb = relp.tile([2, NLOC], dt.int8, name='rel8')
                    nc.sync.dma_start(out=rel8_sb[:], in_=relq[b, t, :, :])
                    rel_sb = relp.tile([2, NLOC], BF)
                    nc.vector.tensor_copy(out=rel_sb[:], in_=rel8_sb[:])
                    fpt = fsp_all[b][t // 2]
                    tb = (t % 2) * 64
                    qT_sb = qkp.tile([128, 64 * 64], BF)
                    kT_sb = qkp.tile([128, 64 * 64], BF)
                    s_ps = psp.tile([64, 64], F32, name='s_ps')
                    for jg in range(8):          # groups of 8 pixel-tiles
                        q_ps = psp.tile([128, 512], F32)
                        k_ps = psp.tile([128, 512], F32)
                        for jj in range(8):
                            j = jg * 8 + jj
                            csl = slice(j * 128, (j + 1) * 128)
                            osl = slice(jj * 64, (jj + 1) * 64)
                            nc.tensor.matmul(
                                out=q_ps[:, osl], lhsT=rel_sb[:, csl],
                                rhs=wq_sb[:], start=True, stop=False)
                            nc.tensor.matmul(
                                out=q_ps[:, osl], lhsT=ones_nl[:, csl],
                                rhs=bq_sb[:], start=False, stop=True)
                            nc.tensor.matmul(
                                out=k_ps[:, osl], lhsT=fpt[tb:tb + 64, csl],
                                rhs=wk_sb[tb:tb + 64, :], start=True, stop=False)
                            nc.tensor.matmul(
                                out=k_ps[:, osl], lhsT=ones_nl[:, csl],
                                rhs=bk_sb[:], start=False, stop=True)
                        gsl = slice(jg * 512, (jg + 1) * 512)
                        nc.scalar.activation(out=qT_sb[:, gsl], in_=q_ps[:], func=AF.Relu)
                        nc.vector.tensor_scalar_max(out=kT_sb[:, gsl], in0=k_ps[:], scalar1=0.0)
                    for j in range(64):
                        nc.tensor.matmul(
                            out=s_ps[:],
                            lhsT=qT_sb[:, j * 64:(j + 1) * 64],
                            rhs=kT_sb[:, j * 64:(j + 1) * 64],
                            start=(j == 0), stop=(j == 63))
                    u = b * 4 + t
                    nc.vector.tensor_copy(out=Sp_sb[:, u * 64:(u + 1) * 64], in_=s_ps[:])

            # =========== phase 3: AllReduce of logits ===========
            cc_in = dramp.tile([64, NU * 64], F32)
            cc_out = dramp.tile([64, NU * 64], F32)
            nc.gpsimd.dma_start(out=cc_in[:], in_=Sp_sb[:])
            nc.gpsimd.collective_compute(
                'AllReduce', mybir.AluOpType.add,
                replica_groups=[list(range(NCORES))],
                ins=[cc_in.opt()], outs=[cc_out.opt()],
            )
            S_sb = constp.tile([64, NU * 64], F32)
            nc.gpsimd.dma_start(out=S_sb[:], in_=cc_out[:])

            # =========== phase 4: softmax + A_t^T ===========
            attn_sb = constp.tile([64, NU * 64], BF)
            AT_tiles = []
            for u in range(NU):
                usl = slice(u * 64, (u + 1) * 64)
                mx = smallp.tile([64, 1], F32)
                nmx = smallp.tile([64, 1], F32)
                ex = smallp.tile([64, 64], F32)
                sm = smallp.tile([64, 1], F32)
                rs = smallp.tile([64, 1], F32)
                nc.vector.tensor_reduce(out=mx[:], in_=S_sb[:, usl],
                                        axis=mybir.AxisListType.X, op=ALU.max)
                nc.vector.tensor_scalar_mul(out=nmx[:], in0=mx[:], scalar1=-1.0)
                nc.scalar.activation(out=ex[:], in_=S_sb[:, usl], func=AF.Exp,
                                     bias=nmx[:, 0:1])
                nc.vector.tensor_reduce(out=sm[:], in_=ex[:],
                                        axis=mybir.AxisListType.X, op=ALU.add)
                nc.vector.reciprocal(out=rs[:], in_=sm[:])
                nc.vector.tensor_scalar_mul(out=attn_sb[:, usl], in0=ex[:],
                                            scalar1=rs[:, 0:1])
            for b in range(B):
                for p in range(2):
                    a_full = psp.tile([128, 512], F32, name='misc_ps')
                    for h in range(2):
                        t = 2 * p + h
                        u = b * 4 + t
                        nc.tensor.matmul(
                            out=a_full[h * 64:(h + 1) * 64, 0:256],
                            lhsT=attn_sb[:, u * 64:(u + 1) * 64],
                            rhs=w00o_sb[:, t * 256:(t + 1) * 256],
                            start=True, stop=True)
                    at = constp.tile([128, 256], BF, name=f'atp{b}_{p}')
                    nc.vector.tensor_copy(out=at[:], in_=a_full[:, 0:256])
                    AT_tiles.append(at)

            # =========== phase 5: MLP over resident fs pairs ===========
            for b in range(B):
                bil_sb = smallp.tile([1, NLOC], BF, name='bil_sb')
                nc.sync.dma_start(out=bil_sb[:], in_=bil[b, :][None, :])

                for pc in range(NLOC // PCH):
                    # transient stacked v pair tiles for this pixel super-chunk
                    v_tiles = []
                    for p in range(2):
                        vt = vp.tile([128, PCH], BF, name=f'vt{p}')
                        for cc in range(PCH // CHUNK):
                            vsl_l = slice(cc * CHUNK, (cc + 1) * CHUNK)
                            vsl_g = slice(pc * PCH + cc * CHUNK, pc * PCH + (cc + 1) * CHUNK)
                            v_ps = psp.tile([128, CHUNK], F32)
                            nc.tensor.matmul(out=v_ps[0:64, :], lhsT=wv_sb[0:64, :],
                                             rhs=fsp_all[b][p][0:64, vsl_g],
                                             start=True, stop=True)
                            nc.tensor.matmul(out=v_ps[64:128, :], lhsT=wv_sb[64:128, :],
                                             rhs=fsp_all[b][p][64:128, vsl_g],
                                             start=True, stop=True)
                            nc.scalar.activation(out=vt[:, vsl_l], in_=v_ps[:],
                                                 func=AF.Relu, bias=bv_sb[:, 0:1])
                        v_tiles.append(vt)

                    x1_t = [mlpp.tile([128, PCH], BF, name=f'x1_{_m}') for _m in range(2)]
                    x2_t = [mlpp.tile([128, PCH], BF, name=f'x2_{_m}') for _m in range(2)]
                    for cc in range(PCH // CHUNK):
                        lsl = slice(cc * CHUNK, (cc + 1) * CHUNK)
                        gsl = slice(pc * PCH + cc * CHUNK, pc * PCH + (cc + 1) * CHUNK)
                        for m in range(2):
                            msl = slice(m * 128, (m + 1) * 128)
                            x_ps = psxp.tile([128, CHUNK], F32)
                            for p in range(2):
                                nc.tensor.matmul(
                                    out=x_ps[:],
                                    lhsT=w00fp_sb[:, p, msl],
                                    rhs=fsp_all[b][p][:, gsl],
                                    start=(p == 0), stop=False)
                            for p in range(2):
                                at = AT_tiles[b * 2 + p]
                                nc.tensor.matmul(
                                    out=x_ps[:],
                                    lhsT=at[:, msl],
                                    rhs=v_tiles[p][:, lsl],
                                    start=False, stop=(p == 1))
                            nc.vector.tensor_copy(out=x1_t[m][:, lsl], in_=x_ps[:])
                        # W1 + gelu
                        for m in range(2):
                            msl = slice(m * 128, (m + 1) * 128)
                            x2_ps = psxp.tile([128, CHUNK], F32)
                            for kk in range(2):
                                nc.tensor.matmul(out=x2_ps[:],
                                                 lhsT=w1_sb[:, kk, msl],
                                                 rhs=x1_t[kk][:, lsl],
                                                 start=(kk == 0), stop=(kk == 1))
                            nc.scalar.activation(out=x2_t[m][:, lsl], in_=x2_ps[:],
                                                 func=AF.Gelu, bias=b1_sb[:, b, m:m + 1])
                        # W2 + bil add
                        o_full = psp.tile([64, 512], F32, name='misc_ps')
                        o_ps = o_full[0:1, :]
                        for kk in range(2):
                            nc.tensor.matmul(out=o_ps, lhsT=w2_sb[:, kk:kk + 1],
                                             rhs=x2_t[kk][:, lsl],
                                             start=(kk == 0), stop=(kk == 1))
                        o_sb = smallp.tile([1, CHUNK], BF)
                        nc.vector.tensor_tensor(out=o_sb[:], in0=o_ps,
                                                in1=bil_sb[:, gsl], op=ALU.add)
                        nc.sync.dma_start(out=out[b, gsl][None, :], in_=o_sb[:])

    nc.compile()
    return nc


# --------------------------------------------------------------------------

def kernel(**inputs) -> np.ndarray:
    from concourse.bass_utils import run_bass_kernel_spmd
    in_maps = _host_prep(inputs)
    nc = _build()
    res = run_bass_kernel_spmd(nc, in_maps, core_ids=list(range(NCORES)))
    full = np.empty((B, 1, HQ, WQ), np.float32)
    flat = full.reshape(B, NPB)
    for cidx in range(NCORES):
        flat[:, cidx * NLOC:(cidx + 1) * NLOC] = \
            res.results[cidx]['out'].astype(np.float32)
    return full


# revision 75
# speedup vs baseline: 1998.0501x; 1.3765x over previous
"""Trainium2 Bass kernel for nn_AnyTSRpp (sparse_attention).

Strategy: pure data-parallel over the HR pixel grid (65536 px/batch),
8192 px/batch/core on 8 NeuronCores. Host computes per-pixel corner
indices/scalars; device gathers feat rows directly (per-corner indirect
DMA, pixel-major), applies the RBF weight per-partition pre-transpose,
PE transposes to channel-major, runs all matmuls/relu/softmax/gelu, and
a tiny AllReduce for the global attention logits (contraction over all
pixels). off_t = attn_t @ v_t is folded as (W00_off_t @ attn_t) @ v_t
so the attention output is never materialized.

Self-contained: hardcodes all shapes. kernel(**inputs) -> np.ndarray.
"""

import functools
import numpy as np
import ml_dtypes

BF16 = ml_dtypes.bfloat16


def _setup_jax_cache():
    """Persistent XLA compilation cache: repeated/every-process calls skip
    the neuronx-cc recompile of the identical kernel graph."""
    import jax
    try:
        jax.config.update('jax_compilation_cache_dir', '/root/.cache/jax_pcache')
        jax.config.update('jax_persistent_cache_min_compile_time_secs', 0.0)
        jax.config.update('jax_persistent_cache_min_entry_size_bytes', 0)
    except Exception:
        pass


_setup_jax_cache()

NCORES = 8
B = 2
C = 64
HLR = WLR = 64
HQ = WQ = 256
NPB = HQ * WQ            # 65536 pixels per batch
NLOC = NPB // NCORES     # 8192 pixels per batch per core
NROW = HLR * WLR         # 4096 feat rows (y-major)
CHUNK = 512              # matmul moving-N chunk
NCHUNK = NLOC // CHUNK   # 16
PCH = 1024               # MLP pixel super-chunk
EPS = np.float32(1e-6)

# bf16 weight blob layout (flat element offsets)
WOFF_WQ = 0                        # [2, 64]  Wq.T/QK
WOFF_BQ = WOFF_WQ + 2 * 64         # [1, 64]
WOFF_WK = WOFF_BQ + 64             # [64, 64] Wk.T
WOFF_BK = WOFF_WK + 64 * 64        # [1, 64]
WOFF_WV = WOFF_BK + 64             # [64, 64]
WOFF_W00O = WOFF_WV + 64 * 64      # [4, 64, 256]
WOFF_W00F = WOFF_W00O + 4 * 64 * 256   # [2, 128, 256] stacked corner pairs
WOFF_W1 = WOFF_W00F + 2 * 128 * 256    # [2, 128, 256]
WOFF_W2 = WOFF_W1 + 2 * 128 * 256  # [2, 128, 1]
WBLOB = WOFF_W2 + 2 * 128 + 512    # pad to 205824 = 8 * 128 * 201
WSH = WBLOB // NCORES


# --------------------------------------------------------------------------
# host-side math (mirrors reference semantics in f32)
# --------------------------------------------------------------------------

def _corner_indices(co):
    """co: [N] f32 coords in one axis. Returns (base j in [0,65], i_minus,
    i_plus) exactly matching the reference's per-corner nearest indices."""
    # reference: c_t = clip(co + v/64 + eps, -1+1e-6, 1-1e-6);
    #            i_t = clip(round((c_t+1)*32 - 0.5), 0, 63)
    out = []
    for v in (-1.0, 1.0):
        c = np.clip(co + np.float32(v / 64.0) + EPS,
                    np.float32(-1 + 1e-6), np.float32(1 - 1e-6))
        i = np.clip(np.round((c + 1) * np.float32(32.0) - np.float32(0.5)),
                    0, 63).astype(np.int32)
        out.append(i)
    im, ip = out
    # padded-table base: j = clip(floor(ay), -1, 64) + 1, ay = 32*(co+eps)+31.5
    ay = (co + EPS) * np.float32(32.0) + np.float32(31.5)
    j = np.clip(np.floor(ay), -1, 64).astype(np.int32) + 1
    return j, im, ip


def _host_prep(inputs):
    feat = np.asarray(inputs['feat'], np.float32)
    inp = np.asarray(inputs['inp'], np.float32)
    coord = np.asarray(inputs['coord'], np.float32)
    cell = np.asarray(inputs['cell'], np.float32)
    scale = np.asarray(inputs['scale'], np.float32)
    Wq = np.asarray(inputs['Wq'], np.float32); bq = np.asarray(inputs['bq'], np.float32)
    Wk = np.asarray(inputs['Wk'], np.float32); bk = np.asarray(inputs['bk'], np.float32)
    Wv = np.asarray(inputs['Wv'], np.float32); bv = np.asarray(inputs['bv'], np.float32)
    W00 = np.asarray(inputs['W00'], np.float32); b00 = np.asarray(inputs['b00'], np.float32)
    W1 = np.asarray(inputs['W1'], np.float32); b1 = np.asarray(inputs['b1'], np.float32)
    W2 = np.asarray(inputs['W2'], np.float32); b2 = np.asarray(inputs['b2'], np.float32)
    ls = np.asarray(inputs['ls'], np.float32)

    # feat as bf16 rows [B, 4096, 64]: row iy*64+ix = feat[b, :, iy, ix]
    featrows = np.ascontiguousarray(
        feat.transpose(0, 2, 3, 1).reshape(B, NROW, C)).astype(BF16)

    coord_y = coord[..., 0].reshape(B, NPB)
    coord_x = coord[..., 1].reshape(B, NPB)

    # per-(b) padded-table base index; per-corner rel offsets + RBF weights
    idx_all = np.empty((B, NPB), np.int16)
    rel_all = np.empty((B, 4, 2, NPB), BF16)   # [rel_y, rel_x]
    w_all = np.empty((B, 4, NPB), BF16)
    hw = np.float32(64.0)
    ls2 = ls[0] * ls[0]
    for b in range(B):
        jy, iym, iyp = _corner_indices(coord_y[b])
        jx, ixm, ixp = _corner_indices(coord_x[b])
        idx_all[b] = (jy * np.int32(66) + jx).astype(np.int16)
        iy = {-1: iym, 1: iyp}
        ix = {-1: ixm, 1: ixp}
        t = 0
        for vx in (-1, 1):          # y offset
            for vy in (-1, 1):      # x offset
                oy = (iy[vx].astype(np.float32) + np.float32(0.5)) / np.float32(32.0) - 1
                ox = (ix[vy].astype(np.float32) + np.float32(0.5)) / np.float32(32.0) - 1
                ry = coord_y[b] - oy
                rx = coord_x[b] - ox
                rel_all[b, t, 0] = ry.astype(BF16)
                rel_all[b, t, 1] = rx.astype(BF16)
                rd = (ry * hw) ** 2 + (rx * hw) ** 2
                w_all[b, t] = np.exp(rd / ls2 * np.float32(-0.5)).astype(BF16)
                t += 1

    # ---- bilinear sample of inp (border, align_corners=False) + b2 ----
    bil = np.empty((B, NPB), BF16)
    for b in range(B):
        im = inp[b, 0]
        y = np.clip((coord_y[b] + 1) * np.float32(32.0) - np.float32(0.5), 0.0, 63.0)
        x = np.clip((coord_x[b] + 1) * np.float32(32.0) - np.float32(0.5), 0.0, 63.0)
        y0 = np.floor(y); x0 = np.floor(x)
        wy = (y - y0).astype(np.float32); wx = (x - x0).astype(np.float32)
        y0i = np.clip(y0.astype(np.int32), 0, 63)
        y1i = np.clip(y0.astype(np.int32) + 1, 0, 63)
        x0i = np.clip(x0.astype(np.int32), 0, 63)
        x1i = np.clip(x0.astype(np.int32) + 1, 0, 63)
        v00 = im[y0i, x0i]; v01 = im[y0i, x1i]
        v10 = im[y1i, x0i]; v11 = im[y1i, x1i]
        bil[b] = ((v00 * (1 - wy) * (1 - wx) + v01 * (1 - wy) * wx
                   + v10 * wy * (1 - wx) + v11 * wy * wx) + b2[0]).astype(BF16)

    # ---- rel -> int8 with the dequant scale folded into Wq's rel rows ----
    relmax = float(np.max(np.abs(rel_all.astype(np.float32)))) or 1.0
    QK = np.float32(127.0 / relmax)
    rel8 = np.clip(np.round(rel_all.astype(np.float32) * QK),
                   -127, 127).astype(np.int8)                               # [B,4,2,NPB]

    # ---- weight repacks ----
    wq_rhs = (Wq.T / QK).astype(BF16)                                       # [2, 64]
    wv_lhsT = Wv.T.astype(BF16)                                             # [64, 64]
    w00off_rhs = np.stack([W00[:, t * 64:(t + 1) * 64].T for t in range(4)]
                          ).astype(BF16)                                    # [4, 64, 256]
    # stacked corner-pair lhsT for the x1 fs-term: rows 0:64 = corner 2p,
    # rows 64:128 = corner 2p+1
    w00fp = np.stack(
        [np.concatenate([W00[:, 256 + 2 * p * 64: 256 + (2 * p + 1) * 64].T,
                         W00[:, 256 + (2 * p + 1) * 64: 256 + (2 * p + 2) * 64].T],
                        axis=0) for p in range(2)]).astype(BF16)            # [2, 128, 256]
    # fold the scalar grid tail (b00 + W00[:,512:516] @ [cell*hw, scale])
    # through W1 into the gelu bias: b1eff = b1 + W1 @ b00eff
    b1eff = np.empty((B, 2, 128, 1), np.float32)
    for b in range(B):
        vec4 = np.concatenate([cell[b] * hw, scale[b]]).astype(np.float32)
        b00eff = b00 + W00[:, 512:516] @ vec4
        b1eff[b] = (b1 + W1 @ b00eff).reshape(2, 128, 1)
    w1_lhsT = np.ascontiguousarray(W1.T.astype(BF16).reshape(2, 128, 256))  # [2, 128, 256]
    w2_lhsT = np.ascontiguousarray(W2.T.astype(BF16).reshape(2, 128, 1))    # [2, 128, 1]

    # ---- bf16 weight blob (AllGathered on device): flat row-major concat ----
    wflat = np.concatenate([
        wq_rhs.reshape(-1), bq.astype(BF16), Wk.T.astype(BF16).reshape(-1),
        bk.astype(BF16), wv_lhsT.reshape(-1),
        w00off_rhs.reshape(-1), w00fp.reshape(-1),
        w1_lhsT.reshape(-1), w2_lhsT.reshape(-1),
        np.zeros(512, BF16)])
    assert wflat.size == WBLOB, wflat.size

    # ---- shard per core ----
    NFS = NROW // NCORES     # 512 feat rows per core shard (AllGathered on device)
    in_maps = []
    for cidx in range(NCORES):
        sl = slice(cidx * NLOC, (cidx + 1) * NLOC)
        # pixel-major tiles: local pixel j*128+p at [p, j]; each gathered
        # table row holds all 4 corners (c00|c01|c10|c11), so wsm is laid
        # out corner-minor [p, j*4+t] to broadcast-multiply the row.
        idx2d = np.ascontiguousarray(
            idx_all[:, sl].reshape(B, 64, 128).transpose(0, 2, 1))
        wsm2d = np.ascontiguousarray(
            w_all[:, :, sl].reshape(B, 4, 64, 128).transpose(0, 3, 2, 1)
            .reshape(B, 128, 4 * 64))
        m = {
            'feati': np.ascontiguousarray(
                featrows[:, cidx * NFS:(cidx + 1) * NFS, :]).reshape(B, 128, 256),
            'wblob': np.ascontiguousarray(
                wflat[cidx * WSH:(cidx + 1) * WSH]).reshape(128, WSH // 128),
            'idx': idx2d,
            'wsm': wsm2d,
            'relq': np.ascontiguousarray(rel8[:, :, :, sl]).reshape(B, 8, NLOC),
            'bil': np.ascontiguousarray(bil[:, sl]),
            'bv': np.concatenate([bv, bv]).reshape(128, 1).astype(np.float32),
            'b1': b1eff,
        }
        in_maps.append(m)
    return in_maps


# --------------------------------------------------------------------------
# device kernel
# --------------------------------------------------------------------------

@functools.lru_cache(maxsize=4)
def _build(qk_bias=False):
    import concourse.bass as bass
    import concourse.tile as tile
    from concourse import bacc, mybir
    dt = mybir.dt
    F32, BF, I16 = dt.float32, dt.bfloat16, dt.int16
    AF = mybir.ActivationFunctionType
    ALU = mybir.AluOpType

    nc = bacc.Bacc(None, target_bir_lowering=False)

    feati = nc.dram_tensor('feati', [B, 128, 256], BF, kind='ExternalInput')
    wblob = nc.dram_tensor('wblob', [128, WSH // 128], BF, kind='ExternalInput')
    idx = nc.dram_tensor('idx', [B, 128, 64], I16, kind='ExternalInput')
    wsm = nc.dram_tensor('wsm', [B, 128, 4 * 64], BF, kind='ExternalInput')
    relq = nc.dram_tensor('relq', [B, 8, NLOC], dt.int8, kind='ExternalInput')
    bil = nc.dram_tensor('bil', [B, NLOC], BF, kind='ExternalInput')
    bv = nc.dram_tensor('bv', [128, 1], F32, kind='ExternalInput')
    b1 = nc.dram_tensor('b1', [B, 2, 128, 1], F32, kind='ExternalInput')
    out = nc.dram_tensor('out', [B, NLOC], BF, kind='ExternalOutput')

    NU = B * 4  # 8 attention units

    with tile.TileContext(nc) as tc:
        with (
            tc.tile_pool(name='const', bufs=1) as constp,
            tc.tile_pool(name='fs', bufs=1) as fsp,
            tc.tile_pool(name='gat', bufs=1) as gatp,
            tc.tile_pool(name='qk', bufs=1) as qkp,
            tc.tile_pool(name='rel', bufs=1) as relp,
            tc.tile_pool(name='v', bufs=1) as vp,
            tc.tile_pool(name='mlp', bufs=1) as mlpp,
            tc.tile_pool(name='small', bufs=1) as smallp,
            tc.tile_pool(name='ps', bufs=1, space='PSUM') as psp,
            tc.tile_pool(name='psx', bufs=1, space='PSUM') as psxp,
            tc.tile_pool(name='dram', bufs=1, space='DRAM') as dramp,
        ):
            # ---- AllGather feat row shards and the weight blob ----
            featfull = [dramp.tile([NROW, C], BF, name=f'featfull{_b}')
                        for _b in range(B)]
            for _b in range(B):
                fstage = gatp.tile([128, 256], BF, name='fstage')
                ccf_in = dramp.tile([128, 256], BF, name=f'ccf_in{_b}')
                nc.sync.dma_start(out=fstage[:], in_=feati[_b, :, :])
                nc.gpsimd.dma_start(out=ccf_in[:], in_=fstage[:])
                nc.gpsimd.collective_compute(
                    'AllGather', mybir.AluOpType.bypass,
                    replica_groups=[list(range(NCORES))],
                    ins=[ccf_in.opt()], outs=[featfull[_b].opt()],
                )
            wfull = dramp.tile([WBLOB], BF, name='wfull')
            wstage = gatp.tile([128, WSH // 128], BF, name='wstage')
            wcc_in = dramp.tile([128, WSH // 128], BF, name='wcc_in')
            nc.sync.dma_start(out=wstage[:], in_=wblob[:, :])
            nc.gpsimd.dma_start(out=wcc_in[:], in_=wstage[:])
            nc.gpsimd.collective_compute(
                'AllGather', mybir.AluOpType.bypass,
                replica_groups=[list(range(NCORES))],
                ins=[wcc_in.opt()], outs=[wfull.opt()],
            )

            # ---- 66x66 edge-replicated 2x2-patch table, built on device ----
            # ptable[b][jy*66+jx] = [c00|c01|c10|c11],
            # c(dy,dx) = feat[b, :, clip(jy-1+dy,0,63), clip(jx-1+dx,0,63)]
            NTAB = 66 * 66
            ptable = [dramp.tile([NTAB, 256], BF, name=f'ptable{_b}')
                      for _b in range(B)]
            for _b in range(B):
                pt_t = ptable[_b][:, :].tensor
                ff_t = featfull[_b][:, :].tensor
                for dy in (0, 1):
                    yr = ([(0, 1, 0), (1, 64, 0), (65, 1, 63)] if dy == 0
                          else [(0, 64, 0), (64, 2, 63)])
                    for dx in (0, 1):
                        xr = ([(0, 1, 0), (1, 64, 0), (65, 1, 63)] if dx == 0
                              else [(0, 64, 0), (64, 2, 63)])
                        qoff = (dy * 2 + dx) * 64
                        for (jy0, ny, sy0) in yr:
                            for (jx0, nx, sx0) in xr:
                                dst = bass.AP(
                                    pt_t, (jy0 * 66 + jx0) * 256 + qoff,
                                    [(66 * 256, ny), (256, nx), (1, 64)])
                                src = bass.AP(
                                    ff_t, (sy0 * 64 + sx0) * 64,
                                    [(4096 if ny > 1 and sy0 == 0 else 0, ny),
                                     (64 if nx > 1 and sx0 == 0 else 0, nx),
                                     (1, 64)])
                                nc.sync.dma_start(out=dst, in_=src)

            # ---- constant weights to SBUF (from the gathered blob) ----
            # Wq.T/QK in rows 2t:2t+2 of per-corner slot t, zero elsewhere,
            # so the q matmul can take the full [8, .] rel tile as lhsT
            wq_sb = constp.tile([8, 4, 64], BF)
            bq_sb = constp.tile([1, 64], BF)
            wk_sb = constp.tile([128, 64], BF)   # Wk.T duplicated in both halves
            bk_sb = constp.tile([1, 64], BF)
            wv_sb = constp.tile([128, 64], BF)   # Wv.T duplicated in both halves
            bv_sb = constp.tile([128, 1], F32)
            w00o_sb = constp.tile([64, 4 * 256], BF)
            w00fp_sb = constp.tile([128, 2, 256], BF)
            w1_sb = constp.tile([128, 2, 256], BF)
            b1_sb = constp.tile([128, B, 2], F32)
            w2_sb = constp.tile([128, 2], BF)
            nc.vector.memset(wq_sb[:], 0.0)
            for _r in range(4):
                nc.sync.dma_start(out=wq_sb[2 * _r:2 * _r + 2, _r, :],
                                  in_=wfull[WOFF_WQ:WOFF_BQ])
            nc.sync.dma_start(out=bq_sb[:], in_=wfull[WOFF_BQ:WOFF_WK])
            nc.sync.dma_start(out=wk_sb[0:64, :], in_=wfull[WOFF_WK:WOFF_BK])
            nc.sync.dma_start(out=wk_sb[64:128, :], in_=wfull[WOFF_WK:WOFF_BK])
            nc.sync.dma_start(out=bk_sb[:], in_=wfull[WOFF_BK:WOFF_WV])
            nc.sync.dma_start(out=wv_sb[0:64, :], in_=wfull[WOFF_WV:WOFF_W00O])
            nc.sync.dma_start(out=wv_sb[64:128, :], in_=wfull[WOFF_WV:WOFF_W00O])
            nc.sync.dma_start(out=bv_sb[:], in_=bv[:, :])
            for t in range(4):
                nc.sync.dma_start(
                    out=w00o_sb[:, t * 256:(t + 1) * 256],
                    in_=wfull[WOFF_W00O + t * 16384:WOFF_W00O + (t + 1) * 16384])
            for kk in range(2):
                nc.sync.dma_start(
                    out=w00fp_sb[:, kk, :],
                    in_=wfull[WOFF_W00F + kk * 32768:WOFF_W00F + (kk + 1) * 32768])
                nc.sync.dma_start(
                    out=w1_sb[:, kk, :],
                    in_=wfull[WOFF_W1 + kk * 32768:WOFF_W1 + (kk + 1) * 32768])
                for _b in range(B):
                    nc.sync.dma_start(out=b1_sb[:, _b, kk:kk + 1],
                                      in_=b1[_b, kk, :, :])
                nc.sync.dma_start(
                    out=w2_sb[:, kk:kk + 1],
                    in_=wfull[WOFF_W2 + kk * 128:WOFF_W2 + (kk + 1) * 128])

            Sp_sb = constp.tile([64, NU * 64], F32)   # partial logits, all units

            # =========== phases 1+2 per batch: gather, fs, q/k, S ===========
            from concourse.masks import make_identity
            ident_sb = constp.tile([128, 128], BF)
            make_identity(nc, ident_sb[:])


            ones_nl = constp.tile([1, NLOC], BF)
            nc.vector.memset(ones_nl[:], 1.0)

            def gather_fs(b, fsp_tiles):
                idx16_sb = gatp.tile([128, 64], I16)
                wsm_sb = gatp.tile([128, 4 * 64], BF)
                idx_sb = gatp.tile([128, 64], dt.int32)
                nc.sync.dma_start(out=idx16_sb[:], in_=idx[b, :, :])
                nc.sync.dma_start(out=wsm_sb[:], in_=wsm[b, :, :])
                nc.vector.tensor_copy(out=idx_sb[:], in_=idx16_sb[:])
                # quarters of 16 pixel-tiles; each gathered 512B table row
                # carries all 4 corners; one broadcast multiply per quarter.
                # [128,128] transposes put corner 2p on partitions 0:64 and
                # corner 2p+1 on 64:128 -> stacked pair tiles, no ones rows.
                for q in range(4):
                    g_pm = gatp.tile([128, 16, 4 * C], BF, name=f'g_pm{q % 2}')
                    for o in range(16):
                        nc.gpsimd.indirect_dma_start(
                            out=g_pm[:, o, :], out_offset=None,
                            in_=ptable[b][:, :],
                            in_offset=bass.IndirectOffsetOnAxis(
                                ap=idx_sb[:, q * 16 + o:q * 16 + o + 1], axis=0))
                    wap = wsm_sb[:, q * 64:(q + 1) * 64]
                    wbc = bass.AP(wap.tensor, wap.offset, wap.ap + [(0, C)])
                    nc.vector.tensor_tensor(out=g_pm[:, :, :],
                                            in0=g_pm[:, :, :], in1=wbc,
                                            op=ALU.mult)
                    for p in range(2):
                        for jg in range(4):
                            tp_ps = psp.tile([128, 512], BF)
                            for jj in range(4):
                                jl = jg * 4 + jj
                                nc.tensor.transpose(
                                    out=tp_ps[:, jj * 128:(jj + 1) * 128],
                                    in_=g_pm[:, jl, p * 128:(p + 1) * 128],
                                    identity=ident_sb[:])
                            goff = (q * 16 + jg * 4) * 128
                            nc.scalar.copy(
                                out=fsp_tiles[p][:, goff:goff + 512],
                                in_=tp_ps[:])

            fsp_all = [[fsp.tile([128, NLOC], BF, name=f'fsp{_b}_{_p}')
                        for _p in range(2)] for _b in range(B)]
            for b in range(B):
                gather_fs(b, fsp_all[b])

                rel8_sb = relp.tile([8, NLOC], dt.int8, name='rel8')
                nc.sync.dma_start(out=rel8_sb[:], in_=relq[b, :, :])
                rel_sb = relp.tile([8, NLOC], BF)
                nc.vector.tensor_copy(out=rel_sb[:], in_=rel8_sb[:])
                for t in range(4):
                    fpt = fsp_all[b][t // 2]
                    tb = (t % 2) * 64
                    qT_sb = qkp.tile([128, 64 * 64], BF)
                    kT_sb = qkp.tile([128, 64 * 64], BF)
                    s_ps = psp.tile([64, 64], F32, name='s_ps')
                    for jg in range(8):          # groups of 8 pixel-tiles
                        q_ps = psp.tile([128, 512], F32)
                        k_ps = psp.tile([128, 512], F32)
                        for jj in range(8):
                            j = jg * 8 + jj
                            csl = slice(j * 128, (j + 1) * 128)
                            osl = slice(jj * 64, (jj + 1) * 64)
                            nc.tensor.matmul(
                                out=q_ps[:, osl],
                                lhsT=rel_sb[:, csl],
                                rhs=wq_sb[:, t, :],
                                start=True, stop=not qk_bias)
                            nc.tensor.matmul(
                                out=k_ps[:, osl], lhsT=fpt[tb:tb + 64, csl],
                                rhs=wk_sb[tb:tb + 64, :],
                                start=True, stop=not qk_bias)
                            if qk_bias:
                                nc.tensor.matmul(
                                    out=q_ps[:, osl], lhsT=ones_nl[:, csl],
                                    rhs=bq_sb[:], start=False, stop=True)
                                nc.tensor.matmul(
                                    out=k_ps[:, osl], lhsT=ones_nl[:, csl],
                                    rhs=bk_sb[:], start=False, stop=True)
                        gsl = slice(jg * 512, (jg + 1) * 512)
                        nc.scalar.activation(out=qT_sb[:, gsl], in_=q_ps[:], func=AF.Relu)
                        nc.vector.tensor_scalar_max(out=kT_sb[:, gsl], in0=k_ps[:], scalar1=0.0)
                    for j in range(64):
                        nc.tensor.matmul(
                            out=s_ps[:],
                            lhsT=qT_sb[:, j * 64:(j + 1) * 64],
                            rhs=kT_sb[:, j * 64:(j + 1) * 64],
                            start=(j == 0), stop=(j == 63))
                    u = b * 4 + t
                    nc.vector.tensor_copy(out=Sp_sb[:, u * 64:(u + 1) * 64], in_=s_ps[:])

            # =========== phase 3: AllReduce of logits ===========
            cc_in = dramp.tile([64, NU * 64], F32)
            cc_out = dramp.tile([64, NU * 64], F32)
            nc.gpsimd.dma_start(out=cc_in[:], in_=Sp_sb[:])
            nc.gpsimd.collective_compute(
                'AllReduce', mybir.AluOpType.add,
                replica_groups=[list(range(NCORES))],
                ins=[cc_in.opt()], outs=[cc_out.opt()],
            )
            S_sb = constp.tile([64, NU * 64], F32)
            nc.gpsimd.dma_start(out=S_sb[:], in_=cc_out[:])

            # =========== phase 4: softmax + A_t^T ===========
            attn_sb = constp.tile([64, NU * 64], BF)
            AT_tiles = []
            for u in range(NU):
                usl = slice(u * 64, (u + 1) * 64)
                mx = smallp.tile([64, 1], F32)
                nmx = smallp.tile([64, 1], F32)
                ex = smallp.tile([64, 64], F32)
                sm = smallp.tile([64, 1], F32)
                rs = smallp.tile([64, 1], F32)
                nc.vector.tensor_reduce(out=mx[:], in_=S_sb[:, usl],
                                        axis=mybir.AxisListType.X, op=ALU.max)
                nc.vector.tensor_scalar_mul(out=nmx[:], in0=mx[:], scalar1=-1.0)
                nc.scalar.activation(out=ex[:], in_=S_sb[:, usl], func=AF.Exp,
                                     bias=nmx[:, 0:1])
                nc.vector.tensor_reduce(out=sm[:], in_=ex[:],
                                        axis=mybir.AxisListType.X, op=ALU.add)
                nc.vector.reciprocal(out=rs[:], in_=sm[:])
                nc.vector.tensor_scalar_mul(out=attn_sb[:, usl], in0=ex[:],
                                            scalar1=rs[:, 0:1])
            for b in range(B):
                for p in range(2):
                    a_full = psp.tile([128, 512], F32, name='misc_ps')
                    for h in range(2):
                        t = 2 * p + h
                        u = b * 4 + t
                        nc.tensor.matmul(
                            out=a_full[h * 64:(h + 1) * 64, 0:256],
                            lhsT=attn_sb[:, u * 64:(u + 1) * 64],
                            rhs=w00o_sb[:, t * 256:(t + 1) * 256],
                            start=True, stop=True)
                    at = constp.tile([128, 256], BF, name=f'atp{b}_{p}')
                    nc.vector.tensor_copy(out=at[:], in_=a_full[:, 0:256])
                    AT_tiles.append(at)

            # =========== phase 5: MLP over resident fs pairs ===========
            for b in range(B):
                bil_sb = smallp.tile([1, NLOC], BF, name='bil_sb')
                nc.sync.dma_start(out=bil_sb[:], in_=bil[b, :][None, :])
                o_row = smallp.tile([1, NLOC], BF, name='o_row')

                for pc in range(NLOC // PCH):
                    # transient stacked v pair tiles for this pixel super-chunk
                    v_tiles = []
                    for p in range(2):
                        vt = vp.tile([128, PCH], BF, name=f'vt{p}')
                        for cc in range(PCH // CHUNK):
                            vsl_l = slice(cc * CHUNK, (cc + 1) * CHUNK)
                            vsl_g = slice(pc * PCH + cc * CHUNK, pc * PCH + (cc + 1) * CHUNK)
                            v_ps = psp.tile([128, CHUNK], F32)
                            nc.tensor.matmul(out=v_ps[0:64, :], lhsT=wv_sb[0:64, :],
                                             rhs=fsp_all[b][p][0:64, vsl_g],
                                             start=True, stop=True)
                            nc.tensor.matmul(out=v_ps[64:128, :], lhsT=wv_sb[64:128, :],
                                             rhs=fsp_all[b][p][64:128, vsl_g],
                                             start=True, stop=True)
                            nc.scalar.activation(out=vt[:, vsl_l], in_=v_ps[:],
                                                 func=AF.Relu, bias=bv_sb[:, 0:1])
                        v_tiles.append(vt)

                    x1_t = [mlpp.tile([128, PCH], BF, name=f'x1_{_m}') for _m in range(2)]
                    x2_t = [mlpp.tile([128, PCH], BF, name=f'x2_{_m}') for _m in range(2)]
                    for cc in range(PCH // CHUNK):
                        lsl = slice(cc * CHUNK, (cc + 1) * CHUNK)
                        gsl = slice(pc * PCH + cc * CHUNK, pc * PCH + (cc + 1) * CHUNK)
                        for m in range(2):
                            msl = slice(m * 128, (m + 1) * 128)
                            x_ps = psxp.tile([128, CHUNK], F32)
                            for p in range(2):
                                nc.tensor.matmul(
                                    out=x_ps[:],
                                    lhsT=w00fp_sb[:, p, msl],
                                    rhs=fsp_all[b][p][:, gsl],
                                    start=(p == 0), stop=False)
                            for p in range(2):
                                at = AT_tiles[b * 2 + p]
                                nc.tensor.matmul(
                                    out=x_ps[:],
                                    lhsT=at[:, msl],
                                    rhs=v_tiles[p][:, lsl],
                                    start=False, stop=(p == 1))
                            nc.vector.tensor_copy(out=x1_t[m][:, lsl], in_=x_ps[:])
                        # W1 + gelu
                        for m in range(2):
                            msl = slice(m * 128, (m + 1) * 128)
                            x2_ps = psxp.tile([128, CHUNK], F32)
                            for kk in range(2):
                                nc.tensor.matmul(out=x2_ps[:],
                                                 lhsT=w1_sb[:, kk, msl],
                                                 rhs=x1_t[kk][:, lsl],
                                                 start=(kk == 0), stop=(kk == 1))
                            nc.scalar.activation(out=x2_t[m][:, lsl], in_=x2_ps[:],
                                                 func=AF.Gelu, bias=b1_sb[:, b, m:m + 1])
                        # W2; bil add batched once per batch below
                        o_full = psp.tile([64, 512], F32, name='misc_ps')
                        o_ps = o_full[0:1, :]
                        for kk in range(2):
                            nc.tensor.matmul(out=o_ps, lhsT=w2_sb[:, kk:kk + 1],
                                             rhs=x2_t[kk][:, lsl],
                                             start=(kk == 0), stop=(kk == 1))
                        nc.scalar.copy(out=o_row[:, gsl], in_=o_ps)
                nc.vector.tensor_tensor(out=o_row[:], in0=o_row[:],
                                        in1=bil_sb[:], op=ALU.add)
                nc.sync.dma_start(out=out[b, :][None, :], in_=o_row[:])

    nc.compile()
    return nc


# --------------------------------------------------------------------------

def kernel(**inputs) -> np.ndarray:
    from concourse.bass_utils import run_bass_kernel_spmd
    in_maps = _host_prep(inputs)
    qk_bias = bool(np.any(np.asarray(inputs['bq']))
                   or np.any(np.asarray(inputs['bk'])))
    nc = _build(qk_bias)
    res = run_bass_kernel_spmd(nc, in_maps, core_ids=list(range(NCORES)))
    full = np.empty((B, 1, HQ, WQ), np.float32)
    flat = full.reshape(B, NPB)
    for cidx in range(NCORES):
        flat[:, cidx * NLOC:(cidx + 1) * NLOC] = \
            res.results[cidx]['out'].astype(np.float32)
    return full


# revision 81
# speedup vs baseline: 2078.2360x; 1.0401x over previous
"""Trainium2 Bass kernel for nn_AnyTSRpp (sparse_attention).

Strategy: pure data-parallel over the HR pixel grid (65536 px/batch),
8192 px/batch/core on 8 NeuronCores. Host computes per-pixel corner
indices/scalars; device gathers feat rows directly (per-corner indirect
DMA, pixel-major), applies the RBF weight per-partition pre-transpose,
PE transposes to channel-major, runs all matmuls/relu/softmax/gelu, and
a tiny AllReduce for the global attention logits (contraction over all
pixels). off_t = attn_t @ v_t is folded as (W00_off_t @ attn_t) @ v_t
so the attention output is never materialized.

Self-contained: hardcodes all shapes. kernel(**inputs) -> np.ndarray.
"""

import functools
import numpy as np
import ml_dtypes

BF16 = ml_dtypes.bfloat16


def _setup_jax_cache():
    """Persistent XLA compilation cache: repeated/every-process calls skip
    the neuronx-cc recompile of the identical kernel graph."""
    import jax
    try:
        jax.config.update('jax_compilation_cache_dir', '/root/.cache/jax_pcache')
        jax.config.update('jax_persistent_cache_min_compile_time_secs', 0.0)
        jax.config.update('jax_persistent_cache_min_entry_size_bytes', 0)
    except Exception:
        pass


_setup_jax_cache()

NCORES = 8
B = 2
C = 64
HLR = WLR = 64
HQ = WQ = 256
NPB = HQ * WQ            # 65536 pixels per batch
NLOC = NPB // NCORES     # 8192 pixels per batch per core
NROW = HLR * WLR         # 4096 feat rows (y-major)
CHUNK = 512              # matmul moving-N chunk
NCHUNK = NLOC // CHUNK   # 16
PCH = 512                # MLP pixel super-chunk
EPS = np.float32(1e-6)

# bf16 weight blob layout (flat element offsets)
WOFF_WQ = 0                        # [2, 64]  Wq.T/QK
WOFF_BQ = WOFF_WQ + 2 * 64         # [1, 64]
WOFF_WK = WOFF_BQ + 64             # [64, 64] Wk.T
WOFF_BK = WOFF_WK + 64 * 64        # [1, 64]
WOFF_WV = WOFF_BK + 64             # [64, 64]
WOFF_W00O = WOFF_WV + 64 * 64      # [4, 64, 256]
WOFF_W00F = WOFF_W00O + 4 * 64 * 256   # [2, 128, 256] stacked corner pairs
WOFF_W1 = WOFF_W00F + 2 * 128 * 256    # [2, 128, 256]
WOFF_W2 = WOFF_W1 + 2 * 128 * 256  # [2, 128, 1]
WBLOB = WOFF_W2 + 2 * 128 + 512    # pad to 205824 = 8 * 128 * 201
WSH = WBLOB // NCORES


# --------------------------------------------------------------------------
# host-side math (mirrors reference semantics in f32)
# --------------------------------------------------------------------------

def _corner_indices(co):
    """co: [N] f32 coords in one axis. Returns (base j in [0,65], i_minus,
    i_plus) exactly matching the reference's per-corner nearest indices."""
    # reference: c_t = clip(co + v/64 + eps, -1+1e-6, 1-1e-6);
    #            i_t = clip(round((c_t+1)*32 - 0.5), 0, 63)
    out = []
    for v in (-1.0, 1.0):
        c = np.clip(co + np.float32(v / 64.0) + EPS,
                    np.float32(-1 + 1e-6), np.float32(1 - 1e-6))
        i = np.clip(np.round((c + 1) * np.float32(32.0) - np.float32(0.5)),
                    0, 63).astype(np.int32)
        out.append(i)
    im, ip = out
    # padded-table base: j = clip(floor(ay), -1, 64) + 1, ay = 32*(co+eps)+31.5
    ay = (co + EPS) * np.float32(32.0) + np.float32(31.5)
    j = np.clip(np.floor(ay), -1, 64).astype(np.int32) + 1
    return j, im, ip


def _host_prep(inputs):
    feat = np.asarray(inputs['feat'], np.float32)
    inp = np.asarray(inputs['inp'], np.float32)
    coord = np.asarray(inputs['coord'], np.float32)
    cell = np.asarray(inputs['cell'], np.float32)
    scale = np.asarray(inputs['scale'], np.float32)
    Wq = np.asarray(inputs['Wq'], np.float32); bq = np.asarray(inputs['bq'], np.float32)
    Wk = np.asarray(inputs['Wk'], np.float32); bk = np.asarray(inputs['bk'], np.float32)
    Wv = np.asarray(inputs['Wv'], np.float32); bv = np.asarray(inputs['bv'], np.float32)
    W00 = np.asarray(inputs['W00'], np.float32); b00 = np.asarray(inputs['b00'], np.float32)
    W1 = np.asarray(inputs['W1'], np.float32); b1 = np.asarray(inputs['b1'], np.float32)
    W2 = np.asarray(inputs['W2'], np.float32); b2 = np.asarray(inputs['b2'], np.float32)
    ls = np.asarray(inputs['ls'], np.float32)

    # feat as bf16 rows [B, 4096, 64]: row iy*64+ix = feat[b, :, iy, ix]
    featrows = np.ascontiguousarray(
        feat.transpose(0, 2, 3, 1).reshape(B, NROW, C)).astype(BF16)

    coord_y = coord[..., 0].reshape(B, NPB)
    coord_x = coord[..., 1].reshape(B, NPB)

    # per-(b) padded-table base index; per-corner rel offsets + RBF weights
    idx_all = np.empty((B, NPB), np.int16)
    rel_all = np.empty((B, 4, 2, NPB), BF16)   # [rel_y, rel_x]
    w_all = np.empty((B, 4, NPB), BF16)
    hw = np.float32(64.0)
    ls2 = ls[0] * ls[0]
    for b in range(B):
        jy, iym, iyp = _corner_indices(coord_y[b])
        jx, ixm, ixp = _corner_indices(coord_x[b])
        idx_all[b] = (jy * np.int32(66) + jx).astype(np.int16)
        iy = {-1: iym, 1: iyp}
        ix = {-1: ixm, 1: ixp}
        t = 0
        for vx in (-1, 1):          # y offset
            for vy in (-1, 1):      # x offset
                oy = (iy[vx].astype(np.float32) + np.float32(0.5)) / np.float32(32.0) - 1
                ox = (ix[vy].astype(np.float32) + np.float32(0.5)) / np.float32(32.0) - 1
                ry = coord_y[b] - oy
                rx = coord_x[b] - ox
                rel_all[b, t, 0] = ry.astype(BF16)
                rel_all[b, t, 1] = rx.astype(BF16)
                rd = (ry * hw) ** 2 + (rx * hw) ** 2
                w_all[b, t] = np.exp(rd / ls2 * np.float32(-0.5)).astype(BF16)
                t += 1

    # ---- bilinear sample of inp (border, align_corners=False) + b2 ----
    bil = np.empty((B, NPB), BF16)
    for b in range(B):
        im = inp[b, 0]
        y = np.clip((coord_y[b] + 1) * np.float32(32.0) - np.float32(0.5), 0.0, 63.0)
        x = np.clip((coord_x[b] + 1) * np.float32(32.0) - np.float32(0.5), 0.0, 63.0)
        y0 = np.floor(y); x0 = np.floor(x)
        wy = (y - y0).astype(np.float32); wx = (x - x0).astype(np.float32)
        y0i = np.clip(y0.astype(np.int32), 0, 63)
        y1i = np.clip(y0.astype(np.int32) + 1, 0, 63)
        x0i = np.clip(x0.astype(np.int32), 0, 63)
        x1i = np.clip(x0.astype(np.int32) + 1, 0, 63)
        v00 = im[y0i, x0i]; v01 = im[y0i, x1i]
        v10 = im[y1i, x0i]; v11 = im[y1i, x1i]
        bil[b] = ((v00 * (1 - wy) * (1 - wx) + v01 * (1 - wy) * wx
                   + v10 * wy * (1 - wx) + v11 * wy * wx) + b2[0]).astype(BF16)

    # ---- rel -> int8 with the dequant scale folded into Wq's rel rows ----
    relmax = float(np.max(np.abs(rel_all.astype(np.float32)))) or 1.0
    QK = np.float32(127.0 / relmax)
    rel8 = np.clip(np.round(rel_all.astype(np.float32) * QK),
                   -127, 127).astype(np.int8)                               # [B,4,2,NPB]

    # ---- weight repacks ----
    wq_rhs = (Wq.T / QK).astype(BF16)                                       # [2, 64]
    wv_lhsT = Wv.T.astype(BF16)                                             # [64, 64]
    w00off_rhs = np.stack([W00[:, t * 64:(t + 1) * 64].T for t in range(4)]
                          ).astype(BF16)                                    # [4, 64, 256]
    # stacked corner-pair lhsT for the x1 fs-term: rows 0:64 = corner 2p,
    # rows 64:128 = corner 2p+1
    w00fp = np.stack(
        [np.concatenate([W00[:, 256 + 2 * p * 64: 256 + (2 * p + 1) * 64].T,
                         W00[:, 256 + (2 * p + 1) * 64: 256 + (2 * p + 2) * 64].T],
                        axis=0) for p in range(2)]).astype(BF16)            # [2, 128, 256]
    # fold the scalar grid tail (b00 + W00[:,512:516] @ [cell*hw, scale])
    # through W1 into the gelu bias: b1eff = b1 + W1 @ b00eff
    b1eff = np.empty((B, 2, 128, 1), np.float32)
    for b in range(B):
        vec4 = np.concatenate([cell[b] * hw, scale[b]]).astype(np.float32)
        b00eff = b00 + W00[:, 512:516] @ vec4
        b1eff[b] = (b1 + W1 @ b00eff).reshape(2, 128, 1)
    w1_lhsT = np.ascontiguousarray(W1.T.astype(BF16).reshape(2, 128, 256))  # [2, 128, 256]
    w2_lhsT = np.ascontiguousarray(W2.T.astype(BF16).reshape(2, 128, 1))    # [2, 128, 1]

    # ---- bf16 weight blob (AllGathered on device): flat row-major concat ----
    wflat = np.concatenate([
        wq_rhs.reshape(-1), bq.astype(BF16), Wk.T.astype(BF16).reshape(-1),
        bk.astype(BF16), wv_lhsT.reshape(-1),
        w00off_rhs.reshape(-1), w00fp.reshape(-1),
        w1_lhsT.reshape(-1), w2_lhsT.reshape(-1),
        np.zeros(512, BF16)])
    assert wflat.size == WBLOB, wflat.size

    # ---- shard per core ----
    NFS = NROW // NCORES     # 512 feat rows per core shard (AllGathered on device)
    in_maps = []
    for cidx in range(NCORES):
        sl = slice(cidx * NLOC, (cidx + 1) * NLOC)
        # pixel-major tiles: local pixel j*128+p at [p, j]; each gathered
        # table row holds all 4 corners (c00|c01|c10|c11), so wsm is laid
        # out corner-minor [p, j*4+t] to broadcast-multiply the row.
        idx2d = np.ascontiguousarray(
            idx_all[:, sl].reshape(B, 64, 128).transpose(0, 2, 1))
        wsm2d = np.ascontiguousarray(
            w_all[:, :, sl].reshape(B, 4, 64, 128).transpose(0, 3, 2, 1)
            .reshape(B, 128, 4 * 64))
        m = {
            'feati': np.ascontiguousarray(
                featrows[:, cidx * NFS:(cidx + 1) * NFS, :]).reshape(B, 128, 256),
            'wblob': np.ascontiguousarray(
                wflat[cidx * WSH:(cidx + 1) * WSH]).reshape(128, WSH // 128),
            'idx': idx2d,
            'wsm': wsm2d,
            'relq': np.ascontiguousarray(rel8[:, :, :, sl]).reshape(B, 8, NLOC),
            'bil': np.ascontiguousarray(bil[:, sl]),
            'bv': np.concatenate([bv, bv]).reshape(128, 1).astype(np.float32),
            'b1': b1eff,
        }
        in_maps.append(m)
    return in_maps


# --------------------------------------------------------------------------
# device kernel
# --------------------------------------------------------------------------

@functools.lru_cache(maxsize=4)
def _build(qk_bias=False):
    import concourse.bass as bass
    import concourse.tile as tile
    from concourse import bacc, mybir
    dt = mybir.dt
    F32, BF, I16 = dt.float32, dt.bfloat16, dt.int16
    AF = mybir.ActivationFunctionType
    ALU = mybir.AluOpType

    nc = bacc.Bacc(None, target_bir_lowering=False)

    feati = nc.dram_tensor('feati', [B, 128, 256], BF, kind='ExternalInput')
    wblob = nc.dram_tensor('wblob', [128, WSH // 128], BF, kind='ExternalInput')
    idx = nc.dram_tensor('idx', [B, 128, 64], I16, kind='ExternalInput')
    wsm = nc.dram_tensor('wsm', [B, 128, 4 * 64], BF, kind='ExternalInput')
    relq = nc.dram_tensor('relq', [B, 8, NLOC], dt.int8, kind='ExternalInput')
    bil = nc.dram_tensor('bil', [B, NLOC], BF, kind='ExternalInput')
    bv = nc.dram_tensor('bv', [128, 1], F32, kind='ExternalInput')
    b1 = nc.dram_tensor('b1', [B, 2, 128, 1], F32, kind='ExternalInput')
    out = nc.dram_tensor('out', [B, NLOC], BF, kind='ExternalOutput')

    NU = B * 4  # 8 attention units

    with tile.TileContext(nc) as tc:
        with (
            tc.tile_pool(name='const', bufs=1) as constp,
            tc.tile_pool(name='fs', bufs=1) as fsp,
            tc.tile_pool(name='gat', bufs=1) as gatp,
            tc.tile_pool(name='qk', bufs=1) as qkp,
            tc.tile_pool(name='rel', bufs=1) as relp,
            tc.tile_pool(name='v', bufs=1) as vp,
            tc.tile_pool(name='mlp', bufs=1) as mlpp,
            tc.tile_pool(name='small', bufs=1) as smallp,
            tc.tile_pool(name='ps', bufs=1, space='PSUM') as psp,
            tc.tile_pool(name='psx', bufs=1, space='PSUM') as psxp,
            tc.tile_pool(name='dram', bufs=1, space='DRAM') as dramp,
        ):
            # ---- AllGather feat row shards and the weight blob ----
            featfull = [dramp.tile([NROW, C], BF, name=f'featfull{_b}')
                        for _b in range(B)]
            for _b in range(B):
                ccf_in = dramp.tile([128, 256], BF, name=f'ccf_in{_b}')
                nc.sync.dma_start(out=ccf_in[:], in_=feati[_b, :, :])
                nc.gpsimd.collective_compute(
                    'AllGather', mybir.AluOpType.bypass,
                    replica_groups=[list(range(NCORES))],
                    ins=[ccf_in.opt()], outs=[featfull[_b].opt()],
                )
            wfull = dramp.tile([WBLOB], BF, name='wfull')
            wcc_in = dramp.tile([128, WSH // 128], BF, name='wcc_in')
            nc.sync.dma_start(out=wcc_in[:], in_=wblob[:, :])
            nc.gpsimd.collective_compute(
                'AllGather', mybir.AluOpType.bypass,
                replica_groups=[list(range(NCORES))],
                ins=[wcc_in.opt()], outs=[wfull.opt()],
            )

            # ---- 66x66 edge-replicated 2x2-patch table, built on device ----
            # ptable[b][jy*66+jx] = [c00|c01|c10|c11],
            # c(dy,dx) = feat[b, :, clip(jy-1+dy,0,63), clip(jx-1+dx,0,63)]
            NTAB = 66 * 66
            ptable = [dramp.tile([NTAB, 256], BF, name=f'ptable{_b}')
                      for _b in range(B)]
            for _b in range(B):
                pt_t = ptable[_b][:, :].tensor
                ff_t = featfull[_b][:, :].tensor
                for dy in (0, 1):
                    yr = ([(0, 1, 0), (1, 64, 0), (65, 1, 63)] if dy == 0
                          else [(0, 64, 0), (64, 2, 63)])
                    for dx in (0, 1):
                        xr = ([(0, 1, 0), (1, 64, 0), (65, 1, 63)] if dx == 0
                              else [(0, 64, 0), (64, 2, 63)])
                        qoff = (dy * 2 + dx) * 64
                        for (jy0, ny, sy0) in yr:
                            for (jx0, nx, sx0) in xr:
                                dst = bass.AP(
                                    pt_t, (jy0 * 66 + jx0) * 256 + qoff,
                                    [(66 * 256, ny), (256, nx), (1, 64)])
                                src = bass.AP(
                                    ff_t, (sy0 * 64 + sx0) * 64,
                                    [(4096 if ny > 1 and sy0 == 0 else 0, ny),
                                     (64 if nx > 1 and sx0 == 0 else 0, nx),
                                     (1, 64)])
                                nc.sync.dma_start(out=dst, in_=src)

            # ---- constant weights to SBUF (from the gathered blob) ----
            # Wq.T/QK in rows 2t:2t+2 of per-corner slot t, zero elsewhere,
            # so the q matmul can take the full [8, .] rel tile as lhsT
            wq_sb = constp.tile([8, 4, 64], BF)
            bq_sb = constp.tile([1, 64], BF)
            wk_sb = constp.tile([128, 64], BF)   # Wk.T duplicated in both halves
            bk_sb = constp.tile([1, 64], BF)
            wv_sb = constp.tile([128, 64], BF)   # Wv.T duplicated in both halves
            bv_sb = constp.tile([128, 1], F32)
            w00o_sb = constp.tile([64, 4 * 256], BF)
            w00fp_sb = constp.tile([128, 2, 256], BF)
            w1_sb = constp.tile([128, 2, 256], BF)
            b1_sb = constp.tile([128, B, 2], F32)
            w2_sb = constp.tile([128, 2], BF)
            nc.vector.memset(wq_sb[:], 0.0)
            for _r in range(4):
                nc.sync.dma_start(out=wq_sb[2 * _r:2 * _r + 2, _r, :],
                                  in_=wfull[WOFF_WQ:WOFF_BQ])
            nc.sync.dma_start(out=bq_sb[:], in_=wfull[WOFF_BQ:WOFF_WK])
            nc.sync.dma_start(out=wk_sb[0:64, :], in_=wfull[WOFF_WK:WOFF_BK])
            nc.sync.dma_start(out=wk_sb[64:128, :], in_=wfull[WOFF_WK:WOFF_BK])
            nc.sync.dma_start(out=bk_sb[:], in_=wfull[WOFF_BK:WOFF_WV])
            nc.sync.dma_start(out=wv_sb[0:64, :], in_=wfull[WOFF_WV:WOFF_W00O])
            nc.sync.dma_start(out=wv_sb[64:128, :], in_=wfull[WOFF_WV:WOFF_W00O])
            nc.sync.dma_start(out=bv_sb[:], in_=bv[:, :])
            for t in range(4):
                nc.sync.dma_start(
                    out=w00o_sb[:, t * 256:(t + 1) * 256],
                    in_=wfull[WOFF_W00O + t * 16384:WOFF_W00O + (t + 1) * 16384])
            for kk in range(2):
                nc.sync.dma_start(
                    out=w00fp_sb[:, kk, :],
                    in_=wfull[WOFF_W00F + kk * 32768:WOFF_W00F + (kk + 1) * 32768])
                nc.sync.dma_start(
                    out=w1_sb[:, kk, :],
                    in_=wfull[WOFF_W1 + kk * 32768:WOFF_W1 + (kk + 1) * 32768])
                for _b in range(B):
                    nc.sync.dma_start(out=b1_sb[:, _b, kk:kk + 1],
                                      in_=b1[_b, kk, :, :])
                nc.sync.dma_start(
                    out=w2_sb[:, kk:kk + 1],
                    in_=wfull[WOFF_W2 + kk * 128:WOFF_W2 + (kk + 1) * 128])

            Sp_sb = constp.tile([64, NU * 64], F32)   # partial logits, all units

            # =========== phases 1+2 per batch: gather, fs, q/k, S ===========
            from concourse.masks import make_identity
            ident_sb = constp.tile([128, 128], BF)
            make_identity(nc, ident_sb[:])


            ones_nl = constp.tile([1, NLOC], BF)
            nc.vector.memset(ones_nl[:], 1.0)

            # Per batch: quarters of 16 pixel-tiles stream through gather ->
            # RBF multiply -> [128,128] pair transposes -> fs pair chunks,
            # and each quarter's q/k matmuls + logit accumulation run right
            # behind it so the PE overlaps the gather instead of idling.
            fsp_all = [[fsp.tile([128, NLOC], BF, name=f'fsp{_b}_{_p}')
                        for _p in range(2)] for _b in range(B)]
            for b in range(B):
                idx16_sb = gatp.tile([128, 64], I16)
                wsm_sb = gatp.tile([128, 4 * 64], BF)
                idx_sb = gatp.tile([128, 64], dt.int32)
                nc.sync.dma_start(out=idx16_sb[:], in_=idx[b, :, :])
                nc.sync.dma_start(out=wsm_sb[:], in_=wsm[b, :, :])
                nc.vector.tensor_copy(out=idx_sb[:], in_=idx16_sb[:])
                rel8_sb = relp.tile([8, NLOC], dt.int8, name='rel8')
                nc.sync.dma_start(out=rel8_sb[:], in_=relq[b, :, :])
                rel_sb = relp.tile([8, NLOC], BF)
                nc.vector.tensor_copy(out=rel_sb[:], in_=rel8_sb[:])

                s_all = psp.tile([64, 4 * 64], F32, name='s_all')
                for q in range(4):
                    g_pm = gatp.tile([128, 16, 4 * C], BF, name=f'g_pm{q % 2}')
                    for o in range(16):
                        nc.gpsimd.indirect_dma_start(
                            out=g_pm[:, o, :], out_offset=None,
                            in_=ptable[b][:, :],
                            in_offset=bass.IndirectOffsetOnAxis(
                                ap=idx_sb[:, q * 16 + o:q * 16 + o + 1], axis=0))
                    wap = wsm_sb[:, q * 64:(q + 1) * 64]
                    wbc = bass.AP(wap.tensor, wap.offset, wap.ap + [(0, C)])
                    nc.vector.tensor_tensor(out=g_pm[:, :, :],
                                            in0=g_pm[:, :, :], in1=wbc,
                                            op=ALU.mult)
                    for p in range(2):
                        for jg in range(4):
                            tp_ps = psp.tile([128, 512], BF, name='tp')
                            for jj in range(4):
                                jl = jg * 4 + jj
                                nc.tensor.transpose(
                                    out=tp_ps[:, jj * 128:(jj + 1) * 128],
                                    in_=g_pm[:, jl, p * 128:(p + 1) * 128],
                                    identity=ident_sb[:])
                            goff = (q * 16 + jg * 4) * 128
                            nc.scalar.copy(
                                out=fsp_all[b][p][:, goff:goff + 512],
                                in_=tp_ps[:])
                    # q/k + logit accumulation over this quarter's pixels
                    for t in range(4):
                        fpt = fsp_all[b][t // 2]
                        tb = (t % 2) * 64
                        for jg in range(2):      # 2 groups of 8 pixel-tiles
                            q_ps = psp.tile([128, 512], F32)
                            k_ps = psp.tile([128, 512], F32)
                            for jj in range(8):
                                j = q * 16 + jg * 8 + jj
                                csl = slice(j * 128, (j + 1) * 128)
                                osl = slice(jj * 64, (jj + 1) * 64)
                                nc.tensor.matmul(
                                    out=q_ps[:, osl],
                                    lhsT=rel_sb[:, csl],
                                    rhs=wq_sb[:, t, :],
                                    start=True, stop=not qk_bias)
                                nc.tensor.matmul(
                                    out=k_ps[:, osl], lhsT=fpt[tb:tb + 64, csl],
                                    rhs=wk_sb[tb:tb + 64, :],
                                    start=True, stop=not qk_bias)
                                if qk_bias:
                                    nc.tensor.matmul(
                                        out=q_ps[:, osl], lhsT=ones_nl[:, csl],
                                        rhs=bq_sb[:], start=False, stop=True)
                                    nc.tensor.matmul(
                                        out=k_ps[:, osl], lhsT=ones_nl[:, csl],
                                        rhs=bk_sb[:], start=False, stop=True)
                            qs_sb = qkp.tile([128, 512], BF, name=f'qs{jg % 2}')
                            ks_sb = qkp.tile([128, 512], BF, name=f'ks{jg % 2}')
                            nc.scalar.activation(out=qs_sb[:], in_=q_ps[:], func=AF.Relu)
                            nc.vector.tensor_scalar_max(out=ks_sb[:], in0=k_ps[:], scalar1=0.0)
                            for jj in range(8):
                                nc.tensor.matmul(
                                    out=s_all[:, t * 64:(t + 1) * 64],
                                    lhsT=qs_sb[:, jj * 64:(jj + 1) * 64],
                                    rhs=ks_sb[:, jj * 64:(jj + 1) * 64],
                                    start=(q == 0 and jg == 0 and jj == 0),
                                    stop=(q == 3 and jg == 1 and jj == 7))
                nc.vector.tensor_copy(out=Sp_sb[:, b * 256:(b + 1) * 256],
                                      in_=s_all[:])

            # =========== phase 3: AllReduce of logits ===========
            cc_in = dramp.tile([64, NU * 64], F32)
            cc_out = dramp.tile([64, NU * 64], F32)
            nc.gpsimd.dma_start(out=cc_in[:], in_=Sp_sb[:])
            nc.gpsimd.collective_compute(
                'AllReduce', mybir.AluOpType.add,
                replica_groups=[list(range(NCORES))],
                ins=[cc_in.opt()], outs=[cc_out.opt()],
            )
            S_sb = constp.tile([64, NU * 64], F32)
            nc.gpsimd.dma_start(out=S_sb[:], in_=cc_out[:])

            # =========== phase 4: softmax + A_t^T ===========
            attn_sb = constp.tile([64, NU * 64], BF)
            AT_tiles = []
            for u in range(NU):
                usl = slice(u * 64, (u + 1) * 64)
                mx = smallp.tile([64, 1], F32)
                nmx = smallp.tile([64, 1], F32)
                ex = smallp.tile([64, 64], F32)
                sm = smallp.tile([64, 1], F32)
                rs = smallp.tile([64, 1], F32)
                nc.vector.tensor_reduce(out=mx[:], in_=S_sb[:, usl],
                                        axis=mybir.AxisListType.X, op=ALU.max)
                nc.vector.tensor_scalar_mul(out=nmx[:], in0=mx[:], scalar1=-1.0)
                nc.scalar.activation(out=ex[:], in_=S_sb[:, usl], func=AF.Exp,
                                     bias=nmx[:, 0:1])
                nc.vector.tensor_reduce(out=sm[:], in_=ex[:],
                                        axis=mybir.AxisListType.X, op=ALU.add)
                nc.vector.reciprocal(out=rs[:], in_=sm[:])
                nc.vector.tensor_scalar_mul(out=attn_sb[:, usl], in0=ex[:],
                                            scalar1=rs[:, 0:1])
            for b in range(B):
                for p in range(2):
                    a_full = psp.tile([128, 512], F32, name='misc_ps')
                    for h in range(2):
                        t = 2 * p + h
                        u = b * 4 + t
                        nc.tensor.matmul(
                            out=a_full[h * 64:(h + 1) * 64, 0:256],
                            lhsT=attn_sb[:, u * 64:(u + 1) * 64],
                            rhs=w00o_sb[:, t * 256:(t + 1) * 256],
                            start=True, stop=True)
                    at = constp.tile([128, 256], BF, name=f'atp{b}_{p}')
                    nc.vector.tensor_copy(out=at[:], in_=a_full[:, 0:256])
                    AT_tiles.append(at)

            # =========== phase 5: MLP over resident fs pairs ===========
            for b in range(B):
                bil_sb = smallp.tile([1, NLOC], BF, name='bil_sb')
                nc.sync.dma_start(out=bil_sb[:], in_=bil[b, :][None, :])
                o_row = smallp.tile([1, NLOC], BF, name='o_row')

                for pc in range(NLOC // PCH):
                    # transient stacked v pair tiles for this pixel super-chunk
                    v_tiles = []
                    for p in range(2):
                        vt = vp.tile([128, PCH], BF, name=f'vt{p}_{pc % 2}')
                        for cc in range(PCH // CHUNK):
                            vsl_l = slice(cc * CHUNK, (cc + 1) * CHUNK)
                            vsl_g = slice(pc * PCH + cc * CHUNK, pc * PCH + (cc + 1) * CHUNK)
                            v_ps = psp.tile([128, CHUNK], F32)
                            nc.tensor.matmul(out=v_ps[0:64, :], lhsT=wv_sb[0:64, :],
                                             rhs=fsp_all[b][p][0:64, vsl_g],
                                             start=True, stop=True)
                            nc.tensor.matmul(out=v_ps[64:128, :], lhsT=wv_sb[64:128, :],
                                             rhs=fsp_all[b][p][64:128, vsl_g],
                                             start=True, stop=True)
                            nc.scalar.activation(out=vt[:, vsl_l], in_=v_ps[:],
                                                 func=AF.Relu, bias=bv_sb[:, 0:1])
                        v_tiles.append(vt)

                    x1_t = [mlpp.tile([128, PCH], BF, name=f'x1_{_m}_{pc % 2}')
                            for _m in range(2)]
                    x2_t = [mlpp.tile([128, PCH], BF, name=f'x2_{_m}_{pc % 2}')
                            for _m in range(2)]
                    for cc in range(PCH // CHUNK):
                        lsl = slice(cc * CHUNK, (cc + 1) * CHUNK)
                        gsl = slice(pc * PCH + cc * CHUNK, pc * PCH + (cc + 1) * CHUNK)
                        for m in range(2):
                            msl = slice(m * 128, (m + 1) * 128)
                            x_ps = psxp.tile([128, CHUNK], F32)
                            for p in range(2):
                                nc.tensor.matmul(
                                    out=x_ps[:],
                                    lhsT=w00fp_sb[:, p, msl],
                                    rhs=fsp_all[b][p][:, gsl],
                                    start=(p == 0), stop=False)
                            for p in range(2):
                                at = AT_tiles[b * 2 + p]
                                nc.tensor.matmul(
                                    out=x_ps[:],
                                    lhsT=at[:, msl],
                                    rhs=v_tiles[p][:, lsl],
                                    start=False, stop=(p == 1))
                            nc.vector.tensor_copy(out=x1_t[m][:, lsl], in_=x_ps[:])
                        # W1 + gelu
                        for m in range(2):
                            msl = slice(m * 128, (m + 1) * 128)
                            x2_ps = psxp.tile([128, CHUNK], F32)
                            for kk in range(2):
                                nc.tensor.matmul(out=x2_ps[:],
                                                 lhsT=w1_sb[:, kk, msl],
                                                 rhs=x1_t[kk][:, lsl],
                                                 start=(kk == 0), stop=(kk == 1))
                            nc.scalar.activation(out=x2_t[m][:, lsl], in_=x2_ps[:],
                                                 func=AF.Gelu, bias=b1_sb[:, b, m:m + 1])
                        # W2; bil add batched once per batch below
                        o_full = psp.tile([64, 512], F32, name='misc_ps')
                        o_ps = o_full[0:1, :]
                        for kk in range(2):
                            nc.tensor.matmul(out=o_ps, lhsT=w2_sb[:, kk:kk + 1],
                                             rhs=x2_t[kk][:, lsl],
                                             start=(kk == 0), stop=(kk == 1))
                        nc.scalar.copy(out=o_row[:, gsl], in_=o_ps)
                nc.vector.tensor_tensor(out=o_row[:], in0=o_row[:],
                                        in1=bil_sb[:], op=ALU.add)
                nc.sync.dma_start(out=out[b, :][None, :], in_=o_row[:])

    nc.compile()
    return nc


# --------------------------------------------------------------------------

def kernel(**inputs) -> np.ndarray:
    from concourse.bass_utils import run_bass_kernel_spmd
    in_maps = _host_prep(inputs)
    qk_bias = bool(np.any(np.asarray(inputs['bq']))
                   or np.any(np.asarray(inputs['bk'])))
    nc = _build(qk_bias)
    res = run_bass_kernel_spmd(nc, in_maps, core_ids=list(range(NCORES)))
    full = np.empty((B, 1, HQ, WQ), np.float32)
    flat = full.reshape(B, NPB)
    for cidx in range(NCORES):
        flat[:, cidx * NLOC:(cidx + 1) * NLOC] = \
            res.results[cidx]['out'].astype(np.float32)
    return full
